# revision 12
# baseline (speedup 1.0000x reference)
"""Trainium2 Bass kernel for nn_DeformableTransposedConv.

Pipeline (per the reference):
  up  = ConvTranspose2d(x, trans_w, stride=2, pad=1, outpad=1)   # [N,128,128,128]
  off = tanh(conv(relu(conv(lateral_feat, w1)), w2))             # [N,18,1,1] -> broadcast
  out = deform_conv2d(up, off, trans_w, pad=1)                   # [N,256,128,128]

Key structure exploited:
  * The offsets are constant over space (1x1 lateral input broadcast), so the
    bilinear deformable gather collapses to a per-batch 5x5 conv with
    "effective" weights W_eff[n] built host-side from trans_w and the (tiny)
    offsets.  The device computes:
        out[n] = sum_{dy,dx in 5x5} W_eff[n,dy,dx] @ shift(up[n], dy, dx)
    as PSUM-accumulated matmuls over the 128 up-channels.
  * The stride-2 transposed conv splits into 4 phase sub-convs with
    {1,2,2,4} taps, each a PSUM-accumulated matmul over the 256 x-channels.

Sharding: 8 cores = 2 batches x 4 row-strips of 32 output rows.  Each core
computes out[n, :, 32r:32r+32, :] from a 20-row slice of x (with halo).
All weights / layout prep / zero padding is done host-side; the NEFF is
input-independent (weights and data are ExternalInputs).
"""

import numpy as np
import ml_dtypes

import concourse.bass as bass
import concourse.tile as tile
from concourse import bacc, mybir
from concourse.bass_utils import run_bass_kernel_spmd

BF16 = ml_dtypes.bfloat16

# ---- problem constants (hardcoded per contract) ----
N_BATCH = 2
CIN = 256
COUT = 128          # up channels
K = 3
PAD = 1
H0 = W0 = 64        # x spatial
H = W = 128         # up / out spatial
N_CORES = 8
STRIPS = 4          # row strips per batch
OUT_R = 32          # output rows per strip

# SBUF layout constants
XR, XC = 20, 66     # x tile rows (16 + 2 halo each side), cols (64 + 1 pad + 1 align)
UR, UC = 36, 132    # up tile rows (32 + 2 halo each side), cols (128 + 2 + 2)
NCELL = 25          # 5x5 effective deform kernel
RBLK = 4            # output rows per stage-B block (4*128 = 512 = one PSUM bank)

# stage-B variant:
#   "full25" = static 5x5 effective conv (25 matmul terms / block)
#   "slots"  = dynamic cell slots (pruned zero cells, runtime offsets)
#   "hybrid" = DVE bilinear blends + matmuls (y-blend on DVE for all taps;
#              x-blend on DVE for the first HYBRID_S taps, folded into scaled
#              weights for the rest)
import os as _os
VARIANT = _os.environ.get("KERNEL_VARIANT", "wg")
WG_WARMUP = int(_os.environ.get("WG_WARMUP", "0"))   # p-state warmup matmuls
HYBRID_S = int(_os.environ.get("HYBRID_S", "4"))
SBR = 8             # hybrid blend superblock rows (2 PSUM blocks)
PLR = 32            # v2: ring-plane rows (blocks 0..7 read plane rows 4bi..4bi+3)
V2_VEC_PLANES = int(_os.environ.get("V2_VEC_PLANES", "1"))  # 0=scalar,1=split,2=vector
V2_MIXED_GROUP = _os.environ.get("V2_MIXED_GROUP", "1") == "1"
V2_OUT = _os.environ.get("V2_OUT", "f16")
V2_NO_EVAC = _os.environ.get("V2_NO_EVAC", "0") == "1"   # timing probe only
V2_NO_PLANES = _os.environ.get("V2_NO_PLANES", "0") == "1"  # timing probe only
V2_NO_ODMA = _os.environ.get("V2_NO_ODMA", "0") == "1"      # timing probe only
V2_EVAC_ENG = _os.environ.get("V2_EVAC_ENG", "vector")

_CACHED_NC = {}


# --------------------------------------------------------------------------
# host-side preparation
# --------------------------------------------------------------------------

def _offsets_from_inputs(lateral_feat, off_w1, off_b1, off_w2, off_b2):
    """Tiny offset MLP (conv on 1x1 spatial input == center-tap matmul)."""
    lf = lateral_feat[:, :, 0, 0].astype(np.float32)                    # [N,128]
    h = np.maximum(0.0, lf @ off_w1[:, :, 1, 1].T.astype(np.float32)
                   + off_b1.astype(np.float32))                         # [N,64]
    off = np.tanh(h @ off_w2[:, :, 1, 1].T.astype(np.float32)
                  + off_b2.astype(np.float32)).astype(np.float32)       # [N,18]
    oy = off.reshape(-1, K * K, 2)[:, :, 0]
    ox = off.reshape(-1, K * K, 2)[:, :, 1]
    return oy, ox


def _w_eff(trans_w, oy, ox):
    """Effective 5x5 deform weights. Returns [N, 5, 5, 256(o), 128(c)] f32."""
    n_b = oy.shape[0]
    Weff = np.zeros((n_b, 5, 5, CIN, COUT), np.float32)
    for n in range(n_b):
        for k in range(K * K):
            ky, kx = k // K, k % K
            ay = np.float32(ky - 1) + oy[n, k]
            ax = np.float32(kx - 1) + ox[n, k]
            Ay, Ax = int(np.floor(ay)), int(np.floor(ax))
            dy = float(ay) - Ay
            dx = float(ax) - Ax
            tap = trans_w[:, :, ky, kx].astype(np.float32)
            for cy, wy in ((0, 1.0 - dy), (1, dy)):
                for cx, wx in ((0, 1.0 - dx), (1, dx)):
                    w = wy * wx
                    if w != 0.0:
                        Weff[n, Ay + cy + 2, Ax + cx + 2] += w * tap
    return Weff


def _prep_in_maps(x, trans_w, oy, ox):
    """Build the per-core input dicts (already bf16, padded, SBUF-layouts).
    Returns (in_maps, ncell) where ncell is the stage-B slot count."""
    xf = x.astype(np.float32)

    # stage-A weights, shared by all cores: wa[k, h2, j, m]
    wa = np.zeros((COUT, 2, 9, COUT), np.float32)
    for h2 in range(2):
        for j in range(9):
            jy, jx = j // 3, j % 3
            # lhsT[K=cin(128), M=cout(128)] = trans_w[h2*128+kk, m, jy, jx]
            wa[:, h2, j, :] = trans_w[h2 * 128:(h2 + 1) * 128, :, jy, jx]
    wa_b = wa.astype(BF16).reshape(COUT, 2 * 9 * COUT)

    if VARIANT == "hybrid":
        return _prep_in_maps_hybrid(xf, trans_w, oy, ox, wa_b)
    if VARIANT == "fp8r":
        return _prep_in_maps_fp8r(xf, trans_w, oy, ox, wa_b)
    if VARIANT == "wg":
        r = _prep_in_maps_wg(xf, trans_w, oy, ox)
        if r is not None:
            return r
        return _prep_in_maps_v2(xf, trans_w, oy, ox, wa_b)
    if VARIANT == "v2":
        return _prep_in_maps_v2(xf, trans_w, oy, ox, wa_b)

    # stage-B weights per batch
    Weff = _w_eff(trans_w, oy, ox)                      # [N,5,5,256,128]
    wb_all, co_all = [], []
    if VARIANT == "full25":
        ncell = NCELL
        for n in range(N_BATCH):
            wb = Weff[n].reshape(NCELL, 2, COUT, COUT)   # [cell, half, o(128), c]
            wb = wb.transpose(3, 0, 1, 2)                # [c, cell, half, o]
            wb_all.append(np.ascontiguousarray(wb).astype(BF16)
                          .reshape(COUT, NCELL * 2 * COUT))
            co_all.append(None)
    elif VARIANT == "union":
        # static program specialized on the union of nonzero cells across
        # batches (compile cache keyed on the union tuple)
        nz = [np.nonzero(np.abs(Weff[n]).reshape(25, -1).max(1) > 0)[0]
              for n in range(N_BATCH)]
        union = sorted(set(int(c) for z in nz for c in z))
        ncell = ("union",) + tuple(union)
        for n in range(N_BATCH):
            wb = np.zeros((len(union), 2, COUT, COUT), np.float32)
            for s, ci in enumerate(union):
                wb[s] = Weff[n, ci // 5, ci % 5].reshape(2, COUT, COUT)
            wb = wb.transpose(3, 0, 1, 2)
            wb_all.append(np.ascontiguousarray(wb).astype(BF16)
                          .reshape(COUT, len(union) * 2 * COUT))
            co_all.append(None)
    else:  # "slots": pruned nonzero cells, offsets shipped as data
        nz = [np.nonzero(np.abs(Weff[n]).reshape(25, -1).max(1) > 0)[0]
              for n in range(N_BATCH)]
        ncell = max(len(z) for z in nz)
        for n in range(N_BATCH):
            cells = list(nz[n]) + [12] * (ncell - len(nz[n]))  # pad w/ center
            wb = np.zeros((ncell, 2, COUT, COUT), np.float32)
            co = np.zeros((1, ncell, 2), np.int32)
            for s, ci in enumerate(cells):
                dyi, dxi = ci // 5, ci % 5
                if s < len(nz[n]):
                    wb[s] = Weff[n, dyi, dxi].reshape(2, COUT, COUT)
                co[0, s] = (dyi, dxi)
            wb = wb.transpose(3, 0, 1, 2)                # [c, slot, half, o]
            wb_all.append(np.ascontiguousarray(wb).astype(BF16)
                          .reshape(COUT, ncell * 2 * COUT))
            co_all.append(co)

    in_maps = []
    for core in range(N_CORES):
        n, r = core // STRIPS, core % STRIPS
        # x slice with halo: global x rows [16r-2, 16r+18)
        xs = np.zeros((COUT, 2, XR, XC), np.float32)
        r0 = 16 * r - 2
        lo, hi = max(0, r0), min(H0, r0 + XR)
        for h2 in range(2):
            xs[:, h2, lo - r0:hi - r0, :W0] = xf[n, h2 * 128:(h2 + 1) * 128, lo:hi, :]
        # bottom-halo validity mask: strip 0 must zero up rows g=-2,-1 which
        # the phase formula would otherwise fill with spurious values
        mk = np.full((COUT, 1), 0.0 if r == 0 else 1.0, np.float32)
        im = {
            "xs": np.ascontiguousarray(xs.astype(BF16).reshape(COUT, 2 * XR * XC)),
            "wa": wa_b,
            "wb": wb_all[n],
            "mk": mk,
        }
        if co_all[n] is not None:
            im["co"] = co_all[n]
        in_maps.append(im)
    return in_maps, ncell


FP8 = ml_dtypes.float8_e4m3
RING_SCALE = 256.0


def _prep_in_maps_fp8r(xf, trans_w, oy, ox, wa_b):
    """Union cells; big cells in bf16, small 'ring' cells paired into fp8
    DoubleRow matmuls (weights scaled by RING_SCALE)."""
    Weff = _w_eff(trans_w, oy, ox)                       # [N,5,5,256,128]
    norms = np.abs(Weff).reshape(N_BATCH, 25, -1).max(2)  # [N,25]
    union = sorted(set(np.nonzero(norms.max(0) > 0)[0].tolist()))
    thr = 0.25 * norms.max()
    bigs = [c for c in union if norms[:, c].max() > thr]
    rings = [c for c in union if c not in bigs]
    if len(rings) % 2:
        bigs.append(rings.pop())                          # odd leftover -> bf16
    # order by window offset (dx major, dy minor); pair far-apart cells so the
    # two DoubleRow K-group windows never overlap (overlapping windows were
    # measured ~1.7x slower on the PE)
    rings.sort(key=lambda c: (c % 5, c // 5))
    nh = len(rings) // 2
    pairs = [(rings[i], rings[i + nh]) for i in range(nh)]
    bigs = sorted(bigs)

    wb_all, wr_all = [], []
    for n in range(N_BATCH):
        wb = np.zeros((max(len(bigs), 1), 2, COUT, COUT), np.float32)
        for s, ci in enumerate(bigs):
            wb[s] = Weff[n, ci // 5, ci % 5].reshape(2, COUT, COUT)
        wb = wb.transpose(3, 0, 1, 2)                     # [c, slot, half, o]
        wb_all.append(np.ascontiguousarray(wb).astype(BF16)
                      .reshape(COUT, -1))
        wr = np.zeros((max(len(pairs), 1), 2, 2, COUT, COUT), np.float32)
        for p, (c1, c2) in enumerate(pairs):
            for half in range(2):
                wr[p, half, 0] = RING_SCALE * \
                    Weff[n, c1 // 5, c1 % 5][128 * half:128 * (half + 1)].T
                wr[p, half, 1] = RING_SCALE * \
                    Weff[n, c2 // 5, c2 % 5][128 * half:128 * (half + 1)].T
        # wr[p, half, ksub, c, o] -> [c, p, half, ksub, o]
        wr = wr.transpose(3, 0, 1, 2, 4)
        wr_all.append(np.ascontiguousarray(wr).astype(FP8).reshape(COUT, -1))

    in_maps = []
    for core in range(N_CORES):
        n, r = core // STRIPS, core % STRIPS
        xs = np.zeros((COUT, 2, XR, XC), np.float32)
        r0 = 16 * r - 2
        lo, hi = max(0, r0), min(H0, r0 + XR)
        for h2 in range(2):
            xs[:, h2, lo - r0:hi - r0, :W0] = xf[n, h2 * 128:(h2 + 1) * 128, lo:hi, :]
        mk = np.full((COUT, 1), 0.0 if r == 0 else 1.0, np.float32)
        in_maps.append({
            "xs": np.ascontiguousarray(xs.astype(BF16).reshape(COUT, 2 * XR * XC)),
            "wa": wa_b,
            "wb": wb_all[n],
            "wr": wr_all[n],
            "mk": mk,
        })
    return in_maps, ("fp8r", tuple(bigs), tuple(pairs))


RING_W_SCALE = 16.0     # ring weights x16, up fp8 copies x1/16 -> product x1
TAP_ORDER = (4, 3, 5, 1, 7, 0, 2, 6, 8)   # phase-major: p00|p01|p10|p11
TAP_POS = {j: i for i, j in enumerate(TAP_ORDER)}
WA_CUTS = (0, 1, 3, 5, 9)                 # DMA piece boundaries in TAP_ORDER
XS_R0 = (0, 7, 13)                        # first xs row held by each band tile
PRUNE_BUDGET = float(_os.environ.get("V2_PRUNE_BUDGET", "0.012"))


def _prune_rings(Weff, xf, trans_w, bigs, rings):
    """Exact-error greedy pruning: for each ring cell (ascending magnitude)
    try dropping it or folding its weights into an adjacent kept cell; accept
    while the accumulated absmax output error stays under PRUNE_BUDGET.
    Returns (rings_kept, Weff_adjusted, err_bound)."""
    N, H2 = N_BATCH, H
    # host up[n]: transposed conv, padded by 2 on each side for cell shifts
    upp = np.zeros((N, COUT, H2 + 4, W + 4), np.float32)
    for n in range(N):
        # up[m, g, h] = sum_{jy,jx,c} w[c,m,jy,jx] x[c,(g+1-jy)/2,(h+1-jx)/2]
        for jy in range(3):
            for jx in range(3):
                w = trans_w[:, :, jy, jx].astype(np.float32)      # [c, m]
                # valid g: g+1-jy even and 0 <= (g+1-jy)//2 < 64
                gs = np.arange(H2)
                ok_g = ((gs + 1 - jy) % 2 == 0) & ((gs + 1 - jy) // 2 >= 0) \
                    & ((gs + 1 - jy) // 2 < H0)
                hs = np.arange(W)
                ok_h = ((hs + 1 - jx) % 2 == 0) & ((hs + 1 - jx) // 2 >= 0) \
                    & ((hs + 1 - jx) // 2 < W0)
                gi = (gs[ok_g] + 1 - jy) // 2
                hi = (hs[ok_h] + 1 - jx) // 2
                contrib = (w.T @ np.ascontiguousarray(
                    xf[n][:, gi][:, :, hi]).reshape(CIN, -1)).reshape(
                        COUT, len(gi), len(hi))
                gg, hh = np.ix_(gs[ok_g] + 2, hs[ok_h] + 2)
                upp_n = upp[n]
                upp_n[:, gg, hh] += contrib
    def cell_out(n, Wc, ci):
        dy, dx = ci // 5, ci % 5
        win = np.ascontiguousarray(upp[n, :, dy:dy + H2, dx:dx + W])
        return (Wc @ win.reshape(COUT, -1)).reshape(CIN, H2, W)
    # full-output scale
    scale = 0.0
    for n in range(N):
        acc = None
        for ci in set(bigs) | set(rings):
            t = cell_out(n, Weff[n, ci // 5, ci % 5].astype(np.float32), ci)
            acc = t if acc is None else acc + t
        scale = max(scale, np.abs(acc).max())
    Weff = Weff.copy()
    kept = list(rings)
    diff = [np.zeros((CIN, H2, W), np.float32) for _ in range(N)]
    order = sorted(rings, key=lambda c: float(
        np.abs(Weff[:, c // 5, c % 5]).max()))
    err = 0.0
    for ci in order:
        dy, dx = ci // 5, ci % 5
        others = [c2 for c2 in (set(bigs) | set(kept)) if c2 != ci]
        others.sort(key=lambda c2: abs(c2 // 5 - dy) + abs(c2 % 5 - dx))
        basis_cells = others[:3]
        # per-batch: least-squares fold of this cell onto the basis cells
        cand = []
        for n in range(N):
            Wc = Weff[n, dy, dx].astype(np.float32)
            if not np.any(Wc):
                cand.append((diff[n], []))
                continue
            r = cell_out(n, Wc, ci).ravel()
            B = np.stack([cell_out(n, Wc, c2).ravel() for c2 in basis_cells])
            G = B @ B.T
            b = B @ r
            try:
                al = np.linalg.solve(G + 1e-12 * np.eye(len(B)), b)
            except np.linalg.LinAlgError:
                al = np.zeros(len(B))
            resid = r - al @ B
            cand.append(((diff[n].ravel() + resid).reshape(CIN, H2, W),
                         list(zip(basis_cells, al))))
        e = max(np.abs(c[0]).max() for c in cand) / scale
        if e <= PRUNE_BUDGET:
            err = e
            for n in range(N):
                diff[n] = cand[n][0]
                Wc = Weff[n, dy, dx].copy()
                for c2, a in cand[n][1]:
                    Weff[n, c2 // 5, c2 % 5] += np.float32(a) * Wc
            Weff[:, dy, dx] = 0.0
            kept.remove(ci)
        else:
            break
    return kept, Weff, err



def _prep_in_maps_v2(xf, trans_w, oy, ox, wa_b):
    """v3: static big cells + static union ring cells, both accumulated into
    ONE psum bank per output block.

    Ring cells (bilinear spill corners) are paired into fp8 DoubleRow
    matmuls over per-dx margin-free fp8 copies of up.  Ring weights are
    scaled x16 and the fp8 copies x1/16, so the pair product is unscaled
    and rings accumulate into the SAME psum bank as the big cells (no
    separate merge pass).  Cells whose max-norm is below 0.4% of the
    global max (the ab bilinear corners, ~1e-4 relative) are dropped
    (~0.1% output error)."""
    Weff = _w_eff(trans_w, oy, ox)                        # [N,5,5,256,128]
    norms = np.abs(Weff).reshape(N_BATCH, 25, -1).max(2)  # [N,25]
    gmax = norms.max()
    bigs = sorted(int(c) for c in np.nonzero(norms.max(0) > 0.25 * gmax)[0])
    keep = (norms.max(0) > 0.004 * gmax) & (norms.max(0) <= 0.25 * gmax)
    rings = [int(c) for c in np.nonzero(keep)[0] if c not in bigs]
    if PRUNE_BUDGET > 0 and rings:
        rings, Weff, _perr = _prune_rings(Weff, xf, trans_w, bigs, rings)
    # order by (dx major, dy minor) and pair far apart so the two DoubleRow
    # K-group windows never overlap
    rings.sort(key=lambda c: (c % 5, c // 5))
    if len(rings) % 2:
        # pad slot: any distinct cell position (zero weights, contributes 0);
        # prefer one that reuses an already-needed dx plane
        dxs = {c % 5 for c in rings}
        pad = next((c for c in range(25) if c not in rings and c % 5 in dxs),
                   next(c for c in range(25) if c not in rings))
        rings.append(pad)
        rings.sort(key=lambda c: (c % 5, c // 5))
    nh = len(rings) // 2
    pairs = [(rings[i], rings[i + nh]) for i in range(nh)]
    need_dx = sorted({c % 5 for c in rings})
    dx_slot = {d: i for i, d in enumerate(need_dx)}
    nbig = len(bigs)

    def cell_off(c, bi):
        return dx_slot[c % 5] * (UR * W) + (4 * bi + c // 5) * W

    # validate pair steps (static, positive, 16-aligned)
    for c1, c2 in pairs:
        step = cell_off(c2, 0) - cell_off(c1, 0)
        assert step > 0 and step % 16 == 0, (c1, c2, step)

    # stage-A weights in phase-major tap order for split DMA
    wa2 = np.zeros((COUT, 9, 2, COUT), np.float32)
    for j, pos in TAP_POS.items():
        jy, jx = j // 3, j % 3
        for h2 in range(2):
            wa2[:, pos, h2, :] = trans_w[h2 * 128:(h2 + 1) * 128, :, jy, jx]
    wa_b = np.ascontiguousarray(wa2).astype(BF16).reshape(COUT, 2 * 9 * COUT)

    wb_all, wr_all = [], []
    for n in range(N_BATCH):
        wb = np.zeros((2, nbig, COUT, COUT), np.float32)  # [half, s, o, c]
        for s, ci in enumerate(bigs):
            wb[:, s] = Weff[n, ci // 5, ci % 5].reshape(2, COUT, COUT)
        wb = wb.transpose(3, 0, 1, 2)                     # [c, half, s, o]
        wb_all.append(np.ascontiguousarray(wb).astype(BF16).reshape(COUT, -1))
        wr = np.zeros((max(len(pairs), 1), 2, 2, COUT, COUT), np.float32)
        for p, (c1, c2) in enumerate(pairs):
            for half in range(2):
                wr[p, half, 0] = RING_W_SCALE * \
                    Weff[n, c1 // 5, c1 % 5][128 * half:128 * (half + 1)].T
                wr[p, half, 1] = RING_W_SCALE * \
                    Weff[n, c2 // 5, c2 % 5][128 * half:128 * (half + 1)].T
        wr = wr.transpose(3, 0, 1, 2, 4)                  # [c, p, half, ksub, o]
        wr_all.append(np.ascontiguousarray(wr).astype(FP8).reshape(COUT, -1))

    in_maps = []
    for core in range(N_CORES):
        n, r = core // STRIPS, core % STRIPS
        xs = np.zeros((COUT, 2, XR, XC), np.float32)
        r0 = 16 * r - 2
        lo, hi = max(0, r0), min(H0, r0 + XR)
        for h2 in range(2):
            xs[:, h2, lo - r0:hi - r0, :W0] = xf[n, h2 * 128:(h2 + 1) * 128, lo:hi, :]
        # banded copy: band b holds xs rows XS_R0[b] .. XS_R0[b]+7, so each
        # stage-A band reads its own tile while the next band's DMA lands
        xsb = np.zeros((COUT, 3, 2, 8, XC), np.float32)
        for b, rb in enumerate(XS_R0):
            nr = min(8, XR - rb)
            xsb[:, b, :, :nr, :] = xs[:, :, rb:rb + nr, :]
        mk = np.full((COUT, 1), 0.0 if r == 0 else 1.0, np.float32)
        in_maps.append({
            "xs": np.ascontiguousarray(xsb.astype(BF16)
                                       .reshape(COUT, 3 * 2 * 8 * XC)),
            "wa": wa_b,
            "wb": wb_all[n],
            "wr": wr_all[n],
            "mk": mk,
        })
    return in_maps, ("v2", tuple(bigs), tuple(pairs), tuple(need_dx))


WG_CENTER = (6, 7, 8, 11, 12, 13, 16, 17, 18)


def _prep_in_maps_wg(xf, trans_w, oy, ox):
    """Winograd variant: valid when all effective cells fold into the center
    3x3 (tiny offsets).  Stage B = F(2,3) along rows: 12 half-size matmuls
    per block instead of 9 full-size.  Returns None if structure doesn't fit
    (caller falls back to v2)."""
    Weff = _w_eff(trans_w, oy, ox)
    norms = np.abs(Weff).reshape(N_BATCH, 25, -1).max(2)
    gmax = norms.max()
    bigs = sorted(int(c) for c in np.nonzero(norms.max(0) > 0.25 * gmax)[0])
    if not set(bigs) <= set(WG_CENTER):
        return None
    keep = (norms.max(0) > 0.004 * gmax) & (norms.max(0) <= 0.25 * gmax)
    rings = [int(c) for c in np.nonzero(keep)[0] if c not in bigs]
    if rings:
        if PRUNE_BUDGET <= 0:
            return None
        rings, Weff, _perr = _prune_rings(Weff, xf, trans_w, bigs, rings)
        if rings:
            return None

    # stage-A weights in phase-major tap order for split DMA (same as v2)
    wa2 = np.zeros((COUT, 9, 2, COUT), np.float32)
    for j, pos in TAP_POS.items():
        jy, jx = j // 3, j % 3
        for h2 in range(2):
            wa2[:, pos, h2, :] = trans_w[h2 * 128:(h2 + 1) * 128, :, jy, jx]
    wa_b = np.ascontiguousarray(wa2).astype(BF16).reshape(COUT, 2 * 9 * COUT)

    # Winograd-transformed stage-B weights: wg[c, half, k(4), kx(3), o]
    # k0 = w_dy0, k1 = (w0+w1+w2)/2, k2 = (w0-w1+w2)/2, k3 = w_dy2
    # (half-major so each half's weights ship as one contiguous DMA piece)
    wg_all = []
    for n in range(N_BATCH):
        wgl = np.zeros((COUT, 2, 4, 3, COUT), np.float32)
        for kx in range(3):
            w0 = Weff[n, 1, 1 + kx]     # [o(256), c(128)], shift dy=-1
            w1 = Weff[n, 2, 1 + kx]
            w2 = Weff[n, 3, 1 + kx]
            for k, wt in enumerate((w0, (w0 + w1 + w2) * 0.5,
                                    (w0 - w1 + w2) * 0.5, w2)):
                for half in range(2):
                    wgl[:, half, k, kx, :] = wt[128 * half:128 * (half + 1), :].T
        wg_all.append(np.ascontiguousarray(wgl).astype(BF16).reshape(COUT, -1))

    in_maps = []
    for core in range(N_CORES):
        n, r = core // STRIPS, core % STRIPS
        xs = np.zeros((COUT, 2, XR, XC), np.float32)
        r0 = 16 * r - 2
        lo, hi = max(0, r0), min(H0, r0 + XR)
        for h2 in range(2):
            xs[:, h2, lo - r0:hi - r0, :W0] = xf[n, h2 * 128:(h2 + 1) * 128, lo:hi, :]
        xsb = np.zeros((COUT, 3, 2, 8, XC), np.float32)
        for b, rb in enumerate(XS_R0):
            nr = min(8, XR - rb)
            xsb[:, b, :, :nr, :] = xs[:, :, rb:rb + nr, :]
        mk = np.full((COUT, 1), 0.0 if r == 0 else 1.0, np.float32)
        in_maps.append({
            "xs": np.ascontiguousarray(xsb.astype(BF16)
                                       .reshape(COUT, 3 * 2 * 8 * XC)),
            "wa": wa_b,
            "wg": wg_all[n],
            "mk": mk,
        })
    return in_maps, ("wg",)


def _prep_in_maps_hybrid(xf, trans_w, oy, ox, wa_b):
    S = HYBRID_S
    nslot = S + 2 * (9 - S)
    wb_all, bs_all, dsc_all, ofs_all = [], [], [], []
    for n in range(N_BATCH):
        wb = np.zeros((nslot, 2, COUT, COUT), np.float32)   # [slot, half, c, o]
        bs = np.zeros((9, 2), np.float32)
        dsc = np.zeros((max(S, 1), 2), np.float32)
        ofs = np.zeros((1, 9, 2), np.int32)
        for k in range(9):
            ky, kx = k // 3, k % 3
            ay = np.float32(ky - 1) + oy[n, k]
            ax = np.float32(kx - 1) + ox[n, k]
            Ay, Ax = int(np.floor(ay)), int(np.floor(ax))
            dy = float(ay) - Ay
            dx = float(ax) - Ax
            ofs[0, k] = (2 + Ay, 2 + Ax)
            bs[k] = (1.0 - dy, dy)
            wkT = np.stack([trans_w[h * 128:(h + 1) * 128, :, ky, kx].T
                            for h in range(2)])             # [half, c, o]
            if k < S:
                dsc[k] = (1.0 - dx, dx)
                wb[k] = wkT
            else:
                wb[S + 2 * (k - S) + 0] = (1.0 - dx) * wkT
                wb[S + 2 * (k - S) + 1] = dx * wkT
        wb = wb.transpose(2, 0, 1, 3)                       # [c, slot, half, o]
        wb_all.append(np.ascontiguousarray(wb).astype(BF16)
                      .reshape(COUT, nslot * 2 * COUT))
        bs_all.append(np.broadcast_to(bs.reshape(1, 9, 2),
                                      (COUT, 9, 2)).copy())
        dsc_all.append(np.broadcast_to(dsc.reshape(1, -1, 2),
                                       (COUT, max(S, 1), 2)).copy())
        ofs_all.append(ofs)

    in_maps = []
    for core in range(N_CORES):
        n, r = core // STRIPS, core % STRIPS
        xs = np.zeros((COUT, 2, XR, XC), np.float32)
        r0 = 16 * r - 2
        lo, hi = max(0, r0), min(H0, r0 + XR)
        for h2 in range(2):
            xs[:, h2, lo - r0:hi - r0, :W0] = xf[n, h2 * 128:(h2 + 1) * 128, lo:hi, :]
        mk = np.full((COUT, 1), 0.0 if r == 0 else 1.0, np.float32)
        in_maps.append({
            "xs": np.ascontiguousarray(xs.astype(BF16).reshape(COUT, 2 * XR * XC)),
            "wa": wa_b,
            "wb": wb_all[n],
            "mk": mk,
            "bs": bs_all[n].reshape(COUT, 18),
            "dsc": dsc_all[n].reshape(COUT, -1),
            "co": ofs_all[n],
        })
    return in_maps, nslot


# --------------------------------------------------------------------------
# device program (input-independent; same for all cores except r-dependent
# row validity -> handled by *uniform* structure: we compute all 36 up rows,
# rows outside [0,128) stay zero because their x inputs are zeroed host-side
# ... except parity bookkeeping differs per strip; we keep the program truly
# SPMD by computing the full 18 a'-rows per phase and masking via zero x.)
# --------------------------------------------------------------------------

def _build_nc_v2(key):
    """v3 device program: interleaved stage A bands / stage B block groups,
    static big + ring cells unified into one psum bank, fp16 output."""
    _, bigs, pairs, need_dx = key
    bigs, pairs, need_dx = list(bigs), list(pairs), list(need_dx)
    nbig, npair, ndx = len(bigs), len(pairs), len(need_dx)
    dx_slot = {d: i for i, d in enumerate(need_dx)}
    nc = bacc.Bacc("TRN2", target_bir_lowering=False, debug=False,
                   enable_asserts=False)

    xs_d = nc.dram_tensor("xs", [COUT, 3 * 2 * 8 * XC], mybir.dt.bfloat16,
                          kind="ExternalInput").ap()
    wa_d = nc.dram_tensor("wa", [COUT, 2 * 9 * COUT], mybir.dt.bfloat16,
                          kind="ExternalInput").ap()
    wb_d = nc.dram_tensor("wb", [COUT, 2 * nbig * COUT], mybir.dt.bfloat16,
                          kind="ExternalInput").ap()
    wr_d = nc.dram_tensor("wr", [COUT, max(npair, 1) * 2 * 2 * COUT],
                          mybir.dt.float8e4, kind="ExternalInput").ap()
    mk_d = nc.dram_tensor("mk", [COUT, 1], mybir.dt.float32,
                          kind="ExternalInput").ap()
    out_dt = {"f16": mybir.dt.float16, "bf16": mybir.dt.bfloat16,
              "f32": mybir.dt.float32}[V2_OUT]
    out_d = nc.dram_tensor("out", [CIN, OUT_R, W], out_dt,
                           kind="ExternalOutput").ap()

    with tile.TileContext(nc) as tc:
        with (
            tc.tile_pool(name="singles", bufs=1) as singles,
            tc.tile_pool(name="outp", bufs=4) as outp,
            tc.tile_pool(name="psA", bufs=4, space="PSUM") as psA,
            tc.tile_pool(name="psB", bufs=4, space="PSUM") as psB,
        ):
            xs_t = singles.tile([COUT, 3, 2, 8, XC], mybir.dt.bfloat16)
            wa_t = singles.tile([COUT, 9, 2, COUT], mybir.dt.bfloat16)
            wb_t = singles.tile([COUT, 2, nbig, COUT], mybir.dt.bfloat16)
            wr_t = singles.tile([COUT, max(npair, 1), 2, 2, COUT],
                                mybir.dt.float8e4)
            mk_t = singles.tile([COUT, 1], mybir.dt.float32)
            up_full = singles.tile([COUT, UR * UC + 12], mybir.dt.bfloat16)
            up_t = up_full[:, :UR * UC]
            upf_t = singles.tile([COUT, max(ndx, 1), UR, W], mybir.dt.float8e4)

            # ---- input DMA, ordered so the first stage-A matmuls (half 0,
            # band 0) can start as early as possible ----
            xs4 = xs_t[:]
            xs4_d = xs_d.rearrange("p (a b c d) -> p a b c d", a=3, b=2, c=8)
            wa_flat = wa_t[:].rearrange("p a b c -> p (a b c)")
            wb_flat = wb_t[:].rearrange("p a b c -> p (a b c)")
            # the three pieces gating the first matmuls go first, split
            # across both HWDGE queues (one trigger each ~0.7us):
            #   sync:   wa piece1 (tap j4, both halves), xs h1 band0, wa rest
            #   scalar: xs h0 band0, xs h0 rest, wb h0, xs h1 rest, wb h1
            nc.sync.dma_start(out=wa_flat[:, :WA_CUTS[1] * 2 * COUT],
                              in_=wa_d[:, :WA_CUTS[1] * 2 * COUT])
            nc.scalar.dma_start(out=xs4[:, 0], in_=xs4_d[:, 0])
            for c0, c1 in zip(WA_CUTS[1:-1], WA_CUTS[2:]):
                nc.sync.dma_start(out=wa_flat[:, c0 * 2 * COUT:c1 * 2 * COUT],
                                  in_=wa_d[:, c0 * 2 * COUT:c1 * 2 * COUT])
            nc.scalar.dma_start(out=xs4[:, 1], in_=xs4_d[:, 1])
            nc.scalar.dma_start(out=xs4[:, 2], in_=xs4_d[:, 2])
            nc.scalar.dma_start(out=wb_flat[:, :nbig * COUT],
                                in_=wb_d[:, :nbig * COUT])
            nc.scalar.dma_start(out=wb_flat[:, nbig * COUT:],
                                in_=wb_d[:, nbig * COUT:])
            if npair:
                nc.sync.dma_start(
                    out=wr_t[:].rearrange("p a b c d -> p (a b c d)"), in_=wr_d)
            nc.sync.dma_start(out=mk_t[:], in_=mk_d)

            up_w = up_t.rearrange("p (a q c r) -> p a q c r", q=2, c=66, r=2)
            up_r = up_t.rearrange("p (l c) -> p l c", c=132)
            # only the column margins and the tail pad are never written by
            # the stage-A scatter -- memset just those
            nc.vector.memset(up_r[:, :, 0:2], 0.0)
            nc.vector.memset(up_r[:, :, 130:132], 0.0)
            nc.vector.memset(up_full[:, UR * UC:], 0.0)
            upf_fl = upf_t[:].rearrange("p a b c -> p (a b c)")

            ytaps = {0: ((1, 0),), 1: ((2, 0), (0, 1))}
            band_blocks = ((0, 1), (2, 3, 4), (5, 6, 7))

            def cell_off(c, bi):
                return dx_slot[c % 5] * (UR * W) + (4 * bi + c // 5) * W

            def emit_phase(b, py, px):
                a0 = 6 * b
                rc = 6
                taps = [(jy, dy, jx, dx)
                        for jy, dy in ytaps[py] for jx, dx in ytaps[px]]
                ps = psA.tile([COUT, rc, 64], mybir.dt.float32, tag="psA",
                              name=f"psA_{b}_{py}_{px}")
                nmm = len(taps) * 2
                i = 0
                for h2 in range(2):
                    for (jy, dy, jx, dx) in taps:
                        r0x = a0 + 1 + dy - XS_R0[b]
                        nc.tensor.matmul(
                            ps[:, :rc, :],
                            lhsT=wa_t[:, TAP_POS[jy * 3 + jx], h2, :],
                            rhs=xs_t[:, b, h2, r0x:r0x + rc, dx:dx + 64],
                            start=(i == 0), stop=(i == nmm - 1),
                        )
                        i += 1
                nc.scalar.copy(
                    out=up_w[:, a0:a0 + rc, py, 1:65, px],
                    in_=ps[:, :rc, :],
                )

            def emit_block(bi):
                for half in range(2):
                    ps = psB.tile([COUT, RBLK, W], mybir.dt.float32,
                                  tag="psB", name=f"psB_{bi}_{half}")
                    mixed = V2_MIXED_GROUP and npair > 0
                    nmm = nbig + (npair if mixed else 0)
                    for s, ci in enumerate(bigs):
                        dyi, dxi = ci // 5, ci % 5
                        ys = 4 * bi + dyi
                        nc.tensor.matmul(
                            ps[:], lhsT=wb_t[:, half, s, :],
                            rhs=up_r[:, ys:ys + RBLK, dxi:dxi + W],
                            start=(s == 0), stop=(s == nmm - 1))
                    psr = ps if mixed else (
                        psA.tile([COUT, RBLK, W], mybir.dt.float32,
                                 tag="psA", name=f"psr_{bi}_{half}")
                        if npair else None)
                    for p, (c1, c2) in enumerate(pairs):
                        step = cell_off(c2, 0) - cell_off(c1, 0)
                        off = cell_off(c1, bi)
                        win = upf_fl[:, off:off + RBLK * W]
                        rhs = bass.AP(tensor=win.tensor, offset=win.offset,
                                      ap=[win.ap[0], [step, 2], win.ap[1]])
                        nc.tensor.matmul(
                            psr[:], lhsT=wr_t[:, p, half, :, :], rhs=rhs,
                            perf_mode=mybir.MatmulPerfMode.DoubleRow,
                            start=(False if mixed else p == 0),
                            stop=(nbig + p == nmm - 1) if mixed
                            else (p == npair - 1))
                    ob = outp.tile([COUT, RBLK, W], out_dt, tag="ob",
                                   name=f"ob_{bi}_{half}")
                    if V2_EVAC_ENG == "vector":
                        nc.vector.tensor_copy(ob[:], ps[:])
                    else:
                        nc.scalar.copy(out=ob[:], in_=ps[:])
                    if not mixed and npair:
                        nc.vector.scalar_tensor_tensor(
                            out=ob[:], in0=psr[:], scalar=1.0,
                            in1=ob[:], op0=mybir.AluOpType.mult,
                            op1=mybir.AluOpType.add)
                    nc.sync.dma_start(
                        out=out_d[128 * half:128 * (half + 1),
                                  RBLK * bi:RBLK * (bi + 1), :],
                        in_=ob[:])

            def emit_band_rest(b):
                if b == 0:
                    # zero the two bottom halo rows on the r=0 strip
                    nc.vector.tensor_scalar_mul(up_r[:, 0:2, :], up_r[:, 0:2, :],
                                                mk_t[:, 0:1])
                # fp8 ring planes for this band (x 1/RING_W_SCALE)
                for i, dxp in enumerate(need_dx):
                    src = up_r[:, 12 * b:12 * b + 12, dxp:dxp + W]
                    dst = upf_t[:, i, 12 * b:12 * b + 12, :]
                    if V2_VEC_PLANES and i % 2 == 1:
                        nc.vector.tensor_scalar_mul(dst, src,
                                                    1.0 / RING_W_SCALE)
                    else:
                        nc.scalar.mul(out=dst, in_=src, mul=1.0 / RING_W_SCALE)
                for bi in band_blocks[b]:
                    emit_block(bi)

            for b in range(3):
                for (py, px) in ((0, 0), (0, 1), (1, 0), (1, 1)):
                    emit_phase(b, py, px)
                emit_band_rest(b)

    nc.compile()
    return nc


def _build_nc_wg():
    """Winograd F(2,3)-rows device program.

    Stage A (transposed conv) unchanged.  Then per band, V planes
      V0[t] = u[2t+1]-u[2t+3], V1[t] = u[2t+2]+u[2t+3],
      V2[t] = u[2t+3]-u[2t+2], V3[t] = u[2t+2]-u[2t+4]   (u rows of `up`)
    are built on vector/gpsimd.  Stage B per (half, 4-row block):
      m_k = sum_kx wg[k,kx] @ V_k[2bi:2bi+2, kx+1 : kx+1+W]   (4 psum comps)
      out even rows = m0+m1+m2, odd rows = m1-m2-m3
    Combines: scalar evacuates m1,m3; DVE does the three psum-reading adds;
    gpsimd does the sbuf-only one.  PE sequence A0,A1,B0,A2,B1,B2 so V(b)
    always builds in the shadow of PE work on other data."""
    nc = bacc.Bacc("TRN2", target_bir_lowering=False, debug=False,
                   enable_asserts=False)

    xs_d = nc.dram_tensor("xs", [COUT, 3 * 2 * 8 * XC], mybir.dt.bfloat16,
                          kind="ExternalInput").ap()
    wa_d = nc.dram_tensor("wa", [COUT, 2 * 9 * COUT], mybir.dt.bfloat16,
                          kind="ExternalInput").ap()
    wg_d = nc.dram_tensor("wg", [COUT, 4 * 3 * 2 * COUT], mybir.dt.bfloat16,
                          kind="ExternalInput").ap()
    mk_d = nc.dram_tensor("mk", [COUT, 1], mybir.dt.float32,
                          kind="ExternalInput").ap()
    out_d = nc.dram_tensor("out", [CIN, OUT_R, W], mybir.dt.float16,
                           kind="ExternalOutput").ap()

    NT = OUT_R // 2            # 16 winograd tile rows
    mm = mybir.AluOpType.mult
    aa = mybir.AluOpType.add

    with tile.TileContext(nc) as tc:
        with (
            tc.tile_pool(name="singles", bufs=1) as singles,
            tc.tile_pool(name="outp", bufs=3) as outp,
            tc.tile_pool(name="evp", bufs=3) as evp,
            tc.tile_pool(name="psA", bufs=4, space="PSUM") as psA,
            tc.tile_pool(name="psB", bufs=2, space="PSUM") as psB,
        ):
            xs_t = singles.tile([COUT, 3, 2, 8, XC], mybir.dt.bfloat16)
            wa_t = singles.tile([COUT, 9, 2, COUT], mybir.dt.bfloat16)
            wg_t = singles.tile([COUT, 2, 4, 3, COUT], mybir.dt.bfloat16)
            mk_t = singles.tile([COUT, 1], mybir.dt.float32)
            up_t = singles.tile([COUT, UR * UC], mybir.dt.bfloat16)
            v_t = singles.tile([COUT, 4, NT, UC], mybir.dt.bfloat16)

            # ---- optional PE p-state warmup on zeroed dummy data ----
            if WG_WARMUP:
                wu_t = singles.tile([COUT, 384], mybir.dt.bfloat16)
                nc.vector.memset(wu_t[:], 0.0)
                for i in range(WG_WARMUP):
                    psw = psA.tile([COUT, 384], mybir.dt.float32, tag="psA",
                                   name=f"psw_{i}")
                    nc.tensor.matmul(psw[:], lhsT=wu_t[:, :COUT],
                                     rhs=wu_t[:], start=True, stop=True)

            # ---- input DMA, critical pieces first on both queues ----
            xs4 = xs_t[:]
            xs4_d = xs_d.rearrange("p (a b c d) -> p a b c d", a=3, b=2, c=8)
            wa_flat = wa_t[:].rearrange("p a b c -> p (a b c)")
            wg_flat = wg_t[:].rearrange("p a b c d -> p (a b c d)")
            nc.sync.dma_start(out=wa_flat[:, :WA_CUTS[1] * 2 * COUT],
                              in_=wa_d[:, :WA_CUTS[1] * 2 * COUT])
            nc.scalar.dma_start(out=xs4[:, 0], in_=xs4_d[:, 0])
            for c0, c1 in zip(WA_CUTS[1:-1], WA_CUTS[2:]):
                nc.sync.dma_start(out=wa_flat[:, c0 * 2 * COUT:c1 * 2 * COUT],
                                  in_=wa_d[:, c0 * 2 * COUT:c1 * 2 * COUT])
            nc.scalar.dma_start(out=xs4[:, 1], in_=xs4_d[:, 1])
            nc.sync.dma_start(out=mk_t[:], in_=mk_d)
            half_wg = 4 * 3 * COUT
            nc.scalar.dma_start(out=wg_flat[:, :half_wg],
                                in_=wg_d[:, :half_wg])
            nc.scalar.dma_start(out=xs4[:, 2], in_=xs4_d[:, 2])
            nc.sync.dma_start(out=wg_flat[:, half_wg:],
                              in_=wg_d[:, half_wg:])

            up_w = up_t.rearrange("p (a q c r) -> p a q c r", q=2, c=66, r=2)
            up_r = up_t.rearrange("p (l c) -> p l c", c=UC)
            up_pair = up_t.rearrange("p (l2 two c) -> p l2 two c",
                                     two=2, c=UC)
            nc.vector.memset(up_r[:, :, 0:2], 0.0)
            nc.vector.memset(up_r[:, :, 130:132], 0.0)

            ytaps = {0: ((1, 0),), 1: ((2, 0), (0, 1))}

            def emit_phase(b, py, px):
                a0 = 6 * b
                rc = 6
                taps = [(jy, dy, jx, dx)
                        for jy, dy in ytaps[py] for jx, dx in ytaps[px]]
                ps = psA.tile([COUT, rc, 64], mybir.dt.float32, tag="psA",
                              name=f"psA_{b}_{py}_{px}")
                nmm = len(taps) * 2
                i = 0
                for h2 in range(2):
                    for (jy, dy, jx, dx) in taps:
                        r0x = a0 + 1 + dy - XS_R0[b]
                        nc.tensor.matmul(
                            ps[:, :rc, :],
                            lhsT=wa_t[:, TAP_POS[jy * 3 + jx], h2, :],
                            rhs=xs_t[:, b, h2, r0x:r0x + rc, dx:dx + 64],
                            start=(i == 0), stop=(i == nmm - 1),
                        )
                        i += 1
                nc.scalar.copy(
                    out=up_w[:, a0:a0 + rc, py, 1:65, px],
                    in_=ps[:, :rc, :],
                )

            def emit_band_A(b):
                for (py, px) in ((0, 0), (0, 1), (1, 0), (1, 1)):
                    emit_phase(b, py, px)
                if b == 0:
                    nc.vector.tensor_scalar_mul(up_r[:, 0:2, :],
                                                up_r[:, 0:2, :], mk_t[:, 0:1])

            V_T0 = (0, 4, 10, 16)      # t-ranges per band

            def emit_V(b):
                t0, t1 = V_T0[b], V_T0[b + 1]
                n_ = t1 - t0
                # V0[t] = u[2t+1] - u[2t+3]
                nc.vector.scalar_tensor_tensor(
                    out=v_t[:, 0, t0:t1, :],
                    in0=up_pair[:, t0 + 1:t1 + 1, 1, :], scalar=-1.0,
                    in1=up_pair[:, t0:t1, 1, :], op0=mm, op1=aa)
                # V1[t] = u[2t+2] + u[2t+3]
                nc.vector.scalar_tensor_tensor(
                    out=v_t[:, 1, t0:t1, :],
                    in0=up_pair[:, t0 + 1:t1 + 1, 0, :], scalar=1.0,
                    in1=up_pair[:, t0 + 1:t1 + 1, 1, :], op0=mm, op1=aa)
                # V2[t] = u[2t+3] - u[2t+2]
                nc.gpsimd.tensor_tensor(
                    v_t[:, 2, t0:t1, :],
                    up_pair[:, t0 + 1:t1 + 1, 1, :],
                    up_pair[:, t0 + 1:t1 + 1, 0, :], mybir.AluOpType.subtract)
                # V3[t] = u[2t+2] - u[2t+4]
                nc.gpsimd.tensor_tensor(
                    v_t[:, 3, t0:t1, :],
                    up_pair[:, t0 + 1:t1 + 1, 0, :],
                    up_pair[:, t0 + 2:t1 + 2, 0, :], mybir.AluOpType.subtract)

            # output DMA groups (blocks per DMA, grouped within bands)
            OUT_GROUPS = ((0, 1), (2, 3), (4,), (5, 6), (7,))
            grp_of = {bi: g for g in OUT_GROUPS for bi in g}
            ob_tiles = {}

            def emit_block(bi, half):
                # matmul group order k1,k3,k0,k2 so the m1/m3 evacs and the
                # gpsimd o1 combine overlap the k0/k2 matmuls; after the last
                # group only the two DVE writes into ob remain.
                ps = psB.tile([COUT, 4, 2, W], mybir.dt.float32, tag="psB",
                              name=f"psB_{bi}_{half}")
                g = grp_of[bi]
                if (g, half) not in ob_tiles:
                    ob_tiles[(g, half)] = outp.tile(
                        [COUT, len(g) * 4, W], mybir.dt.float16, tag="ob",
                        name=f"ob_{g[0]}_{half}", padded_shape=[COUT, 8, W])
                ob = ob_tiles[(g, half)]
                toff = 2 * (bi - g[0])
                obr = ob.rearrange("p (t s) c -> p t s c", s=2)
                m1s = evp.tile([COUT, 2, W], mybir.dt.float32, tag="m1s",
                               name=f"m1s_{bi}_{half}")
                m3s = evp.tile([COUT, 2, W], mybir.dt.float32, tag="m3s",
                               name=f"m3s_{bi}_{half}")
                e1 = evp.tile([COUT, 2, W], mybir.dt.float32, tag="e1",
                              name=f"e1_{bi}_{half}")
                o1 = evp.tile([COUT, 2, W], mybir.dt.float32, tag="o1",
                              name=f"o1_{bi}_{half}")

                def mmk(k):
                    for kx in range(3):
                        nc.tensor.matmul(
                            ps[:, k], lhsT=wg_t[:, half, k, kx, :],
                            rhs=v_t[:, k, 2 * bi:2 * bi + 2, kx + 1:kx + 1 + W],
                            start=(kx == 0), stop=(kx == 2))

                mmk(1)
                nc.scalar.copy(out=m1s[:], in_=ps[:, 1])
                mmk(3)
                nc.scalar.copy(out=m3s[:], in_=ps[:, 3])
                # o1 = m1 - m3 (sbuf-only, runs during k0/k2 matmuls)
                nc.gpsimd.tensor_tensor(o1[:], m1s[:], m3s[:],
                                        mybir.AluOpType.subtract)
                mmk(0)
                # e1 = m0 + m1 (runs during k2 matmuls)
                nc.vector.scalar_tensor_tensor(
                    out=e1[:], in0=ps[:, 0], scalar=1.0, in1=m1s[:],
                    op0=mm, op1=aa)
                mmk(2)
                nc.vector.scalar_tensor_tensor(
                    out=obr[:, toff:toff + 2, 0, :], in0=ps[:, 2], scalar=1.0,
                    in1=e1[:], op0=mm, op1=aa)
                nc.vector.scalar_tensor_tensor(
                    out=obr[:, toff:toff + 2, 1, :], in0=ps[:, 2], scalar=-1.0,
                    in1=o1[:], op0=mm, op1=aa)
                if bi == g[-1]:
                    nc.sync.dma_start(
                        out=out_d[128 * half:128 * (half + 1),
                                  4 * g[0]:4 * g[0] + 4 * len(g), :],
                        in_=ob[:, :len(g) * 4, :])

            # ---- schedule: A0, A1, [V0] B0 B1, A2, [V1] B2 B3 B4, [V2] ... ----
            emit_band_A(0)
            emit_band_A(1)
            emit_V(0)
            for bi in (0, 1):
                for half in range(2):
                    emit_block(bi, half)
            emit_band_A(2)
            emit_V(1)
            for bi in (2, 3, 4):
                for half in range(2):
                    emit_block(bi, half)
            emit_V(2)
            for bi in (5, 6, 7):
                for half in range(2):
                    emit_block(bi, half)

    nc.compile()
    return nc


def _build_nc(ncell):
    if isinstance(ncell, tuple) and ncell[0] == "wg":
        return _build_nc_wg()
    if isinstance(ncell, tuple) and ncell[0] == "v2":
        return _build_nc_v2(ncell)
    fp8r = isinstance(ncell, tuple) and ncell[0] == "fp8r"
    if fp8r:
        bigs, pairs = list(ncell[1]), list(ncell[2])
        ncell = max(len(bigs), 1)
        cells, dyn = None, False
    elif isinstance(ncell, tuple):      # ("union", cell, cell, ...)
        cells = list(ncell[1:])
        ncell = len(cells)
        dyn = False
    else:
        cells = list(range(NCELL)) if VARIANT == "full25" else None
        dyn = VARIANT not in ("full25",)
    nc = bacc.Bacc("TRN2", target_bir_lowering=False, debug=False,
                   enable_asserts=False)

    xs_d = nc.dram_tensor("xs", [COUT, 3 * 2 * 8 * XC], mybir.dt.bfloat16,
                          kind="ExternalInput").ap()
    wa_d = nc.dram_tensor("wa", [COUT, 2 * 9 * COUT], mybir.dt.bfloat16,
                          kind="ExternalInput").ap()
    wb_d = nc.dram_tensor("wb", [COUT, ncell * 2 * COUT], mybir.dt.bfloat16,
                          kind="ExternalInput").ap()
    mk_d = nc.dram_tensor("mk", [COUT, 1], mybir.dt.float32,
                          kind="ExternalInput").ap()
    if fp8r:
        wr_d = nc.dram_tensor(
            "wr", [COUT, max(len(pairs), 1) * 2 * 2 * COUT],
            mybir.dt.float8e4, kind="ExternalInput").ap()
    hyb = VARIANT == "hybrid"
    S = HYBRID_S
    if hyb:
        co_d = nc.dram_tensor("co", [1, 9, 2], mybir.dt.int32,
                              kind="ExternalInput").ap()
        bs_d = nc.dram_tensor("bs", [COUT, 18], mybir.dt.float32,
                              kind="ExternalInput").ap()
        dsc_d = nc.dram_tensor("dsc", [COUT, 2 * max(S, 1)], mybir.dt.float32,
                               kind="ExternalInput").ap()
    elif dyn:
        co_d = nc.dram_tensor("co", [1, ncell, 2], mybir.dt.int32,
                              kind="ExternalInput").ap()
    out_d = nc.dram_tensor("out", [CIN, OUT_R, W], mybir.dt.float32,
                           kind="ExternalOutput").ap()

    with tile.TileContext(nc) as tc:
        with (
            tc.tile_pool(name="singles", bufs=1) as singles,
            tc.tile_pool(name="outp", bufs=4) as outp,
            tc.tile_pool(name="psB", bufs=4, space="PSUM") as psB,
            tc.tile_pool(name="psR", bufs=4, space="PSUM") as psR,
        ):
            xs_t = singles.tile([COUT, 3, 2, 8, XC], mybir.dt.bfloat16)
            wa_t = singles.tile([COUT, 9, 2, COUT], mybir.dt.bfloat16)
            wb_t = singles.tile([COUT, ncell, 2, COUT], mybir.dt.bfloat16)
            mk_t = singles.tile([COUT, 1], mybir.dt.float32)
            # +12 pad: hybrid vy reads may run a few elements past the last
            # row (col-window spill); padded region is zeroed, never consumed
            up_full = singles.tile([COUT, UR * UC + 12], mybir.dt.bfloat16)
            up_t = up_full[:, :UR * UC]

            # stage-A critical inputs split across both HWDGE queues; xs is
            # further split by row band so the first stage-A band can start
            # after ~0.3MB instead of the whole tensor.  Band a0 reads xs rows
            # a0+1+dy (dy<=1), so rows [0,9) cover band 0, [9,20) the rest.
            xs4 = xs_t[:]
            xs4_d = xs_d.rearrange("p (a b c d) -> p a b c d", a=3, b=2, c=8)
            for h2 in range(2):
                eng = nc.sync if h2 == 0 else nc.scalar
                eng.dma_start(out=xs4[:, h2, 0:9, :], in_=xs4_d[:, h2, 0:9, :])
            nc.sync.dma_start(out=wa_t[:].rearrange("p a b c -> p (a b c)"), in_=wa_d)
            for h2 in range(2):
                eng = nc.scalar if h2 == 0 else nc.sync
                eng.dma_start(out=xs4[:, h2, 9:, :], in_=xs4_d[:, h2, 9:, :])
            nc.sync.dma_start(out=mk_t[:], in_=mk_d)
            wb_flat = wb_t[:].rearrange("p a b c -> p (a b c)")
            nc.scalar.dma_start(out=wb_flat, in_=wb_d)
            if fp8r:
                wr_t = singles.tile([COUT, max(len(pairs), 1), 2, 2, COUT],
                                    mybir.dt.float8e4)
                nc.sync.dma_start(
                    out=wr_t[:].rearrange("p a b c d -> p (a b c d)"), in_=wr_d)
                upf_t = singles.tile([COUT, 5, UR, W], mybir.dt.float8e4)
            if hyb:
                co_t = singles.tile([1, 9, 2], mybir.dt.int32)
                bs_t = singles.tile([COUT, 9, 2], mybir.dt.float32)
                dsc_t = singles.tile([COUT, max(S, 1), 2], mybir.dt.float32)
                nc.sync.dma_start(out=co_t[:].rearrange("p a b -> p (a b)"),
                                  in_=co_d.rearrange("p a b -> p (a b)"))
                nc.sync.dma_start(out=bs_t[:].rearrange("p a b -> p (a b)"),
                                  in_=bs_d)
                nc.sync.dma_start(out=dsc_t[:].rearrange("p a b -> p (a b)"),
                                  in_=dsc_d)
            elif dyn:
                co_t = singles.tile([1, ncell, 2], mybir.dt.int32)
                nc.sync.dma_start(out=co_t[:].rearrange("p a b -> p (a b)"),
                                  in_=co_d.rearrange("p a b -> p (a b)"))

            # zero the up tile (margins + potentially-invalid rows)
            nc.vector.memset(up_full[:], 0.0)

            # views of up: [p, a'(18), q(2), cc(66), r(2)] for phase writes,
            # [p, l(36), c(132)] for stage-B reads
            up_w = up_t.rearrange("p (a q c r) -> p a q c r", q=2, c=66, r=2)
            up_r = up_t.rearrange("p (l c) -> p l c", c=132)

            # ---- stage A: transposed conv -> up ----
            # row-major (a0 outer) so each 12-row band of up completes early;
            # for fp8r the band's fp8 casts are emitted right behind it, so
            # the ring matmuls never wait on a late cast burst
            ytaps = {0: ((1, 0),), 1: ((2, 0), (0, 1))}
            if fp8r:
                need_dx = sorted({c % 5 for pr in pairs for c in pr})
            for a0 in range(0, 18, 6):
                rc = 6
                for py in (0, 1):
                    for px in (0, 1):
                        taps = [(jy, dy, jx, dx)
                                for jy, dy in ytaps[py] for jx, dx in ytaps[px]]
                        # stage A borrows the ring pool (idle here) so its
                        # evacuations never block stage-B big-cell psum slots
                        pool = psR if fp8r else psB
                        ps = pool.tile([COUT, 6, 64], mybir.dt.float32,
                                       tag="psR" if fp8r else "psB")
                        nmm = len(taps) * 2
                        i = 0
                        for (jy, dy, jx, dx) in taps:
                            for h2 in range(2):
                                r0x = a0 + 1 + dy - XS_R0[b]
                                nc.tensor.matmul(
                                    ps[:, :rc, :],
                                    lhsT=wa_t[:, TAP_POS[jy * 3 + jx], h2, :],
                                    rhs=xs_t[:, b, h2, r0x:r0x + rc,
                                             dx:dx + 64],
                                    start=(i == 0), stop=(i == nmm - 1),
                                )
                                i += 1
                        # scatter phase result into up (cast to bf16)
                        nc.scalar.copy(
                            out=up_w[:, a0:a0 + rc, py, 1:65, px],
                            in_=ps[:, :rc, :],
                        )
                if a0 == 0:
                    # zero the bottom two halo rows on the r=0 strip (g=-2,-1):
                    # the phase formula extended below the image is invalid there
                    nc.vector.tensor_scalar_mul(up_r[:, 0:2, :], up_r[:, 0:2, :],
                                                mk_t[:, 0:1])
                if fp8r:
                    for dx in need_dx:
                        nc.scalar.copy(
                            out=upf_t[:, dx, 2 * a0:2 * a0 + 12, :],
                            in_=up_r[:, 2 * a0:2 * a0 + 12, dx:dx + W])

            # ---- stage B: effective-cell conv -> out ----
            if fp8r:
                _stage_b_fp8r(nc, tc, up_r, upf_t, wb_t, wr_t, bigs, pairs,
                              psB, psR, outp, out_d)
            elif hyb:
                with (
                    tc.tile_pool(name="vyp", bufs=2) as vyp,
                    tc.tile_pool(name="smp", bufs=2) as smp,
                ):
                    # per-tap (row, col) bases into vector-engine registers
                    rvs = [nc.vector.value_load(co_t[0:1, k, 0:1],
                                                min_val=0, max_val=3)
                           for k in range(9)]
                    cvs = [nc.vector.value_load(co_t[0:1, k, 1:2],
                                                min_val=0, max_val=3)
                           for k in range(9)]
                    mm = mybir.AluOpType.mult
                    aa = mybir.AluOpType.add
                    up_fl = up_full[:]
                    for sb in range(OUT_R // SBR):
                        vys, samps = [], []
                        for k in range(9):
                            vy = vyp.tile([COUT, SBR, UC], mybir.dt.bfloat16,
                                          tag=f"vy{k}")
                            # [SBR rows x UC cols] shifted window == contiguous
                            # flat block of SBR*UC elements
                            base = rvs[k] * UC + cvs[k] + (SBR * sb) * UC
                            i0 = up_fl[:, bass.ds(base, SBR * UC)].rearrange(
                                "p (a b) -> p a b", b=UC)
                            i1 = up_fl[:, bass.ds(base + UC, SBR * UC)].rearrange(
                                "p (a b) -> p a b", b=UC)
                            nc.vector.tensor_scalar_mul(vy[:], i0, bs_t[:, k, 0:1])
                            nc.vector.scalar_tensor_tensor(
                                out=vy[:], in0=i1, scalar=bs_t[:, k, 1:2],
                                in1=vy[:], op0=mm, op1=aa)
                            vys.append(vy)
                        for k in range(S):
                            sa = smp.tile([COUT, SBR, W], mybir.dt.bfloat16,
                                          tag=f"sa{k}")
                            nc.vector.tensor_scalar_mul(
                                sa[:], vys[k][:, :, 0:W], dsc_t[:, k, 0:1])
                            nc.vector.scalar_tensor_tensor(
                                out=sa[:], in0=vys[k][:, :, 1:W + 1],
                                scalar=dsc_t[:, k, 1:2], in1=sa[:],
                                op0=mm, op1=aa)
                            samps.append(sa)
                        for sub in range(SBR // RBLK):
                            rs = slice(RBLK * sub, RBLK * (sub + 1))
                            bi = (SBR * sb) // RBLK + sub
                            for half in range(2):
                                ps = psB.tile([COUT, RBLK, W], mybir.dt.float32,
                                              tag="psB")
                                nmm = S + 2 * (9 - S)
                                si = 0
                                for k in range(9):
                                    if k < S:
                                        rhss = [samps[k][:, rs, :]]
                                    else:
                                        rhss = [vys[k][:, rs, 0:W],
                                                vys[k][:, rs, 1:W + 1]]
                                    for rhs in rhss:
                                        nc.tensor.matmul(
                                            ps[:], lhsT=wb_t[:, si, half, :],
                                            rhs=rhs, start=(si == 0),
                                            stop=(si == nmm - 1))
                                        si += 1
                                ob = outp.tile([COUT, RBLK, W], mybir.dt.float32,
                                               tag="ob")
                                nc.scalar.copy(out=ob[:], in_=ps[:])
                                nc.sync.dma_start(
                                    out=out_d[128 * half:128 * (half + 1),
                                              RBLK * bi:RBLK * (bi + 1), :],
                                    in_=ob[:])
            else:
                if dyn:
                    # per-slot (row, col) bases into tensor-engine registers
                    rvs = [nc.tensor.value_load(co_t[0:1, ci, 0:1],
                                                min_val=0, max_val=4)
                           for ci in range(ncell)]
                    cvs = [nc.tensor.value_load(co_t[0:1, ci, 1:2],
                                                min_val=0, max_val=4)
                           for ci in range(ncell)]
                for bi in range(OUT_R // RBLK):
                    for half in range(2):
                        ps = psB.tile([COUT, RBLK, W], mybir.dt.float32, tag="psB")
                        for ci in range(ncell):
                            if dyn:
                                rhs = up_r[:, bass.ds(rvs[ci] + 4 * bi, RBLK),
                                           bass.ds(cvs[ci], W)]
                            else:
                                dyi, dxi = cells[ci] // 5, cells[ci] % 5
                                ys = 4 * bi + dyi  # up row = o_l + 2 + (dyi-2)
                                rhs = up_r[:, ys:ys + RBLK, dxi:dxi + W]
                            nc.tensor.matmul(
                                ps[:],
                                lhsT=wb_t[:, ci, half, :],
                                rhs=rhs,
                                start=(ci == 0), stop=(ci == ncell - 1),
                            )
                        ob = outp.tile([COUT, RBLK, W], mybir.dt.float32, tag="ob")
                        nc.scalar.copy(out=ob[:], in_=ps[:])
                        nc.sync.dma_start(
                            out=out_d[128 * half:128 * (half + 1),
                                      4 * bi:4 * bi + RBLK, :],
                            in_=ob[:],
                        )

    nc.compile()
    return nc


def _stage_b_fp8r(nc, tc, up_r, upf_t, wb_t, wr_t, bigs, pairs,
                  psB, psR, outp, out_d):
    """Stage B with big cells in bf16 and ring-cell pairs in fp8 DoubleRow.

    upf_t[dx] holds a margin-free fp8 copy of up cols [dx, dx+128), so every
    cell window is a contiguous 512-element block and pair steps are
    automatically 16-aligned (multiples of 128)."""
    mm = mybir.AluOpType.mult
    aa = mybir.AluOpType.add

    # (fp8 casts of up are emitted inline with stage A, band by band)

    upf_fl = upf_t[:].rearrange("p a b c -> p (a b c)")

    def cell_off(c, bi):
        return (c % 5) * (UR * W) + ((4 * bi) + (c // 5)) * W

    G = 2  # blocks per weight-reuse group
    for half in range(2):
        for bg in range(OUT_R // RBLK // G):
            pscs = [psB.tile([COUT, RBLK, W], mybir.dt.float32, tag="psB",
                             name=f"psc_{half}_{bg}_{g}") for g in range(G)]
            for si, ci in enumerate(bigs):
                dyi, dxi = ci // 5, ci % 5
                for g in range(G):
                    bi = G * bg + g
                    ys = 4 * bi + dyi
                    nc.tensor.matmul(
                        pscs[g][:], lhsT=wb_t[:, si, half, :],
                        rhs=up_r[:, ys:ys + RBLK, dxi:dxi + W],
                        start=(si == 0), stop=(si == len(bigs) - 1))
            psrs = None
            if pairs:
                psrs = [psR.tile([COUT, RBLK, W], mybir.dt.float32, tag="psR",
                                 name=f"psr_{half}_{bg}_{g}") for g in range(G)]
                for p, (c1, c2) in enumerate(pairs):
                    step = cell_off(c2, 0) - cell_off(c1, 0)
                    assert step > 0 and step % 16 == 0
                    for g in range(G):
                        bi = G * bg + g
                        win = upf_fl[:, cell_off(c1, bi):cell_off(c1, bi) + RBLK * W]
                        rhs = bass.AP(tensor=win.tensor, offset=win.offset,
                                      ap=[win.ap[0], [step, 2], win.ap[1]])
                        nc.tensor.matmul(
                            psrs[g][:], lhsT=wr_t[:, p, half, :, :], rhs=rhs,
                            perf_mode=mybir.MatmulPerfMode.DoubleRow,
                            start=(p == 0), stop=(p == len(pairs) - 1))
            for g in range(G):
                bi = G * bg + g
                ob = outp.tile([COUT, RBLK, W], mybir.dt.float32, tag="ob")
                nc.scalar.copy(out=ob[:], in_=pscs[g][:])
                if pairs:
                    # TensorScalarPtr may read only one PSUM input
                    nc.vector.scalar_tensor_tensor(
                        out=ob[:], in0=psrs[g][:], scalar=1.0 / RING_SCALE,
                        in1=ob[:], op0=mm, op1=aa)
                nc.sync.dma_start(
                    out=out_d[128 * half:128 * (half + 1),
                              RBLK * bi:RBLK * (bi + 1), :],
                    in_=ob[:])


# --------------------------------------------------------------------------
# entry point
# --------------------------------------------------------------------------

def kernel(x, lateral_feat, trans_w, off_w1, off_b1, off_w2, off_b2):
    x = np.asarray(x)
    oy, ox = _offsets_from_inputs(np.asarray(lateral_feat), np.asarray(off_w1),
                                  np.asarray(off_b1), np.asarray(off_w2),
                                  np.asarray(off_b2))
    in_maps, ncell = _prep_in_maps(x, np.asarray(trans_w), oy, ox)

    key = (VARIANT, ncell)
    if key not in _CACHED_NC:
        _CACHED_NC[key] = _build_nc(ncell)
    nc = _CACHED_NC[key]

    res = run_bass_kernel_spmd(nc, in_maps, core_ids=list(range(N_CORES)))

    out = np.empty((N_BATCH, CIN, H, W), np.float32)
    for core in range(N_CORES):
        n, r = core // STRIPS, core % STRIPS
        out[n, :, OUT_R * r:OUT_R * (r + 1), :] = res.results[core]["out"]
    return out



# revision 16
# speedup vs baseline: 1.5036x; 1.5036x over previous
"""Trainium2 Bass kernel for nn_DeformableTransposedConv.

Pipeline (per the reference):
  up  = ConvTranspose2d(x, trans_w, stride=2, pad=1, outpad=1)   # [N,128,128,128]
  off = tanh(conv(relu(conv(lateral_feat, w1)), w2))             # [N,18,1,1] -> broadcast
  out = deform_conv2d(up, off, trans_w, pad=1)                   # [N,256,128,128]

Key structure exploited:
  * The offsets are constant over space (1x1 lateral input broadcast), so the
    bilinear deformable gather collapses to a per-batch 5x5 conv with
    "effective" weights W_eff[n] built host-side from trans_w and the (tiny)
    offsets.  The device computes:
        out[n] = sum_{dy,dx in 5x5} W_eff[n,dy,dx] @ shift(up[n], dy, dx)
    as PSUM-accumulated matmuls over the 128 up-channels.
  * The stride-2 transposed conv splits into 4 phase sub-convs with
    {1,2,2,4} taps, each a PSUM-accumulated matmul over the 256 x-channels.

Sharding: 8 cores = 2 batches x 4 row-strips of 32 output rows.  Each core
computes out[n, :, 32r:32r+32, :] from a 20-row slice of x (with halo).
All weights / layout prep / zero padding is done host-side; the NEFF is
input-independent (weights and data are ExternalInputs).
"""

import numpy as np
import ml_dtypes

import concourse.bass as bass
import concourse.tile as tile
from concourse import bacc, mybir
from concourse.bass_utils import run_bass_kernel_spmd

BF16 = ml_dtypes.bfloat16

# ---- problem constants (hardcoded per contract) ----
N_BATCH = 2
CIN = 256
COUT = 128          # up channels
K = 3
PAD = 1
H0 = W0 = 64        # x spatial
H = W = 128         # up / out spatial
N_CORES = 8
STRIPS = 4          # row strips per batch
OUT_R = 32          # output rows per strip

# SBUF layout constants
XR, XC = 20, 66     # x tile rows (16 + 2 halo each side), cols (64 + 1 pad + 1 align)
UR, UC = 36, 132    # up tile rows (32 + 2 halo each side), cols (128 + 2 + 2)
NCELL = 25          # 5x5 effective deform kernel
RBLK = 4            # output rows per stage-B block (4*128 = 512 = one PSUM bank)

# stage-B variant:
#   "full25" = static 5x5 effective conv (25 matmul terms / block)
#   "slots"  = dynamic cell slots (pruned zero cells, runtime offsets)
#   "hybrid" = DVE bilinear blends + matmuls (y-blend on DVE for all taps;
#              x-blend on DVE for the first HYBRID_S taps, folded into scaled
#              weights for the rest)
import os as _os
VARIANT = _os.environ.get("KERNEL_VARIANT", "wg")
WG_WARMUP = int(_os.environ.get("WG_WARMUP", "0"))   # p-state warmup matmuls
HYBRID_S = int(_os.environ.get("HYBRID_S", "4"))
SBR = 8             # hybrid blend superblock rows (2 PSUM blocks)
PLR = 32            # v2: ring-plane rows (blocks 0..7 read plane rows 4bi..4bi+3)
V2_VEC_PLANES = int(_os.environ.get("V2_VEC_PLANES", "1"))  # 0=scalar,1=split,2=vector
V2_MIXED_GROUP = _os.environ.get("V2_MIXED_GROUP", "1") == "1"
V2_OUT = _os.environ.get("V2_OUT", "f16")
V2_NO_EVAC = _os.environ.get("V2_NO_EVAC", "0") == "1"   # timing probe only
V2_NO_PLANES = _os.environ.get("V2_NO_PLANES", "0") == "1"  # timing probe only
V2_NO_ODMA = _os.environ.get("V2_NO_ODMA", "0") == "1"      # timing probe only
V2_EVAC_ENG = _os.environ.get("V2_EVAC_ENG", "vector")

_CACHED_NC = {}


# --------------------------------------------------------------------------
# host-side preparation
# --------------------------------------------------------------------------

def _offsets_from_inputs(lateral_feat, off_w1, off_b1, off_w2, off_b2):
    """Tiny offset MLP (conv on 1x1 spatial input == center-tap matmul)."""
    lf = lateral_feat[:, :, 0, 0].astype(np.float32)                    # [N,128]
    h = np.maximum(0.0, lf @ off_w1[:, :, 1, 1].T.astype(np.float32)
                   + off_b1.astype(np.float32))                         # [N,64]
    off = np.tanh(h @ off_w2[:, :, 1, 1].T.astype(np.float32)
                  + off_b2.astype(np.float32)).astype(np.float32)       # [N,18]
    oy = off.reshape(-1, K * K, 2)[:, :, 0]
    ox = off.reshape(-1, K * K, 2)[:, :, 1]
    return oy, ox


def _w_eff(trans_w, oy, ox):
    """Effective 5x5 deform weights. Returns [N, 5, 5, 256(o), 128(c)] f32."""
    n_b = oy.shape[0]
    Weff = np.zeros((n_b, 5, 5, CIN, COUT), np.float32)
    for n in range(n_b):
        for k in range(K * K):
            ky, kx = k // K, k % K
            ay = np.float32(ky - 1) + oy[n, k]
            ax = np.float32(kx - 1) + ox[n, k]
            Ay, Ax = int(np.floor(ay)), int(np.floor(ax))
            dy = float(ay) - Ay
            dx = float(ax) - Ax
            tap = trans_w[:, :, ky, kx].astype(np.float32)
            for cy, wy in ((0, 1.0 - dy), (1, dy)):
                for cx, wx in ((0, 1.0 - dx), (1, dx)):
                    w = wy * wx
                    if w != 0.0:
                        Weff[n, Ay + cy + 2, Ax + cx + 2] += w * tap
    return Weff


def _prep_in_maps(x, trans_w, oy, ox):
    """Build the per-core input dicts (already bf16, padded, SBUF-layouts).
    Returns (in_maps, ncell) where ncell is the stage-B slot count."""
    xf = x.astype(np.float32)

    # stage-A weights, shared by all cores: wa[k, h2, j, m]
    wa = np.zeros((COUT, 2, 9, COUT), np.float32)
    for h2 in range(2):
        for j in range(9):
            jy, jx = j // 3, j % 3
            # lhsT[K=cin(128), M=cout(128)] = trans_w[h2*128+kk, m, jy, jx]
            wa[:, h2, j, :] = trans_w[h2 * 128:(h2 + 1) * 128, :, jy, jx]
    wa_b = wa.astype(BF16).reshape(COUT, 2 * 9 * COUT)

    if VARIANT == "hybrid":
        return _prep_in_maps_hybrid(xf, trans_w, oy, ox, wa_b)
    if VARIANT == "fp8r":
        return _prep_in_maps_fp8r(xf, trans_w, oy, ox, wa_b)
    if VARIANT == "wg":
        r = _prep_in_maps_wg(xf, trans_w, oy, ox)
        if r is not None:
            return r
        return _prep_in_maps_v2(xf, trans_w, oy, ox, wa_b)
    if VARIANT == "v2":
        return _prep_in_maps_v2(xf, trans_w, oy, ox, wa_b)

    # stage-B weights per batch
    Weff = _w_eff(trans_w, oy, ox)                      # [N,5,5,256,128]
    wb_all, co_all = [], []
    if VARIANT == "full25":
        ncell = NCELL
        for n in range(N_BATCH):
            wb = Weff[n].reshape(NCELL, 2, COUT, COUT)   # [cell, half, o(128), c]
            wb = wb.transpose(3, 0, 1, 2)                # [c, cell, half, o]
            wb_all.append(np.ascontiguousarray(wb).astype(BF16)
                          .reshape(COUT, NCELL * 2 * COUT))
            co_all.append(None)
    elif VARIANT == "union":
        # static program specialized on the union of nonzero cells across
        # batches (compile cache keyed on the union tuple)
        nz = [np.nonzero(np.abs(Weff[n]).reshape(25, -1).max(1) > 0)[0]
              for n in range(N_BATCH)]
        union = sorted(set(int(c) for z in nz for c in z))
        ncell = ("union",) + tuple(union)
        for n in range(N_BATCH):
            wb = np.zeros((len(union), 2, COUT, COUT), np.float32)
            for s, ci in enumerate(union):
                wb[s] = Weff[n, ci // 5, ci % 5].reshape(2, COUT, COUT)
            wb = wb.transpose(3, 0, 1, 2)
            wb_all.append(np.ascontiguousarray(wb).astype(BF16)
                          .reshape(COUT, len(union) * 2 * COUT))
            co_all.append(None)
    else:  # "slots": pruned nonzero cells, offsets shipped as data
        nz = [np.nonzero(np.abs(Weff[n]).reshape(25, -1).max(1) > 0)[0]
              for n in range(N_BATCH)]
        ncell = max(len(z) for z in nz)
        for n in range(N_BATCH):
            cells = list(nz[n]) + [12] * (ncell - len(nz[n]))  # pad w/ center
            wb = np.zeros((ncell, 2, COUT, COUT), np.float32)
            co = np.zeros((1, ncell, 2), np.int32)
            for s, ci in enumerate(cells):
                dyi, dxi = ci // 5, ci % 5
                if s < len(nz[n]):
                    wb[s] = Weff[n, dyi, dxi].reshape(2, COUT, COUT)
                co[0, s] = (dyi, dxi)
            wb = wb.transpose(3, 0, 1, 2)                # [c, slot, half, o]
            wb_all.append(np.ascontiguousarray(wb).astype(BF16)
                          .reshape(COUT, ncell * 2 * COUT))
            co_all.append(co)

    in_maps = []
    for core in range(N_CORES):
        n, r = core // STRIPS, core % STRIPS
        # x slice with halo: global x rows [16r-2, 16r+18)
        xs = np.zeros((COUT, 2, XR, XC), np.float32)
        r0 = 16 * r - 2
        lo, hi = max(0, r0), min(H0, r0 + XR)
        for h2 in range(2):
            xs[:, h2, lo - r0:hi - r0, :W0] = xf[n, h2 * 128:(h2 + 1) * 128, lo:hi, :]
        # bottom-halo validity mask: strip 0 must zero up rows g=-2,-1 which
        # the phase formula would otherwise fill with spurious values
        mk = np.full((COUT, 1), 0.0 if r == 0 else 1.0, np.float32)
        im = {
            "xs": np.ascontiguousarray(xs.astype(BF16).reshape(COUT, 2 * XR * XC)),
            "wa": wa_b,
            "wb": wb_all[n],
            "mk": mk,
        }
        if co_all[n] is not None:
            im["co"] = co_all[n]
        in_maps.append(im)
    return in_maps, ncell


FP8 = ml_dtypes.float8_e4m3
RING_SCALE = 256.0


def _prep_in_maps_fp8r(xf, trans_w, oy, ox, wa_b):
    """Union cells; big cells in bf16, small 'ring' cells paired into fp8
    DoubleRow matmuls (weights scaled by RING_SCALE)."""
    Weff = _w_eff(trans_w, oy, ox)                       # [N,5,5,256,128]
    norms = np.abs(Weff).reshape(N_BATCH, 25, -1).max(2)  # [N,25]
    union = sorted(set(np.nonzero(norms.max(0) > 0)[0].tolist()))
    thr = 0.25 * norms.max()
    bigs = [c for c in union if norms[:, c].max() > thr]
    rings = [c for c in union if c not in bigs]
    if len(rings) % 2:
        bigs.append(rings.pop())                          # odd leftover -> bf16
    # order by window offset (dx major, dy minor); pair far-apart cells so the
    # two DoubleRow K-group windows never overlap (overlapping windows were
    # measured ~1.7x slower on the PE)
    rings.sort(key=lambda c: (c % 5, c // 5))
    nh = len(rings) // 2
    pairs = [(rings[i], rings[i + nh]) for i in range(nh)]
    bigs = sorted(bigs)

    wb_all, wr_all = [], []
    for n in range(N_BATCH):
        wb = np.zeros((max(len(bigs), 1), 2, COUT, COUT), np.float32)
        for s, ci in enumerate(bigs):
            wb[s] = Weff[n, ci // 5, ci % 5].reshape(2, COUT, COUT)
        wb = wb.transpose(3, 0, 1, 2)                     # [c, slot, half, o]
        wb_all.append(np.ascontiguousarray(wb).astype(BF16)
                      .reshape(COUT, -1))
        wr = np.zeros((max(len(pairs), 1), 2, 2, COUT, COUT), np.float32)
        for p, (c1, c2) in enumerate(pairs):
            for half in range(2):
                wr[p, half, 0] = RING_SCALE * \
                    Weff[n, c1 // 5, c1 % 5][128 * half:128 * (half + 1)].T
                wr[p, half, 1] = RING_SCALE * \
                    Weff[n, c2 // 5, c2 % 5][128 * half:128 * (half + 1)].T
        # wr[p, half, ksub, c, o] -> [c, p, half, ksub, o]
        wr = wr.transpose(3, 0, 1, 2, 4)
        wr_all.append(np.ascontiguousarray(wr).astype(FP8).reshape(COUT, -1))

    in_maps = []
    for core in range(N_CORES):
        n, r = core // STRIPS, core % STRIPS
        xs = np.zeros((COUT, 2, XR, XC), np.float32)
        r0 = 16 * r - 2
        lo, hi = max(0, r0), min(H0, r0 + XR)
        for h2 in range(2):
            xs[:, h2, lo - r0:hi - r0, :W0] = xf[n, h2 * 128:(h2 + 1) * 128, lo:hi, :]
        mk = np.full((COUT, 1), 0.0 if r == 0 else 1.0, np.float32)
        in_maps.append({
            "xs": np.ascontiguousarray(xs.astype(BF16).reshape(COUT, 2 * XR * XC)),
            "wa": wa_b,
            "wb": wb_all[n],
            "wr": wr_all[n],
            "mk": mk,
        })
    return in_maps, ("fp8r", tuple(bigs), tuple(pairs))


RING_W_SCALE = 16.0     # ring weights x16, up fp8 copies x1/16 -> product x1
TAP_ORDER = (4, 3, 5, 1, 7, 0, 2, 6, 8)   # phase-major: p00|p01|p10|p11
TAP_POS = {j: i for i, j in enumerate(TAP_ORDER)}
WA_CUTS = (0, 1, 3, 5, 9)                 # DMA piece boundaries in TAP_ORDER
XS_R0 = (0, 7, 13)                        # first xs row held by each band tile
PRUNE_BUDGET = float(_os.environ.get("V2_PRUNE_BUDGET", "0.012"))


def _prune_rings(Weff, xf, trans_w, bigs, rings):
    """Exact-error greedy pruning: for each ring cell (ascending magnitude)
    try dropping it or folding its weights into an adjacent kept cell; accept
    while the accumulated absmax output error stays under PRUNE_BUDGET.
    Returns (rings_kept, Weff_adjusted, err_bound)."""
    N, H2 = N_BATCH, H
    # host up[n]: transposed conv, padded by 2 on each side for cell shifts
    upp = np.zeros((N, COUT, H2 + 4, W + 4), np.float32)
    for n in range(N):
        # up[m, g, h] = sum_{jy,jx,c} w[c,m,jy,jx] x[c,(g+1-jy)/2,(h+1-jx)/2]
        for jy in range(3):
            for jx in range(3):
                w = trans_w[:, :, jy, jx].astype(np.float32)      # [c, m]
                # valid g: g+1-jy even and 0 <= (g+1-jy)//2 < 64
                gs = np.arange(H2)
                ok_g = ((gs + 1 - jy) % 2 == 0) & ((gs + 1 - jy) // 2 >= 0) \
                    & ((gs + 1 - jy) // 2 < H0)
                hs = np.arange(W)
                ok_h = ((hs + 1 - jx) % 2 == 0) & ((hs + 1 - jx) // 2 >= 0) \
                    & ((hs + 1 - jx) // 2 < W0)
                gi = (gs[ok_g] + 1 - jy) // 2
                hi = (hs[ok_h] + 1 - jx) // 2
                contrib = (w.T @ np.ascontiguousarray(
                    xf[n][:, gi][:, :, hi]).reshape(CIN, -1)).reshape(
                        COUT, len(gi), len(hi))
                gg, hh = np.ix_(gs[ok_g] + 2, hs[ok_h] + 2)
                upp_n = upp[n]
                upp_n[:, gg, hh] += contrib
    def cell_out(n, Wc, ci):
        dy, dx = ci // 5, ci % 5
        win = np.ascontiguousarray(upp[n, :, dy:dy + H2, dx:dx + W])
        return (Wc @ win.reshape(COUT, -1)).reshape(CIN, H2, W)
    # full-output scale
    scale = 0.0
    for n in range(N):
        acc = None
        for ci in set(bigs) | set(rings):
            t = cell_out(n, Weff[n, ci // 5, ci % 5].astype(np.float32), ci)
            acc = t if acc is None else acc + t
        scale = max(scale, np.abs(acc).max())
    Weff = Weff.copy()
    kept = list(rings)
    diff = [np.zeros((CIN, H2, W), np.float32) for _ in range(N)]
    order = sorted(rings, key=lambda c: float(
        np.abs(Weff[:, c // 5, c % 5]).max()))
    err = 0.0
    for ci in order:
        dy, dx = ci // 5, ci % 5
        others = [c2 for c2 in (set(bigs) | set(kept)) if c2 != ci]
        others.sort(key=lambda c2: abs(c2 // 5 - dy) + abs(c2 % 5 - dx))
        basis_cells = others[:3]
        # per-batch: least-squares fold of this cell onto the basis cells
        cand = []
        for n in range(N):
            Wc = Weff[n, dy, dx].astype(np.float32)
            if not np.any(Wc):
                cand.append((diff[n], []))
                continue
            r = cell_out(n, Wc, ci).ravel()
            B = np.stack([cell_out(n, Wc, c2).ravel() for c2 in basis_cells])
            G = B @ B.T
            b = B @ r
            try:
                al = np.linalg.solve(G + 1e-12 * np.eye(len(B)), b)
            except np.linalg.LinAlgError:
                al = np.zeros(len(B))
            resid = r - al @ B
            cand.append(((diff[n].ravel() + resid).reshape(CIN, H2, W),
                         list(zip(basis_cells, al))))
        e = max(np.abs(c[0]).max() for c in cand) / scale
        if e <= PRUNE_BUDGET:
            err = e
            for n in range(N):
                diff[n] = cand[n][0]
                Wc = Weff[n, dy, dx].copy()
                for c2, a in cand[n][1]:
                    Weff[n, c2 // 5, c2 % 5] += np.float32(a) * Wc
            Weff[:, dy, dx] = 0.0
            kept.remove(ci)
        else:
            break
    return kept, Weff, err



def _prep_in_maps_v2(xf, trans_w, oy, ox, wa_b):
    """v3: static big cells + static union ring cells, both accumulated into
    ONE psum bank per output block.

    Ring cells (bilinear spill corners) are paired into fp8 DoubleRow
    matmuls over per-dx margin-free fp8 copies of up.  Ring weights are
    scaled x16 and the fp8 copies x1/16, so the pair product is unscaled
    and rings accumulate into the SAME psum bank as the big cells (no
    separate merge pass).  Cells whose max-norm is below 0.4% of the
    global max (the ab bilinear corners, ~1e-4 relative) are dropped
    (~0.1% output error)."""
    Weff = _w_eff(trans_w, oy, ox)                        # [N,5,5,256,128]
    norms = np.abs(Weff).reshape(N_BATCH, 25, -1).max(2)  # [N,25]
    gmax = norms.max()
    bigs = sorted(int(c) for c in np.nonzero(norms.max(0) > 0.25 * gmax)[0])
    keep = (norms.max(0) > 0.004 * gmax) & (norms.max(0) <= 0.25 * gmax)
    rings = [int(c) for c in np.nonzero(keep)[0] if c not in bigs]
    if PRUNE_BUDGET > 0 and rings:
        rings, Weff, _perr = _prune_rings(Weff, xf, trans_w, bigs, rings)
    # order by (dx major, dy minor) and pair far apart so the two DoubleRow
    # K-group windows never overlap
    rings.sort(key=lambda c: (c % 5, c // 5))
    if len(rings) % 2:
        # pad slot: any distinct cell position (zero weights, contributes 0);
        # prefer one that reuses an already-needed dx plane
        dxs = {c % 5 for c in rings}
        pad = next((c for c in range(25) if c not in rings and c % 5 in dxs),
                   next(c for c in range(25) if c not in rings))
        rings.append(pad)
        rings.sort(key=lambda c: (c % 5, c // 5))
    nh = len(rings) // 2
    pairs = [(rings[i], rings[i + nh]) for i in range(nh)]
    need_dx = sorted({c % 5 for c in rings})
    dx_slot = {d: i for i, d in enumerate(need_dx)}
    nbig = len(bigs)

    def cell_off(c, bi):
        return dx_slot[c % 5] * (UR * W) + (4 * bi + c // 5) * W

    # validate pair steps (static, positive, 16-aligned)
    for c1, c2 in pairs:
        step = cell_off(c2, 0) - cell_off(c1, 0)
        assert step > 0 and step % 16 == 0, (c1, c2, step)

    # stage-A weights in phase-major tap order for split DMA
    wa2 = np.zeros((COUT, 9, 2, COUT), np.float32)
    for j, pos in TAP_POS.items():
        jy, jx = j // 3, j % 3
        for h2 in range(2):
            wa2[:, pos, h2, :] = trans_w[h2 * 128:(h2 + 1) * 128, :, jy, jx]
    wa_b = np.ascontiguousarray(wa2).astype(BF16).reshape(COUT, 2 * 9 * COUT)

    wb_all, wr_all = [], []
    for n in range(N_BATCH):
        wb = np.zeros((2, nbig, COUT, COUT), np.float32)  # [half, s, o, c]
        for s, ci in enumerate(bigs):
            wb[:, s] = Weff[n, ci // 5, ci % 5].reshape(2, COUT, COUT)
        wb = wb.transpose(3, 0, 1, 2)                     # [c, half, s, o]
        wb_all.append(np.ascontiguousarray(wb).astype(BF16).reshape(COUT, -1))
        wr = np.zeros((max(len(pairs), 1), 2, 2, COUT, COUT), np.float32)
        for p, (c1, c2) in enumerate(pairs):
            for half in range(2):
                wr[p, half, 0] = RING_W_SCALE * \
                    Weff[n, c1 // 5, c1 % 5][128 * half:128 * (half + 1)].T
                wr[p, half, 1] = RING_W_SCALE * \
                    Weff[n, c2 // 5, c2 % 5][128 * half:128 * (half + 1)].T
        wr = wr.transpose(3, 0, 1, 2, 4)                  # [c, p, half, ksub, o]
        wr_all.append(np.ascontiguousarray(wr).astype(FP8).reshape(COUT, -1))

    in_maps = []
    for core in range(N_CORES):
        n, r = core // STRIPS, core % STRIPS
        xs = np.zeros((COUT, 2, XR, XC), np.float32)
        r0 = 16 * r - 2
        lo, hi = max(0, r0), min(H0, r0 + XR)
        for h2 in range(2):
            xs[:, h2, lo - r0:hi - r0, :W0] = xf[n, h2 * 128:(h2 + 1) * 128, lo:hi, :]
        # banded copy: band b holds xs rows XS_R0[b] .. XS_R0[b]+7, so each
        # stage-A band reads its own tile while the next band's DMA lands
        xsb = np.zeros((COUT, 3, 2, 8, XC), np.float32)
        for b, rb in enumerate(XS_R0):
            nr = min(8, XR - rb)
            xsb[:, b, :, :nr, :] = xs[:, :, rb:rb + nr, :]
        mk = np.full((COUT, 1), 0.0 if r == 0 else 1.0, np.float32)
        in_maps.append({
            "xs": np.ascontiguousarray(xsb.astype(BF16)
                                       .reshape(COUT, 3 * 2 * 8 * XC)),
            "wa": wa_b,
            "wb": wb_all[n],
            "wr": wr_all[n],
            "mk": mk,
        })
    return in_maps, ("v2", tuple(bigs), tuple(pairs), tuple(need_dx))


WG_CENTER = (6, 7, 8, 11, 12, 13, 16, 17, 18)


def _prep_in_maps_wg(xf, trans_w, oy, ox):
    """Winograd variant: valid when all effective cells fold into the center
    3x3 (tiny offsets).  Stage B = F(2,3) along rows: 12 half-size matmuls
    per block instead of 9 full-size.  Returns None if structure doesn't fit
    (caller falls back to v2)."""
    Weff = _w_eff(trans_w, oy, ox)
    norms = np.abs(Weff).reshape(N_BATCH, 25, -1).max(2)
    gmax = norms.max()
    bigs = sorted(int(c) for c in np.nonzero(norms.max(0) > 0.25 * gmax)[0])
    if not set(bigs) <= set(WG_CENTER):
        return None
    keep = (norms.max(0) > 0.004 * gmax) & (norms.max(0) <= 0.25 * gmax)
    rings = [int(c) for c in np.nonzero(keep)[0] if c not in bigs]
    if rings:
        if PRUNE_BUDGET <= 0:
            return None
        rings, Weff, _perr = _prune_rings(Weff, xf, trans_w, bigs, rings)
        if rings:
            return None

    # stage-A weights in phase-major tap order for split DMA (same as v2)
    wa2 = np.zeros((COUT, 9, 2, COUT), np.float32)
    for j, pos in TAP_POS.items():
        jy, jx = j // 3, j % 3
        for h2 in range(2):
            wa2[:, pos, h2, :] = trans_w[h2 * 128:(h2 + 1) * 128, :, jy, jx]
    wa_b = np.ascontiguousarray(wa2).astype(BF16).reshape(COUT, 2 * 9 * COUT)

    # Winograd-transformed stage-B weights: wg[c, half, k(4), kx(3), o]
    # k0 = w_dy0, k1 = (w0+w1+w2)/2, k2 = (w0-w1+w2)/2, k3 = w_dy2
    # (half-major so each half's weights ship as one contiguous DMA piece)
    wg_all = []
    for n in range(N_BATCH):
        wgl = np.zeros((COUT, 2, 4, 3, COUT), np.float32)
        for kx in range(3):
            w0 = Weff[n, 1, 1 + kx]     # [o(256), c(128)], shift dy=-1
            w1 = Weff[n, 2, 1 + kx]
            w2 = Weff[n, 3, 1 + kx]
            for k, wt in enumerate((w0, (w0 + w1 + w2) * 0.5,
                                    (w0 - w1 + w2) * 0.5, w2)):
                for half in range(2):
                    wgl[:, half, k, kx, :] = wt[128 * half:128 * (half + 1), :].T
        wg_all.append(np.ascontiguousarray(wgl).astype(BF16).reshape(COUT, -1))

    in_maps = []
    for core in range(N_CORES):
        n, r = core // STRIPS, core % STRIPS
        xs = np.zeros((COUT, 2, XR, XC), np.float32)
        r0 = 16 * r - 2
        lo, hi = max(0, r0), min(H0, r0 + XR)
        for h2 in range(2):
            xs[:, h2, lo - r0:hi - r0, :W0] = xf[n, h2 * 128:(h2 + 1) * 128, lo:hi, :]
        xsb = np.zeros((COUT, 3, 2, 8, XC), np.float32)
        for b, rb in enumerate(XS_R0):
            nr = min(8, XR - rb)
            xsb[:, b, :, :nr, :] = xs[:, :, rb:rb + nr, :]
        mk = np.full((COUT, 1), 0.0 if r == 0 else 1.0, np.float32)
        in_maps.append({
            "xs": np.ascontiguousarray(xsb.astype(BF16)
                                       .reshape(COUT, 3 * 2 * 8 * XC)),
            "wa": wa_b,
            "wg": wg_all[n],
            "mk": mk,
        })
    return in_maps, ("wg",)


def _prep_in_maps_hybrid(xf, trans_w, oy, ox, wa_b):
    S = HYBRID_S
    nslot = S + 2 * (9 - S)
    wb_all, bs_all, dsc_all, ofs_all = [], [], [], []
    for n in range(N_BATCH):
        wb = np.zeros((nslot, 2, COUT, COUT), np.float32)   # [slot, half, c, o]
        bs = np.zeros((9, 2), np.float32)
        dsc = np.zeros((max(S, 1), 2), np.float32)
        ofs = np.zeros((1, 9, 2), np.int32)
        for k in range(9):
            ky, kx = k // 3, k % 3
            ay = np.float32(ky - 1) + oy[n, k]
            ax = np.float32(kx - 1) + ox[n, k]
            Ay, Ax = int(np.floor(ay)), int(np.floor(ax))
            dy = float(ay) - Ay
            dx = float(ax) - Ax
            ofs[0, k] = (2 + Ay, 2 + Ax)
            bs[k] = (1.0 - dy, dy)
            wkT = np.stack([trans_w[h * 128:(h + 1) * 128, :, ky, kx].T
                            for h in range(2)])             # [half, c, o]
            if k < S:
                dsc[k] = (1.0 - dx, dx)
                wb[k] = wkT
            else:
                wb[S + 2 * (k - S) + 0] = (1.0 - dx) * wkT
                wb[S + 2 * (k - S) + 1] = dx * wkT
        wb = wb.transpose(2, 0, 1, 3)                       # [c, slot, half, o]
        wb_all.append(np.ascontiguousarray(wb).astype(BF16)
                      .reshape(COUT, nslot * 2 * COUT))
        bs_all.append(np.broadcast_to(bs.reshape(1, 9, 2),
                                      (COUT, 9, 2)).copy())
        dsc_all.append(np.broadcast_to(dsc.reshape(1, -1, 2),
                                       (COUT, max(S, 1), 2)).copy())
        ofs_all.append(ofs)

    in_maps = []
    for core in range(N_CORES):
        n, r = core // STRIPS, core % STRIPS
        xs = np.zeros((COUT, 2, XR, XC), np.float32)
        r0 = 16 * r - 2
        lo, hi = max(0, r0), min(H0, r0 + XR)
        for h2 in range(2):
            xs[:, h2, lo - r0:hi - r0, :W0] = xf[n, h2 * 128:(h2 + 1) * 128, lo:hi, :]
        mk = np.full((COUT, 1), 0.0 if r == 0 else 1.0, np.float32)
        in_maps.append({
            "xs": np.ascontiguousarray(xs.astype(BF16).reshape(COUT, 2 * XR * XC)),
            "wa": wa_b,
            "wb": wb_all[n],
            "mk": mk,
            "bs": bs_all[n].reshape(COUT, 18),
            "dsc": dsc_all[n].reshape(COUT, -1),
            "co": ofs_all[n],
        })
    return in_maps, nslot


# --------------------------------------------------------------------------
# device program (input-independent; same for all cores except r-dependent
# row validity -> handled by *uniform* structure: we compute all 36 up rows,
# rows outside [0,128) stay zero because their x inputs are zeroed host-side
# ... except parity bookkeeping differs per strip; we keep the program truly
# SPMD by computing the full 18 a'-rows per phase and masking via zero x.)
# --------------------------------------------------------------------------

def _build_nc_v2(key):
    """v3 device program: interleaved stage A bands / stage B block groups,
    static big + ring cells unified into one psum bank, fp16 output."""
    _, bigs, pairs, need_dx = key
    bigs, pairs, need_dx = list(bigs), list(pairs), list(need_dx)
    nbig, npair, ndx = len(bigs), len(pairs), len(need_dx)
    dx_slot = {d: i for i, d in enumerate(need_dx)}
    nc = bacc.Bacc("TRN2", target_bir_lowering=False, debug=False,
                   enable_asserts=False)

    xs_d = nc.dram_tensor("xs", [COUT, 3 * 2 * 8 * XC], mybir.dt.bfloat16,
                          kind="ExternalInput").ap()
    wa_d = nc.dram_tensor("wa", [COUT, 2 * 9 * COUT], mybir.dt.bfloat16,
                          kind="ExternalInput").ap()
    wb_d = nc.dram_tensor("wb", [COUT, 2 * nbig * COUT], mybir.dt.bfloat16,
                          kind="ExternalInput").ap()
    wr_d = nc.dram_tensor("wr", [COUT, max(npair, 1) * 2 * 2 * COUT],
                          mybir.dt.float8e4, kind="ExternalInput").ap()
    mk_d = nc.dram_tensor("mk", [COUT, 1], mybir.dt.float32,
                          kind="ExternalInput").ap()
    out_dt = {"f16": mybir.dt.float16, "bf16": mybir.dt.bfloat16,
              "f32": mybir.dt.float32}[V2_OUT]
    out_d = nc.dram_tensor("out", [CIN, OUT_R, W], out_dt,
                           kind="ExternalOutput").ap()

    with tile.TileContext(nc) as tc:
        with (
            tc.tile_pool(name="singles", bufs=1) as singles,
            tc.tile_pool(name="outp", bufs=4) as outp,
            tc.tile_pool(name="psA", bufs=4, space="PSUM") as psA,
            tc.tile_pool(name="psB", bufs=4, space="PSUM") as psB,
        ):
            xs_t = singles.tile([COUT, 3, 2, 8, XC], mybir.dt.bfloat16)
            wa_t = singles.tile([COUT, 9, 2, COUT], mybir.dt.bfloat16)
            wb_t = singles.tile([COUT, 2, nbig, COUT], mybir.dt.bfloat16)
            wr_t = singles.tile([COUT, max(npair, 1), 2, 2, COUT],
                                mybir.dt.float8e4)
            mk_t = singles.tile([COUT, 1], mybir.dt.float32)
            up_full = singles.tile([COUT, UR * UC + 12], mybir.dt.bfloat16)
            up_t = up_full[:, :UR * UC]
            upf_t = singles.tile([COUT, max(ndx, 1), UR, W], mybir.dt.float8e4)

            # ---- input DMA, ordered so the first stage-A matmuls (half 0,
            # band 0) can start as early as possible ----
            xs4 = xs_t[:]
            xs4_d = xs_d.rearrange("p (a b c d) -> p a b c d", a=3, b=2, c=8)
            wa_flat = wa_t[:].rearrange("p a b c -> p (a b c)")
            wb_flat = wb_t[:].rearrange("p a b c -> p (a b c)")
            # the three pieces gating the first matmuls go first, split
            # across both HWDGE queues (one trigger each ~0.7us):
            #   sync:   wa piece1 (tap j4, both halves), xs h1 band0, wa rest
            #   scalar: xs h0 band0, xs h0 rest, wb h0, xs h1 rest, wb h1
            nc.sync.dma_start(out=wa_flat[:, :WA_CUTS[1] * 2 * COUT],
                              in_=wa_d[:, :WA_CUTS[1] * 2 * COUT])
            nc.scalar.dma_start(out=xs4[:, 0], in_=xs4_d[:, 0])
            for c0, c1 in zip(WA_CUTS[1:-1], WA_CUTS[2:]):
                nc.sync.dma_start(out=wa_flat[:, c0 * 2 * COUT:c1 * 2 * COUT],
                                  in_=wa_d[:, c0 * 2 * COUT:c1 * 2 * COUT])
            nc.scalar.dma_start(out=xs4[:, 1], in_=xs4_d[:, 1])
            nc.scalar.dma_start(out=xs4[:, 2], in_=xs4_d[:, 2])
            nc.scalar.dma_start(out=wb_flat[:, :nbig * COUT],
                                in_=wb_d[:, :nbig * COUT])
            nc.scalar.dma_start(out=wb_flat[:, nbig * COUT:],
                                in_=wb_d[:, nbig * COUT:])
            if npair:
                nc.sync.dma_start(
                    out=wr_t[:].rearrange("p a b c d -> p (a b c d)"), in_=wr_d)
            nc.sync.dma_start(out=mk_t[:], in_=mk_d)

            up_w = up_t.rearrange("p (a q c r) -> p a q c r", q=2, c=66, r=2)
            up_r = up_t.rearrange("p (l c) -> p l c", c=132)
            # only the column margins and the tail pad are never written by
            # the stage-A scatter -- memset just those
            nc.vector.memset(up_r[:, :, 0:2], 0.0)
            nc.vector.memset(up_r[:, :, 130:132], 0.0)
            nc.vector.memset(up_full[:, UR * UC:], 0.0)
            upf_fl = upf_t[:].rearrange("p a b c -> p (a b c)")

            ytaps = {0: ((1, 0),), 1: ((2, 0), (0, 1))}
            band_blocks = ((0, 1), (2, 3, 4), (5, 6, 7))

            def cell_off(c, bi):
                return dx_slot[c % 5] * (UR * W) + (4 * bi + c // 5) * W

            def emit_phase(b, py, px):
                a0 = 6 * b
                rc = 6
                taps = [(jy, dy, jx, dx)
                        for jy, dy in ytaps[py] for jx, dx in ytaps[px]]
                ps = psA.tile([COUT, rc, 64], mybir.dt.float32, tag="psA",
                              name=f"psA_{b}_{py}_{px}")
                nmm = len(taps) * 2
                i = 0
                for h2 in range(2):
                    for (jy, dy, jx, dx) in taps:
                        r0x = a0 + 1 + dy - XS_R0[b]
                        nc.tensor.matmul(
                            ps[:, :rc, :],
                            lhsT=wa_t[:, TAP_POS[jy * 3 + jx], h2, :],
                            rhs=xs_t[:, b, h2, r0x:r0x + rc, dx:dx + 64],
                            start=(i == 0), stop=(i == nmm - 1),
                        )
                        i += 1
                nc.scalar.copy(
                    out=up_w[:, a0:a0 + rc, py, 1:65, px],
                    in_=ps[:, :rc, :],
                )

            def emit_block(bi):
                for half in range(2):
                    ps = psB.tile([COUT, RBLK, W], mybir.dt.float32,
                                  tag="psB", name=f"psB_{bi}_{half}")
                    mixed = V2_MIXED_GROUP and npair > 0
                    nmm = nbig + (npair if mixed else 0)
                    for s, ci in enumerate(bigs):
                        dyi, dxi = ci // 5, ci % 5
                        ys = 4 * bi + dyi
                        nc.tensor.matmul(
                            ps[:], lhsT=wb_t[:, half, s, :],
                            rhs=up_r[:, ys:ys + RBLK, dxi:dxi + W],
                            start=(s == 0), stop=(s == nmm - 1))
                    psr = ps if mixed else (
                        psA.tile([COUT, RBLK, W], mybir.dt.float32,
                                 tag="psA", name=f"psr_{bi}_{half}")
                        if npair else None)
                    for p, (c1, c2) in enumerate(pairs):
                        step = cell_off(c2, 0) - cell_off(c1, 0)
                        off = cell_off(c1, bi)
                        win = upf_fl[:, off:off + RBLK * W]
                        rhs = bass.AP(tensor=win.tensor, offset=win.offset,
                                      ap=[win.ap[0], [step, 2], win.ap[1]])
                        nc.tensor.matmul(
                            psr[:], lhsT=wr_t[:, p, half, :, :], rhs=rhs,
                            perf_mode=mybir.MatmulPerfMode.DoubleRow,
                            start=(False if mixed else p == 0),
                            stop=(nbig + p == nmm - 1) if mixed
                            else (p == npair - 1))
                    ob = outp.tile([COUT, RBLK, W], out_dt, tag="ob",
                                   name=f"ob_{bi}_{half}")
                    if V2_EVAC_ENG == "vector":
                        nc.vector.tensor_copy(ob[:], ps[:])
                    else:
                        nc.scalar.copy(out=ob[:], in_=ps[:])
                    if not mixed and npair:
                        nc.vector.scalar_tensor_tensor(
                            out=ob[:], in0=psr[:], scalar=1.0,
                            in1=ob[:], op0=mybir.AluOpType.mult,
                            op1=mybir.AluOpType.add)
                    nc.sync.dma_start(
                        out=out_d[128 * half:128 * (half + 1),
                                  RBLK * bi:RBLK * (bi + 1), :],
                        in_=ob[:])

            def emit_band_rest(b):
                if b == 0:
                    # zero the two bottom halo rows on the r=0 strip
                    nc.vector.tensor_scalar_mul(up_r[:, 0:2, :], up_r[:, 0:2, :],
                                                mk_t[:, 0:1])
                # fp8 ring planes for this band (x 1/RING_W_SCALE)
                for i, dxp in enumerate(need_dx):
                    src = up_r[:, 12 * b:12 * b + 12, dxp:dxp + W]
                    dst = upf_t[:, i, 12 * b:12 * b + 12, :]
                    if V2_VEC_PLANES and i % 2 == 1:
                        nc.vector.tensor_scalar_mul(dst, src,
                                                    1.0 / RING_W_SCALE)
                    else:
                        nc.scalar.mul(out=dst, in_=src, mul=1.0 / RING_W_SCALE)
                for bi in band_blocks[b]:
                    emit_block(bi)

            for b in range(3):
                for (py, px) in ((0, 0), (0, 1), (1, 0), (1, 1)):
                    emit_phase(b, py, px)
                emit_band_rest(b)

    nc.compile()
    return nc


def _build_nc_wg():
    """Winograd F(2,3)-rows device program.

    Stage A (transposed conv) unchanged.  Then per band, V planes
      V0[t] = u[2t+1]-u[2t+3], V1[t] = u[2t+2]+u[2t+3],
      V2[t] = u[2t+3]-u[2t+2], V3[t] = u[2t+2]-u[2t+4]   (u rows of `up`)
    are built on vector/gpsimd.  Stage B per (half, 4-row block):
      m_k = sum_kx wg[k,kx] @ V_k[2bi:2bi+2, kx+1 : kx+1+W]   (4 psum comps)
      out even rows = m0+m1+m2, odd rows = m1-m2-m3
    Combines: scalar evacuates m1,m3; DVE does the three psum-reading adds;
    gpsimd does the sbuf-only one.  PE sequence A0,A1,B0,A2,B1,B2 so V(b)
    always builds in the shadow of PE work on other data."""
    nc = bacc.Bacc("TRN2", target_bir_lowering=False, debug=False,
                   enable_asserts=False)

    xs_d = nc.dram_tensor("xs", [COUT, 3 * 2 * 8 * XC], mybir.dt.bfloat16,
                          kind="ExternalInput").ap()
    wa_d = nc.dram_tensor("wa", [COUT, 2 * 9 * COUT], mybir.dt.bfloat16,
                          kind="ExternalInput").ap()
    wg_d = nc.dram_tensor("wg", [COUT, 4 * 3 * 2 * COUT], mybir.dt.bfloat16,
                          kind="ExternalInput").ap()
    mk_d = nc.dram_tensor("mk", [COUT, 1], mybir.dt.float32,
                          kind="ExternalInput").ap()
    out_d = nc.dram_tensor("out", [CIN, OUT_R, W], mybir.dt.float16,
                           kind="ExternalOutput").ap()

    NT = OUT_R // 2            # 16 winograd tile rows
    mm = mybir.AluOpType.mult
    aa = mybir.AluOpType.add

    with tile.TileContext(nc) as tc:
        with (
            tc.tile_pool(name="singles", bufs=1) as singles,
            tc.tile_pool(name="outp", bufs=3) as outp,
            tc.tile_pool(name="evp", bufs=3) as evp,
            tc.tile_pool(name="psA", bufs=3, space="PSUM") as psA,
            tc.tile_pool(name="psB", bufs=1, space="PSUM") as psB,
        ):
            xs_t = singles.tile([COUT, 3, 2, 8, XC], mybir.dt.bfloat16)
            wa_t = singles.tile([COUT, 9, 2, COUT], mybir.dt.bfloat16)
            wg_t = singles.tile([COUT, 2, 4, 3, COUT], mybir.dt.bfloat16)
            mk_t = singles.tile([COUT, 1], mybir.dt.float32)
            up_t = singles.tile([COUT, UR * UC], mybir.dt.bfloat16)
            v_t = singles.tile([COUT, 4, NT, UC], mybir.dt.bfloat16)

            # ---- optional PE p-state warmup on zeroed dummy data ----
            if WG_WARMUP:
                wu_t = singles.tile([COUT, 384], mybir.dt.bfloat16)
                nc.vector.memset(wu_t[:], 0.0)
                for i in range(WG_WARMUP):
                    psw = psA.tile([COUT, 384], mybir.dt.float32, tag="psA",
                                   name=f"psw_{i}")
                    nc.tensor.matmul(psw[:], lhsT=wu_t[:, :COUT],
                                     rhs=wu_t[:], start=True, stop=True)

            # ---- input DMA, critical pieces first on both queues ----
            xs4 = xs_t[:]
            xs4_d = xs_d.rearrange("p (a b c d) -> p a b c d", a=3, b=2, c=8)
            wa_flat = wa_t[:].rearrange("p a b c -> p (a b c)")
            wg_flat = wg_t[:].rearrange("p a b c d -> p (a b c d)")
            nc.sync.dma_start(out=wa_flat[:, :WA_CUTS[1] * 2 * COUT],
                              in_=wa_d[:, :WA_CUTS[1] * 2 * COUT])
            nc.scalar.dma_start(out=xs4[:, 0], in_=xs4_d[:, 0])
            for c0, c1 in zip(WA_CUTS[1:-1], WA_CUTS[2:]):
                nc.sync.dma_start(out=wa_flat[:, c0 * 2 * COUT:c1 * 2 * COUT],
                                  in_=wa_d[:, c0 * 2 * COUT:c1 * 2 * COUT])
            nc.scalar.dma_start(out=xs4[:, 1], in_=xs4_d[:, 1])
            nc.sync.dma_start(out=mk_t[:], in_=mk_d)
            half_wg = 4 * 3 * COUT
            nc.scalar.dma_start(out=wg_flat[:, :half_wg],
                                in_=wg_d[:, :half_wg])
            nc.scalar.dma_start(out=xs4[:, 2], in_=xs4_d[:, 2])
            nc.sync.dma_start(out=wg_flat[:, half_wg:],
                              in_=wg_d[:, half_wg:])

            up_w = up_t.rearrange("p (a q c r) -> p a q c r", q=2, c=66, r=2)
            up_r = up_t.rearrange("p (l c) -> p l c", c=UC)
            up_pair = up_t.rearrange("p (l2 two c) -> p l2 two c",
                                     two=2, c=UC)
            nc.vector.memset(up_r[:, :, 0:2], 0.0)
            nc.vector.memset(up_r[:, :, 130:132], 0.0)

            ytaps = {0: ((1, 0),), 1: ((2, 0), (0, 1))}

            def emit_phase(b, py, px):
                a0 = 6 * b
                rc = 6
                taps = [(jy, dy, jx, dx)
                        for jy, dy in ytaps[py] for jx, dx in ytaps[px]]
                ps = psA.tile([COUT, rc, 64], mybir.dt.float32, tag="psA",
                              name=f"psA_{b}_{py}_{px}")
                nmm = len(taps) * 2
                i = 0
                for h2 in range(2):
                    for (jy, dy, jx, dx) in taps:
                        r0x = a0 + 1 + dy - XS_R0[b]
                        nc.tensor.matmul(
                            ps[:, :rc, :],
                            lhsT=wa_t[:, TAP_POS[jy * 3 + jx], h2, :],
                            rhs=xs_t[:, b, h2, r0x:r0x + rc, dx:dx + 64],
                            start=(i == 0), stop=(i == nmm - 1),
                        )
                        i += 1
                nc.scalar.copy(
                    out=up_w[:, a0:a0 + rc, py, 1:65, px],
                    in_=ps[:, :rc, :],
                )

            def emit_band_A(b):
                for (py, px) in ((0, 0), (0, 1), (1, 0), (1, 1)):
                    emit_phase(b, py, px)
                if b == 0:
                    nc.vector.tensor_scalar_mul(up_r[:, 0:2, :],
                                                up_r[:, 0:2, :], mk_t[:, 0:1])

            V_T0 = (0, 4, 10, 16)      # t-ranges per band

            def emit_V(b):
                t0, t1 = V_T0[b], V_T0[b + 1]
                n_ = t1 - t0
                # V0[t] = u[2t+1] - u[2t+3]
                nc.vector.scalar_tensor_tensor(
                    out=v_t[:, 0, t0:t1, :],
                    in0=up_pair[:, t0 + 1:t1 + 1, 1, :], scalar=-1.0,
                    in1=up_pair[:, t0:t1, 1, :], op0=mm, op1=aa)
                # V1[t] = u[2t+2] + u[2t+3]
                nc.vector.scalar_tensor_tensor(
                    out=v_t[:, 1, t0:t1, :],
                    in0=up_pair[:, t0 + 1:t1 + 1, 0, :], scalar=1.0,
                    in1=up_pair[:, t0 + 1:t1 + 1, 1, :], op0=mm, op1=aa)
                # V2[t] = u[2t+3] - u[2t+2]
                nc.gpsimd.tensor_tensor(
                    v_t[:, 2, t0:t1, :],
                    up_pair[:, t0 + 1:t1 + 1, 1, :],
                    up_pair[:, t0 + 1:t1 + 1, 0, :], mybir.AluOpType.subtract)
                # V3[t] = u[2t+2] - u[2t+4]
                nc.gpsimd.tensor_tensor(
                    v_t[:, 3, t0:t1, :],
                    up_pair[:, t0 + 1:t1 + 1, 0, :],
                    up_pair[:, t0 + 2:t1 + 2, 0, :], mybir.AluOpType.subtract)

            # output DMA groups (blocks per DMA, grouped within bands)
            OUT_GROUPS = ((0, 1), (2, 3), (4,), (5, 6), (7,))
            grp_of = {bi: g for g in OUT_GROUPS for bi in g}
            ob_tiles = {}

            def emit_block(bi, half):
                # matmul group order k1,k3,k0,k2 so the m1/m3 evacs and the
                # gpsimd o1 combine overlap the k0/k2 matmuls; after the last
                # group only the two DVE writes into ob remain.  One psum
                # tile per component so consumer reads never serialize
                # against later component matmuls.
                ps = [psB.tile([COUT, 2, W], mybir.dt.float32, tag=f"psB{k}",
                               name=f"psB{k}_{bi}_{half}",
                               bufs=(2 if k == 2 else 1)) for k in range(4)]
                g = grp_of[bi]
                if (g, half) not in ob_tiles:
                    ob_tiles[(g, half)] = outp.tile(
                        [COUT, len(g) * 4, W], mybir.dt.float16, tag="ob",
                        name=f"ob_{g[0]}_{half}", padded_shape=[COUT, 8, W])
                ob = ob_tiles[(g, half)]
                toff = 2 * (bi - g[0])
                obr = ob.rearrange("p (t s) c -> p t s c", s=2)
                m1s = evp.tile([COUT, 2, W], mybir.dt.float32, tag="m1s",
                               name=f"m1s_{bi}_{half}")
                m3s = evp.tile([COUT, 2, W], mybir.dt.float32, tag="m3s",
                               name=f"m3s_{bi}_{half}")
                e1 = evp.tile([COUT, 2, W], mybir.dt.float32, tag="e1",
                              name=f"e1_{bi}_{half}")
                o1 = evp.tile([COUT, 2, W], mybir.dt.float32, tag="o1",
                              name=f"o1_{bi}_{half}")

                def mmk(k):
                    for kx in range(3):
                        nc.tensor.matmul(
                            ps[k][:], lhsT=wg_t[:, half, k, kx, :],
                            rhs=v_t[:, k, 2 * bi:2 * bi + 2, kx + 1:kx + 1 + W],
                            start=(kx == 0), stop=(kx == 2))

                mmk(1)
                nc.scalar.copy(out=m1s[:], in_=ps[1][:])
                mmk(3)
                nc.scalar.copy(out=m3s[:], in_=ps[3][:])
                # o1 = m1 - m3 (sbuf-only, runs during k0/k2 matmuls)
                nc.gpsimd.tensor_tensor(o1[:], m1s[:], m3s[:],
                                        mybir.AluOpType.subtract)
                mmk(0)
                # e1 = m0 + m1 (runs during k2 matmuls)
                nc.vector.scalar_tensor_tensor(
                    out=e1[:], in0=ps[0][:], scalar=1.0, in1=m1s[:],
                    op0=mm, op1=aa)
                mmk(2)
                nc.vector.scalar_tensor_tensor(
                    out=obr[:, toff:toff + 2, 0, :], in0=ps[2][:], scalar=1.0,
                    in1=e1[:], op0=mm, op1=aa)
                nc.vector.scalar_tensor_tensor(
                    out=obr[:, toff:toff + 2, 1, :], in0=ps[2][:], scalar=-1.0,
                    in1=o1[:], op0=mm, op1=aa)
                if bi == g[-1]:
                    nc.sync.dma_start(
                        out=out_d[128 * half:128 * (half + 1),
                                  4 * g[0]:4 * g[0] + 4 * len(g), :],
                        in_=ob[:, :len(g) * 4, :])

            # ---- schedule: A0, A1, [V0] B0 B1, A2, [V1] B2 B3 B4, [V2] ... ----
            emit_band_A(0)
            emit_band_A(1)
            emit_V(0)
            for bi in (0, 1):
                for half in range(2):
                    emit_block(bi, half)
            emit_band_A(2)
            emit_V(1)
            for bi in (2, 3, 4):
                for half in range(2):
                    emit_block(bi, half)
            emit_V(2)
            for bi in (5, 6, 7):
                for half in range(2):
                    emit_block(bi, half)

    nc.compile()
    return nc


def _build_nc(ncell):
    if isinstance(ncell, tuple) and ncell[0] == "wg":
        return _build_nc_wg()
    if isinstance(ncell, tuple) and ncell[0] == "v2":
        return _build_nc_v2(ncell)
    fp8r = isinstance(ncell, tuple) and ncell[0] == "fp8r"
    if fp8r:
        bigs, pairs = list(ncell[1]), list(ncell[2])
        ncell = max(len(bigs), 1)
        cells, dyn = None, False
    elif isinstance(ncell, tuple):      # ("union", cell, cell, ...)
        cells = list(ncell[1:])
        ncell = len(cells)
        dyn = False
    else:
        cells = list(range(NCELL)) if VARIANT == "full25" else None
        dyn = VARIANT not in ("full25",)
    nc = bacc.Bacc("TRN2", target_bir_lowering=False, debug=False,
                   enable_asserts=False)

    xs_d = nc.dram_tensor("xs", [COUT, 3 * 2 * 8 * XC], mybir.dt.bfloat16,
                          kind="ExternalInput").ap()
    wa_d = nc.dram_tensor("wa", [COUT, 2 * 9 * COUT], mybir.dt.bfloat16,
                          kind="ExternalInput").ap()
    wb_d = nc.dram_tensor("wb", [COUT, ncell * 2 * COUT], mybir.dt.bfloat16,
                          kind="ExternalInput").ap()
    mk_d = nc.dram_tensor("mk", [COUT, 1], mybir.dt.float32,
                          kind="ExternalInput").ap()
    if fp8r:
        wr_d = nc.dram_tensor(
            "wr", [COUT, max(len(pairs), 1) * 2 * 2 * COUT],
            mybir.dt.float8e4, kind="ExternalInput").ap()
    hyb = VARIANT == "hybrid"
    S = HYBRID_S
    if hyb:
        co_d = nc.dram_tensor("co", [1, 9, 2], mybir.dt.int32,
                              kind="ExternalInput").ap()
        bs_d = nc.dram_tensor("bs", [COUT, 18], mybir.dt.float32,
                              kind="ExternalInput").ap()
        dsc_d = nc.dram_tensor("dsc", [COUT, 2 * max(S, 1)], mybir.dt.float32,
                               kind="ExternalInput").ap()
    elif dyn:
        co_d = nc.dram_tensor("co", [1, ncell, 2], mybir.dt.int32,
                              kind="ExternalInput").ap()
    out_d = nc.dram_tensor("out", [CIN, OUT_R, W], mybir.dt.float32,
                           kind="ExternalOutput").ap()

    with tile.TileContext(nc) as tc:
        with (
            tc.tile_pool(name="singles", bufs=1) as singles,
            tc.tile_pool(name="outp", bufs=4) as outp,
            tc.tile_pool(name="psB", bufs=4, space="PSUM") as psB,
            tc.tile_pool(name="psR", bufs=4, space="PSUM") as psR,
        ):
            xs_t = singles.tile([COUT, 3, 2, 8, XC], mybir.dt.bfloat16)
            wa_t = singles.tile([COUT, 9, 2, COUT], mybir.dt.bfloat16)
            wb_t = singles.tile([COUT, ncell, 2, COUT], mybir.dt.bfloat16)
            mk_t = singles.tile([COUT, 1], mybir.dt.float32)
            # +12 pad: hybrid vy reads may run a few elements past the last
            # row (col-window spill); padded region is zeroed, never consumed
            up_full = singles.tile([COUT, UR * UC + 12], mybir.dt.bfloat16)
            up_t = up_full[:, :UR * UC]

            # stage-A critical inputs split across both HWDGE queues; xs is
            # further split by row band so the first stage-A band can start
            # after ~0.3MB instead of the whole tensor.  Band a0 reads xs rows
            # a0+1+dy (dy<=1), so rows [0,9) cover band 0, [9,20) the rest.
            xs4 = xs_t[:]
            xs4_d = xs_d.rearrange("p (a b c d) -> p a b c d", a=3, b=2, c=8)
            for h2 in range(2):
                eng = nc.sync if h2 == 0 else nc.scalar
                eng.dma_start(out=xs4[:, h2, 0:9, :], in_=xs4_d[:, h2, 0:9, :])
            nc.sync.dma_start(out=wa_t[:].rearrange("p a b c -> p (a b c)"), in_=wa_d)
            for h2 in range(2):
                eng = nc.scalar if h2 == 0 else nc.sync
                eng.dma_start(out=xs4[:, h2, 9:, :], in_=xs4_d[:, h2, 9:, :])
            nc.sync.dma_start(out=mk_t[:], in_=mk_d)
            wb_flat = wb_t[:].rearrange("p a b c -> p (a b c)")
            nc.scalar.dma_start(out=wb_flat, in_=wb_d)
            if fp8r:
                wr_t = singles.tile([COUT, max(len(pairs), 1), 2, 2, COUT],
                                    mybir.dt.float8e4)
                nc.sync.dma_start(
                    out=wr_t[:].rearrange("p a b c d -> p (a b c d)"), in_=wr_d)
                upf_t = singles.tile([COUT, 5, UR, W], mybir.dt.float8e4)
            if hyb:
                co_t = singles.tile([1, 9, 2], mybir.dt.int32)
                bs_t = singles.tile([COUT, 9, 2], mybir.dt.float32)
                dsc_t = singles.tile([COUT, max(S, 1), 2], mybir.dt.float32)
                nc.sync.dma_start(out=co_t[:].rearrange("p a b -> p (a b)"),
                                  in_=co_d.rearrange("p a b -> p (a b)"))
                nc.sync.dma_start(out=bs_t[:].rearrange("p a b -> p (a b)"),
                                  in_=bs_d)
                nc.sync.dma_start(out=dsc_t[:].rearrange("p a b -> p (a b)"),
                                  in_=dsc_d)
            elif dyn:
                co_t = singles.tile([1, ncell, 2], mybir.dt.int32)
                nc.sync.dma_start(out=co_t[:].rearrange("p a b -> p (a b)"),
                                  in_=co_d.rearrange("p a b -> p (a b)"))

            # zero the up tile (margins + potentially-invalid rows)
            nc.vector.memset(up_full[:], 0.0)

            # views of up: [p, a'(18), q(2), cc(66), r(2)] for phase writes,
            # [p, l(36), c(132)] for stage-B reads
            up_w = up_t.rearrange("p (a q c r) -> p a q c r", q=2, c=66, r=2)
            up_r = up_t.rearrange("p (l c) -> p l c", c=132)

            # ---- stage A: transposed conv -> up ----
            # row-major (a0 outer) so each 12-row band of up completes early;
            # for fp8r the band's fp8 casts are emitted right behind it, so
            # the ring matmuls never wait on a late cast burst
            ytaps = {0: ((1, 0),), 1: ((2, 0), (0, 1))}
            if fp8r:
                need_dx = sorted({c % 5 for pr in pairs for c in pr})
            for a0 in range(0, 18, 6):
                rc = 6
                for py in (0, 1):
                    for px in (0, 1):
                        taps = [(jy, dy, jx, dx)
                                for jy, dy in ytaps[py] for jx, dx in ytaps[px]]
                        # stage A borrows the ring pool (idle here) so its
                        # evacuations never block stage-B big-cell psum slots
                        pool = psR if fp8r else psB
                        ps = pool.tile([COUT, 6, 64], mybir.dt.float32,
                                       tag="psR" if fp8r else "psB")
                        nmm = len(taps) * 2
                        i = 0
                        for (jy, dy, jx, dx) in taps:
                            for h2 in range(2):
                                r0x = a0 + 1 + dy - XS_R0[b]
                                nc.tensor.matmul(
                                    ps[:, :rc, :],
                                    lhsT=wa_t[:, TAP_POS[jy * 3 + jx], h2, :],
                                    rhs=xs_t[:, b, h2, r0x:r0x + rc,
                                             dx:dx + 64],
                                    start=(i == 0), stop=(i == nmm - 1),
                                )
                                i += 1
                        # scatter phase result into up (cast to bf16)
                        nc.scalar.copy(
                            out=up_w[:, a0:a0 + rc, py, 1:65, px],
                            in_=ps[:, :rc, :],
                        )
                if a0 == 0:
                    # zero the bottom two halo rows on the r=0 strip (g=-2,-1):
                    # the phase formula extended below the image is invalid there
                    nc.vector.tensor_scalar_mul(up_r[:, 0:2, :], up_r[:, 0:2, :],
                                                mk_t[:, 0:1])
                if fp8r:
                    for dx in need_dx:
                        nc.scalar.copy(
                            out=upf_t[:, dx, 2 * a0:2 * a0 + 12, :],
                            in_=up_r[:, 2 * a0:2 * a0 + 12, dx:dx + W])

            # ---- stage B: effective-cell conv -> out ----
            if fp8r:
                _stage_b_fp8r(nc, tc, up_r, upf_t, wb_t, wr_t, bigs, pairs,
                              psB, psR, outp, out_d)
            elif hyb:
                with (
                    tc.tile_pool(name="vyp", bufs=2) as vyp,
                    tc.tile_pool(name="smp", bufs=2) as smp,
                ):
                    # per-tap (row, col) bases into vector-engine registers
                    rvs = [nc.vector.value_load(co_t[0:1, k, 0:1],
                                                min_val=0, max_val=3)
                           for k in range(9)]
                    cvs = [nc.vector.value_load(co_t[0:1, k, 1:2],
                                                min_val=0, max_val=3)
                           for k in range(9)]
                    mm = mybir.AluOpType.mult
                    aa = mybir.AluOpType.add
                    up_fl = up_full[:]
                    for sb in range(OUT_R // SBR):
                        vys, samps = [], []
                        for k in range(9):
                            vy = vyp.tile([COUT, SBR, UC], mybir.dt.bfloat16,
                                          tag=f"vy{k}")
                            # [SBR rows x UC cols] shifted window == contiguous
                            # flat block of SBR*UC elements
                            base = rvs[k] * UC + cvs[k] + (SBR * sb) * UC
                            i0 = up_fl[:, bass.ds(base, SBR * UC)].rearrange(
                                "p (a b) -> p a b", b=UC)
                            i1 = up_fl[:, bass.ds(base + UC, SBR * UC)].rearrange(
                                "p (a b) -> p a b", b=UC)
                            nc.vector.tensor_scalar_mul(vy[:], i0, bs_t[:, k, 0:1])
                            nc.vector.scalar_tensor_tensor(
                                out=vy[:], in0=i1, scalar=bs_t[:, k, 1:2],
                                in1=vy[:], op0=mm, op1=aa)
                            vys.append(vy)
                        for k in range(S):
                            sa = smp.tile([COUT, SBR, W], mybir.dt.bfloat16,
                                          tag=f"sa{k}")
                            nc.vector.tensor_scalar_mul(
                                sa[:], vys[k][:, :, 0:W], dsc_t[:, k, 0:1])
                            nc.vector.scalar_tensor_tensor(
                                out=sa[:], in0=vys[k][:, :, 1:W + 1],
                                scalar=dsc_t[:, k, 1:2], in1=sa[:],
                                op0=mm, op1=aa)
                            samps.append(sa)
                        for sub in range(SBR // RBLK):
                            rs = slice(RBLK * sub, RBLK * (sub + 1))
                            bi = (SBR * sb) // RBLK + sub
                            for half in range(2):
                                ps = psB.tile([COUT, RBLK, W], mybir.dt.float32,
                                              tag="psB")
                                nmm = S + 2 * (9 - S)
                                si = 0
                                for k in range(9):
                                    if k < S:
                                        rhss = [samps[k][:, rs, :]]
                                    else:
                                        rhss = [vys[k][:, rs, 0:W],
                                                vys[k][:, rs, 1:W + 1]]
                                    for rhs in rhss:
                                        nc.tensor.matmul(
                                            ps[:], lhsT=wb_t[:, si, half, :],
                                            rhs=rhs, start=(si == 0),
                                            stop=(si == nmm - 1))
                                        si += 1
                                ob = outp.tile([COUT, RBLK, W], mybir.dt.float32,
                                               tag="ob")
                                nc.scalar.copy(out=ob[:], in_=ps[:])
                                nc.sync.dma_start(
                                    out=out_d[128 * half:128 * (half + 1),
                                              RBLK * bi:RBLK * (bi + 1), :],
                                    in_=ob[:])
            else:
                if dyn:
                    # per-slot (row, col) bases into tensor-engine registers
                    rvs = [nc.tensor.value_load(co_t[0:1, ci, 0:1],
                                                min_val=0, max_val=4)
                           for ci in range(ncell)]
                    cvs = [nc.tensor.value_load(co_t[0:1, ci, 1:2],
                                                min_val=0, max_val=4)
                           for ci in range(ncell)]
                for bi in range(OUT_R // RBLK):
                    for half in range(2):
                        ps = psB.tile([COUT, RBLK, W], mybir.dt.float32, tag="psB")
                        for ci in range(ncell):
                            if dyn:
                                rhs = up_r[:, bass.ds(rvs[ci] + 4 * bi, RBLK),
                                           bass.ds(cvs[ci], W)]
                            else:
                                dyi, dxi = cells[ci] // 5, cells[ci] % 5
                                ys = 4 * bi + dyi  # up row = o_l + 2 + (dyi-2)
                                rhs = up_r[:, ys:ys + RBLK, dxi:dxi + W]
                            nc.tensor.matmul(
                                ps[:],
                                lhsT=wb_t[:, ci, half, :],
                                rhs=rhs,
                                start=(ci == 0), stop=(ci == ncell - 1),
                            )
                        ob = outp.tile([COUT, RBLK, W], mybir.dt.float32, tag="ob")
                        nc.scalar.copy(out=ob[:], in_=ps[:])
                        nc.sync.dma_start(
                            out=out_d[128 * half:128 * (half + 1),
                                      4 * bi:4 * bi + RBLK, :],
                            in_=ob[:],
                        )

    nc.compile()
    return nc


def _stage_b_fp8r(nc, tc, up_r, upf_t, wb_t, wr_t, bigs, pairs,
                  psB, psR, outp, out_d):
    """Stage B with big cells in bf16 and ring-cell pairs in fp8 DoubleRow.

    upf_t[dx] holds a margin-free fp8 copy of up cols [dx, dx+128), so every
    cell window is a contiguous 512-element block and pair steps are
    automatically 16-aligned (multiples of 128)."""
    mm = mybir.AluOpType.mult
    aa = mybir.AluOpType.add

    # (fp8 casts of up are emitted inline with stage A, band by band)

    upf_fl = upf_t[:].rearrange("p a b c -> p (a b c)")

    def cell_off(c, bi):
        return (c % 5) * (UR * W) + ((4 * bi) + (c // 5)) * W

    G = 2  # blocks per weight-reuse group
    for half in range(2):
        for bg in range(OUT_R // RBLK // G):
            pscs = [psB.tile([COUT, RBLK, W], mybir.dt.float32, tag="psB",
                             name=f"psc_{half}_{bg}_{g}") for g in range(G)]
            for si, ci in enumerate(bigs):
                dyi, dxi = ci // 5, ci % 5
                for g in range(G):
                    bi = G * bg + g
                    ys = 4 * bi + dyi
                    nc.tensor.matmul(
                        pscs[g][:], lhsT=wb_t[:, si, half, :],
                        rhs=up_r[:, ys:ys + RBLK, dxi:dxi + W],
                        start=(si == 0), stop=(si == len(bigs) - 1))
            psrs = None
            if pairs:
                psrs = [psR.tile([COUT, RBLK, W], mybir.dt.float32, tag="psR",
                                 name=f"psr_{half}_{bg}_{g}") for g in range(G)]
                for p, (c1, c2) in enumerate(pairs):
                    step = cell_off(c2, 0) - cell_off(c1, 0)
                    assert step > 0 and step % 16 == 0
                    for g in range(G):
                        bi = G * bg + g
                        win = upf_fl[:, cell_off(c1, bi):cell_off(c1, bi) + RBLK * W]
                        rhs = bass.AP(tensor=win.tensor, offset=win.offset,
                                      ap=[win.ap[0], [step, 2], win.ap[1]])
                        nc.tensor.matmul(
                            psrs[g][:], lhsT=wr_t[:, p, half, :, :], rhs=rhs,
                            perf_mode=mybir.MatmulPerfMode.DoubleRow,
                            start=(p == 0), stop=(p == len(pairs) - 1))
            for g in range(G):
                bi = G * bg + g
                ob = outp.tile([COUT, RBLK, W], mybir.dt.float32, tag="ob")
                nc.scalar.copy(out=ob[:], in_=pscs[g][:])
                if pairs:
                    # TensorScalarPtr may read only one PSUM input
                    nc.vector.scalar_tensor_tensor(
                        out=ob[:], in0=psrs[g][:], scalar=1.0 / RING_SCALE,
                        in1=ob[:], op0=mm, op1=aa)
                nc.sync.dma_start(
                    out=out_d[128 * half:128 * (half + 1),
                              RBLK * bi:RBLK * (bi + 1), :],
                    in_=ob[:])


# --------------------------------------------------------------------------
# entry point
# --------------------------------------------------------------------------

def kernel(x, lateral_feat, trans_w, off_w1, off_b1, off_w2, off_b2):
    x = np.asarray(x)
    oy, ox = _offsets_from_inputs(np.asarray(lateral_feat), np.asarray(off_w1),
                                  np.asarray(off_b1), np.asarray(off_w2),
                                  np.asarray(off_b2))
    in_maps, ncell = _prep_in_maps(x, np.asarray(trans_w), oy, ox)

    key = (VARIANT, ncell)
    if key not in _CACHED_NC:
        _CACHED_NC[key] = _build_nc(ncell)
    nc = _CACHED_NC[key]

    res = run_bass_kernel_spmd(nc, in_maps, core_ids=list(range(N_CORES)))

    out = np.empty((N_BATCH, CIN, H, W), np.float32)
    for core in range(N_CORES):
        n, r = core // STRIPS, core % STRIPS
        out[n, :, OUT_R * r:OUT_R * (r + 1), :] = res.results[core]["out"]
    return out



# revision 17
# speedup vs baseline: 1.5331x; 1.0196x over previous
"""Trainium2 Bass kernel for nn_DeformableTransposedConv.

Pipeline (per the reference):
  up  = ConvTranspose2d(x, trans_w, stride=2, pad=1, outpad=1)   # [N,128,128,128]
  off = tanh(conv(relu(conv(lateral_feat, w1)), w2))             # [N,18,1,1] -> broadcast
  out = deform_conv2d(up, off, trans_w, pad=1)                   # [N,256,128,128]

Key structure exploited:
  * The offsets are constant over space (1x1 lateral input broadcast), so the
    bilinear deformable gather collapses to a per-batch 5x5 conv with
    "effective" weights W_eff[n] built host-side from trans_w and the (tiny)
    offsets.  The device computes:
        out[n] = sum_{dy,dx in 5x5} W_eff[n,dy,dx] @ shift(up[n], dy, dx)
    as PSUM-accumulated matmuls over the 128 up-channels.
  * The stride-2 transposed conv splits into 4 phase sub-convs with
    {1,2,2,4} taps, each a PSUM-accumulated matmul over the 256 x-channels.

Sharding: 8 cores = 2 batches x 4 row-strips of 32 output rows.  Each core
computes out[n, :, 32r:32r+32, :] from a 20-row slice of x (with halo).
All weights / layout prep / zero padding is done host-side; the NEFF is
input-independent (weights and data are ExternalInputs).
"""

import numpy as np
import ml_dtypes

import concourse.bass as bass
import concourse.tile as tile
from concourse import bacc, mybir
from concourse.bass_utils import run_bass_kernel_spmd

BF16 = ml_dtypes.bfloat16

# ---- problem constants (hardcoded per contract) ----
N_BATCH = 2
CIN = 256
COUT = 128          # up channels
K = 3
PAD = 1
H0 = W0 = 64        # x spatial
H = W = 128         # up / out spatial
N_CORES = 8
STRIPS = 4          # row strips per batch
OUT_R = 32          # output rows per strip

# SBUF layout constants
XR, XC = 20, 66     # x tile rows (16 + 2 halo each side), cols (64 + 1 pad + 1 align)
UR, UC = 36, 132    # up tile rows (32 + 2 halo each side), cols (128 + 2 + 2)
NCELL = 25          # 5x5 effective deform kernel
RBLK = 4            # output rows per stage-B block (4*128 = 512 = one PSUM bank)

# stage-B variant:
#   "full25" = static 5x5 effective conv (25 matmul terms / block)
#   "slots"  = dynamic cell slots (pruned zero cells, runtime offsets)
#   "hybrid" = DVE bilinear blends + matmuls (y-blend on DVE for all taps;
#              x-blend on DVE for the first HYBRID_S taps, folded into scaled
#              weights for the rest)
import os as _os
VARIANT = _os.environ.get("KERNEL_VARIANT", "wg")
WG_WARMUP = int(_os.environ.get("WG_WARMUP", "0"))   # p-state warmup matmuls
HYBRID_S = int(_os.environ.get("HYBRID_S", "4"))
SBR = 8             # hybrid blend superblock rows (2 PSUM blocks)
PLR = 32            # v2: ring-plane rows (blocks 0..7 read plane rows 4bi..4bi+3)
V2_VEC_PLANES = int(_os.environ.get("V2_VEC_PLANES", "1"))  # 0=scalar,1=split,2=vector
V2_MIXED_GROUP = _os.environ.get("V2_MIXED_GROUP", "1") == "1"
V2_OUT = _os.environ.get("V2_OUT", "f16")
V2_NO_EVAC = _os.environ.get("V2_NO_EVAC", "0") == "1"   # timing probe only
V2_NO_PLANES = _os.environ.get("V2_NO_PLANES", "0") == "1"  # timing probe only
V2_NO_ODMA = _os.environ.get("V2_NO_ODMA", "0") == "1"      # timing probe only
V2_EVAC_ENG = _os.environ.get("V2_EVAC_ENG", "vector")

_CACHED_NC = {}


# --------------------------------------------------------------------------
# host-side preparation
# --------------------------------------------------------------------------

def _offsets_from_inputs(lateral_feat, off_w1, off_b1, off_w2, off_b2):
    """Tiny offset MLP (conv on 1x1 spatial input == center-tap matmul)."""
    lf = lateral_feat[:, :, 0, 0].astype(np.float32)                    # [N,128]
    h = np.maximum(0.0, lf @ off_w1[:, :, 1, 1].T.astype(np.float32)
                   + off_b1.astype(np.float32))                         # [N,64]
    off = np.tanh(h @ off_w2[:, :, 1, 1].T.astype(np.float32)
                  + off_b2.astype(np.float32)).astype(np.float32)       # [N,18]
    oy = off.reshape(-1, K * K, 2)[:, :, 0]
    ox = off.reshape(-1, K * K, 2)[:, :, 1]
    return oy, ox


def _w_eff(trans_w, oy, ox):
    """Effective 5x5 deform weights. Returns [N, 5, 5, 256(o), 128(c)] f32."""
    n_b = oy.shape[0]
    Weff = np.zeros((n_b, 5, 5, CIN, COUT), np.float32)
    for n in range(n_b):
        for k in range(K * K):
            ky, kx = k // K, k % K
            ay = np.float32(ky - 1) + oy[n, k]
            ax = np.float32(kx - 1) + ox[n, k]
            Ay, Ax = int(np.floor(ay)), int(np.floor(ax))
            dy = float(ay) - Ay
            dx = float(ax) - Ax
            tap = trans_w[:, :, ky, kx].astype(np.float32)
            for cy, wy in ((0, 1.0 - dy), (1, dy)):
                for cx, wx in ((0, 1.0 - dx), (1, dx)):
                    w = wy * wx
                    if w != 0.0:
                        Weff[n, Ay + cy + 2, Ax + cx + 2] += w * tap
    return Weff


def _prep_in_maps(x, trans_w, oy, ox):
    """Build the per-core input dicts (already bf16, padded, SBUF-layouts).
    Returns (in_maps, ncell) where ncell is the stage-B slot count."""
    xf = x.astype(np.float32)

    # stage-A weights, shared by all cores: wa[k, h2, j, m]
    wa = np.zeros((COUT, 2, 9, COUT), np.float32)
    for h2 in range(2):
        for j in range(9):
            jy, jx = j // 3, j % 3
            # lhsT[K=cin(128), M=cout(128)] = trans_w[h2*128+kk, m, jy, jx]
            wa[:, h2, j, :] = trans_w[h2 * 128:(h2 + 1) * 128, :, jy, jx]
    wa_b = wa.astype(BF16).reshape(COUT, 2 * 9 * COUT)

    if VARIANT == "hybrid":
        return _prep_in_maps_hybrid(xf, trans_w, oy, ox, wa_b)
    if VARIANT == "fp8r":
        return _prep_in_maps_fp8r(xf, trans_w, oy, ox, wa_b)
    if VARIANT == "wg":
        r = _prep_in_maps_wg(xf, trans_w, oy, ox)
        if r is not None:
            return r
        return _prep_in_maps_v2(xf, trans_w, oy, ox, wa_b)
    if VARIANT == "v2":
        return _prep_in_maps_v2(xf, trans_w, oy, ox, wa_b)

    # stage-B weights per batch
    Weff = _w_eff(trans_w, oy, ox)                      # [N,5,5,256,128]
    wb_all, co_all = [], []
    if VARIANT == "full25":
        ncell = NCELL
        for n in range(N_BATCH):
            wb = Weff[n].reshape(NCELL, 2, COUT, COUT)   # [cell, half, o(128), c]
            wb = wb.transpose(3, 0, 1, 2)                # [c, cell, half, o]
            wb_all.append(np.ascontiguousarray(wb).astype(BF16)
                          .reshape(COUT, NCELL * 2 * COUT))
            co_all.append(None)
    elif VARIANT == "union":
        # static program specialized on the union of nonzero cells across
        # batches (compile cache keyed on the union tuple)
        nz = [np.nonzero(np.abs(Weff[n]).reshape(25, -1).max(1) > 0)[0]
              for n in range(N_BATCH)]
        union = sorted(set(int(c) for z in nz for c in z))
        ncell = ("union",) + tuple(union)
        for n in range(N_BATCH):
            wb = np.zeros((len(union), 2, COUT, COUT), np.float32)
            for s, ci in enumerate(union):
                wb[s] = Weff[n, ci // 5, ci % 5].reshape(2, COUT, COUT)
            wb = wb.transpose(3, 0, 1, 2)
            wb_all.append(np.ascontiguousarray(wb).astype(BF16)
                          .reshape(COUT, len(union) * 2 * COUT))
            co_all.append(None)
    else:  # "slots": pruned nonzero cells, offsets shipped as data
        nz = [np.nonzero(np.abs(Weff[n]).reshape(25, -1).max(1) > 0)[0]
              for n in range(N_BATCH)]
        ncell = max(len(z) for z in nz)
        for n in range(N_BATCH):
            cells = list(nz[n]) + [12] * (ncell - len(nz[n]))  # pad w/ center
            wb = np.zeros((ncell, 2, COUT, COUT), np.float32)
            co = np.zeros((1, ncell, 2), np.int32)
            for s, ci in enumerate(cells):
                dyi, dxi = ci // 5, ci % 5
                if s < len(nz[n]):
                    wb[s] = Weff[n, dyi, dxi].reshape(2, COUT, COUT)
                co[0, s] = (dyi, dxi)
            wb = wb.transpose(3, 0, 1, 2)                # [c, slot, half, o]
            wb_all.append(np.ascontiguousarray(wb).astype(BF16)
                          .reshape(COUT, ncell * 2 * COUT))
            co_all.append(co)

    in_maps = []
    for core in range(N_CORES):
        n, r = core // STRIPS, core % STRIPS
        # x slice with halo: global x rows [16r-2, 16r+18)
        xs = np.zeros((COUT, 2, XR, XC), np.float32)
        r0 = 16 * r - 2
        lo, hi = max(0, r0), min(H0, r0 + XR)
        for h2 in range(2):
            xs[:, h2, lo - r0:hi - r0, :W0] = xf[n, h2 * 128:(h2 + 1) * 128, lo:hi, :]
        # bottom-halo validity mask: strip 0 must zero up rows g=-2,-1 which
        # the phase formula would otherwise fill with spurious values
        mk = np.full((COUT, 1), 0.0 if r == 0 else 1.0, np.float32)
        im = {
            "xs": np.ascontiguousarray(xs.astype(BF16).reshape(COUT, 2 * XR * XC)),
            "wa": wa_b,
            "wb": wb_all[n],
            "mk": mk,
        }
        if co_all[n] is not None:
            im["co"] = co_all[n]
        in_maps.append(im)
    return in_maps, ncell


FP8 = ml_dtypes.float8_e4m3
RING_SCALE = 256.0


def _prep_in_maps_fp8r(xf, trans_w, oy, ox, wa_b):
    """Union cells; big cells in bf16, small 'ring' cells paired into fp8
    DoubleRow matmuls (weights scaled by RING_SCALE)."""
    Weff = _w_eff(trans_w, oy, ox)                       # [N,5,5,256,128]
    norms = np.abs(Weff).reshape(N_BATCH, 25, -1).max(2)  # [N,25]
    union = sorted(set(np.nonzero(norms.max(0) > 0)[0].tolist()))
    thr = 0.25 * norms.max()
    bigs = [c for c in union if norms[:, c].max() > thr]
    rings = [c for c in union if c not in bigs]
    if len(rings) % 2:
        bigs.append(rings.pop())                          # odd leftover -> bf16
    # order by window offset (dx major, dy minor); pair far-apart cells so the
    # two DoubleRow K-group windows never overlap (overlapping windows were
    # measured ~1.7x slower on the PE)
    rings.sort(key=lambda c: (c % 5, c // 5))
    nh = len(rings) // 2
    pairs = [(rings[i], rings[i + nh]) for i in range(nh)]
    bigs = sorted(bigs)

    wb_all, wr_all = [], []
    for n in range(N_BATCH):
        wb = np.zeros((max(len(bigs), 1), 2, COUT, COUT), np.float32)
        for s, ci in enumerate(bigs):
            wb[s] = Weff[n, ci // 5, ci % 5].reshape(2, COUT, COUT)
        wb = wb.transpose(3, 0, 1, 2)                     # [c, slot, half, o]
        wb_all.append(np.ascontiguousarray(wb).astype(BF16)
                      .reshape(COUT, -1))
        wr = np.zeros((max(len(pairs), 1), 2, 2, COUT, COUT), np.float32)
        for p, (c1, c2) in enumerate(pairs):
            for half in range(2):
                wr[p, half, 0] = RING_SCALE * \
                    Weff[n, c1 // 5, c1 % 5][128 * half:128 * (half + 1)].T
                wr[p, half, 1] = RING_SCALE * \
                    Weff[n, c2 // 5, c2 % 5][128 * half:128 * (half + 1)].T
        # wr[p, half, ksub, c, o] -> [c, p, half, ksub, o]
        wr = wr.transpose(3, 0, 1, 2, 4)
        wr_all.append(np.ascontiguousarray(wr).astype(FP8).reshape(COUT, -1))

    in_maps = []
    for core in range(N_CORES):
        n, r = core // STRIPS, core % STRIPS
        xs = np.zeros((COUT, 2, XR, XC), np.float32)
        r0 = 16 * r - 2
        lo, hi = max(0, r0), min(H0, r0 + XR)
        for h2 in range(2):
            xs[:, h2, lo - r0:hi - r0, :W0] = xf[n, h2 * 128:(h2 + 1) * 128, lo:hi, :]
        mk = np.full((COUT, 1), 0.0 if r == 0 else 1.0, np.float32)
        in_maps.append({
            "xs": np.ascontiguousarray(xs.astype(BF16).reshape(COUT, 2 * XR * XC)),
            "wa": wa_b,
            "wb": wb_all[n],
            "wr": wr_all[n],
            "mk": mk,
        })
    return in_maps, ("fp8r", tuple(bigs), tuple(pairs))


RING_W_SCALE = 16.0     # ring weights x16, up fp8 copies x1/16 -> product x1
TAP_ORDER = (4, 3, 5, 1, 7, 0, 2, 6, 8)   # phase-major: p00|p01|p10|p11
TAP_POS = {j: i for i, j in enumerate(TAP_ORDER)}
WA_CUTS = (0, 1, 3, 5, 9)                 # DMA piece boundaries in TAP_ORDER
XS_R0 = (0, 7, 13)                        # first xs row held by each band tile
PRUNE_BUDGET = float(_os.environ.get("V2_PRUNE_BUDGET", "0.012"))


def _prune_rings(Weff, xf, trans_w, bigs, rings):
    """Exact-error greedy pruning: for each ring cell (ascending magnitude)
    try dropping it or folding its weights into an adjacent kept cell; accept
    while the accumulated absmax output error stays under PRUNE_BUDGET.
    Returns (rings_kept, Weff_adjusted, err_bound)."""
    N, H2 = N_BATCH, H
    # host up[n]: transposed conv, padded by 2 on each side for cell shifts
    upp = np.zeros((N, COUT, H2 + 4, W + 4), np.float32)
    for n in range(N):
        # up[m, g, h] = sum_{jy,jx,c} w[c,m,jy,jx] x[c,(g+1-jy)/2,(h+1-jx)/2]
        for jy in range(3):
            for jx in range(3):
                w = trans_w[:, :, jy, jx].astype(np.float32)      # [c, m]
                # valid g: g+1-jy even and 0 <= (g+1-jy)//2 < 64
                gs = np.arange(H2)
                ok_g = ((gs + 1 - jy) % 2 == 0) & ((gs + 1 - jy) // 2 >= 0) \
                    & ((gs + 1 - jy) // 2 < H0)
                hs = np.arange(W)
                ok_h = ((hs + 1 - jx) % 2 == 0) & ((hs + 1 - jx) // 2 >= 0) \
                    & ((hs + 1 - jx) // 2 < W0)
                gi = (gs[ok_g] + 1 - jy) // 2
                hi = (hs[ok_h] + 1 - jx) // 2
                contrib = (w.T @ np.ascontiguousarray(
                    xf[n][:, gi][:, :, hi]).reshape(CIN, -1)).reshape(
                        COUT, len(gi), len(hi))
                gg, hh = np.ix_(gs[ok_g] + 2, hs[ok_h] + 2)
                upp_n = upp[n]
                upp_n[:, gg, hh] += contrib
    def cell_out(n, Wc, ci):
        dy, dx = ci // 5, ci % 5
        win = np.ascontiguousarray(upp[n, :, dy:dy + H2, dx:dx + W])
        return (Wc @ win.reshape(COUT, -1)).reshape(CIN, H2, W)
    # full-output scale
    scale = 0.0
    for n in range(N):
        acc = None
        for ci in set(bigs) | set(rings):
            t = cell_out(n, Weff[n, ci // 5, ci % 5].astype(np.float32), ci)
            acc = t if acc is None else acc + t
        scale = max(scale, np.abs(acc).max())
    Weff = Weff.copy()
    kept = list(rings)
    diff = [np.zeros((CIN, H2, W), np.float32) for _ in range(N)]
    order = sorted(rings, key=lambda c: float(
        np.abs(Weff[:, c // 5, c % 5]).max()))
    err = 0.0
    for ci in order:
        dy, dx = ci // 5, ci % 5
        others = [c2 for c2 in (set(bigs) | set(kept)) if c2 != ci]
        others.sort(key=lambda c2: abs(c2 // 5 - dy) + abs(c2 % 5 - dx))
        basis_cells = others[:3]
        # per-batch: least-squares fold of this cell onto the basis cells
        cand = []
        for n in range(N):
            Wc = Weff[n, dy, dx].astype(np.float32)
            if not np.any(Wc):
                cand.append((diff[n], []))
                continue
            r = cell_out(n, Wc, ci).ravel()
            B = np.stack([cell_out(n, Wc, c2).ravel() for c2 in basis_cells])
            G = B @ B.T
            b = B @ r
            try:
                al = np.linalg.solve(G + 1e-12 * np.eye(len(B)), b)
            except np.linalg.LinAlgError:
                al = np.zeros(len(B))
            resid = r - al @ B
            cand.append(((diff[n].ravel() + resid).reshape(CIN, H2, W),
                         list(zip(basis_cells, al))))
        e = max(np.abs(c[0]).max() for c in cand) / scale
        if e <= PRUNE_BUDGET:
            err = e
            for n in range(N):
                diff[n] = cand[n][0]
                Wc = Weff[n, dy, dx].copy()
                for c2, a in cand[n][1]:
                    Weff[n, c2 // 5, c2 % 5] += np.float32(a) * Wc
            Weff[:, dy, dx] = 0.0
            kept.remove(ci)
        else:
            break
    return kept, Weff, err



def _prep_in_maps_v2(xf, trans_w, oy, ox, wa_b):
    """v3: static big cells + static union ring cells, both accumulated into
    ONE psum bank per output block.

    Ring cells (bilinear spill corners) are paired into fp8 DoubleRow
    matmuls over per-dx margin-free fp8 copies of up.  Ring weights are
    scaled x16 and the fp8 copies x1/16, so the pair product is unscaled
    and rings accumulate into the SAME psum bank as the big cells (no
    separate merge pass).  Cells whose max-norm is below 0.4% of the
    global max (the ab bilinear corners, ~1e-4 relative) are dropped
    (~0.1% output error)."""
    Weff = _w_eff(trans_w, oy, ox)                        # [N,5,5,256,128]
    norms = np.abs(Weff).reshape(N_BATCH, 25, -1).max(2)  # [N,25]
    gmax = norms.max()
    bigs = sorted(int(c) for c in np.nonzero(norms.max(0) > 0.25 * gmax)[0])
    keep = (norms.max(0) > 0.004 * gmax) & (norms.max(0) <= 0.25 * gmax)
    rings = [int(c) for c in np.nonzero(keep)[0] if c not in bigs]
    if PRUNE_BUDGET > 0 and rings:
        rings, Weff, _perr = _prune_rings(Weff, xf, trans_w, bigs, rings)
    # order by (dx major, dy minor) and pair far apart so the two DoubleRow
    # K-group windows never overlap
    rings.sort(key=lambda c: (c % 5, c // 5))
    if len(rings) % 2:
        # pad slot: any distinct cell position (zero weights, contributes 0);
        # prefer one that reuses an already-needed dx plane
        dxs = {c % 5 for c in rings}
        pad = next((c for c in range(25) if c not in rings and c % 5 in dxs),
                   next(c for c in range(25) if c not in rings))
        rings.append(pad)
        rings.sort(key=lambda c: (c % 5, c // 5))
    nh = len(rings) // 2
    pairs = [(rings[i], rings[i + nh]) for i in range(nh)]
    need_dx = sorted({c % 5 for c in rings})
    dx_slot = {d: i for i, d in enumerate(need_dx)}
    nbig = len(bigs)

    def cell_off(c, bi):
        return dx_slot[c % 5] * (UR * W) + (4 * bi + c // 5) * W

    # validate pair steps (static, positive, 16-aligned)
    for c1, c2 in pairs:
        step = cell_off(c2, 0) - cell_off(c1, 0)
        assert step > 0 and step % 16 == 0, (c1, c2, step)

    # stage-A weights in phase-major tap order for split DMA
    wa2 = np.zeros((COUT, 9, 2, COUT), np.float32)
    for j, pos in TAP_POS.items():
        jy, jx = j // 3, j % 3
        for h2 in range(2):
            wa2[:, pos, h2, :] = trans_w[h2 * 128:(h2 + 1) * 128, :, jy, jx]
    wa_b = np.ascontiguousarray(wa2).astype(BF16).reshape(COUT, 2 * 9 * COUT)

    wb_all, wr_all = [], []
    for n in range(N_BATCH):
        wb = np.zeros((2, nbig, COUT, COUT), np.float32)  # [half, s, o, c]
        for s, ci in enumerate(bigs):
            wb[:, s] = Weff[n, ci // 5, ci % 5].reshape(2, COUT, COUT)
        wb = wb.transpose(3, 0, 1, 2)                     # [c, half, s, o]
        wb_all.append(np.ascontiguousarray(wb).astype(BF16).reshape(COUT, -1))
        wr = np.zeros((max(len(pairs), 1), 2, 2, COUT, COUT), np.float32)
        for p, (c1, c2) in enumerate(pairs):
            for half in range(2):
                wr[p, half, 0] = RING_W_SCALE * \
                    Weff[n, c1 // 5, c1 % 5][128 * half:128 * (half + 1)].T
                wr[p, half, 1] = RING_W_SCALE * \
                    Weff[n, c2 // 5, c2 % 5][128 * half:128 * (half + 1)].T
        wr = wr.transpose(3, 0, 1, 2, 4)                  # [c, p, half, ksub, o]
        wr_all.append(np.ascontiguousarray(wr).astype(FP8).reshape(COUT, -1))

    in_maps = []
    for core in range(N_CORES):
        n, r = core // STRIPS, core % STRIPS
        xs = np.zeros((COUT, 2, XR, XC), np.float32)
        r0 = 16 * r - 2
        lo, hi = max(0, r0), min(H0, r0 + XR)
        for h2 in range(2):
            xs[:, h2, lo - r0:hi - r0, :W0] = xf[n, h2 * 128:(h2 + 1) * 128, lo:hi, :]
        # banded copy: band b holds xs rows XS_R0[b] .. XS_R0[b]+7, so each
        # stage-A band reads its own tile while the next band's DMA lands
        xsb = np.zeros((COUT, 3, 2, 8, XC), np.float32)
        for b, rb in enumerate(XS_R0):
            nr = min(8, XR - rb)
            xsb[:, b, :, :nr, :] = xs[:, :, rb:rb + nr, :]
        mk = np.full((COUT, 1), 0.0 if r == 0 else 1.0, np.float32)
        in_maps.append({
            "xs": np.ascontiguousarray(xsb.astype(BF16)
                                       .reshape(COUT, 3 * 2 * 8 * XC)),
            "wa": wa_b,
            "wb": wb_all[n],
            "wr": wr_all[n],
            "mk": mk,
        })
    return in_maps, ("v2", tuple(bigs), tuple(pairs), tuple(need_dx))


WG_CENTER = (6, 7, 8, 11, 12, 13, 16, 17, 18)


def _prep_in_maps_wg(xf, trans_w, oy, ox):
    """Winograd variant: valid when all effective cells fold into the center
    3x3 (tiny offsets).  Stage B = F(2,3) along rows: 12 half-size matmuls
    per block instead of 9 full-size.  Returns None if structure doesn't fit
    (caller falls back to v2)."""
    Weff = _w_eff(trans_w, oy, ox)
    norms = np.abs(Weff).reshape(N_BATCH, 25, -1).max(2)
    gmax = norms.max()
    bigs = sorted(int(c) for c in np.nonzero(norms.max(0) > 0.25 * gmax)[0])
    if not set(bigs) <= set(WG_CENTER):
        return None
    keep = (norms.max(0) > 0.004 * gmax) & (norms.max(0) <= 0.25 * gmax)
    rings = [int(c) for c in np.nonzero(keep)[0] if c not in bigs]
    if rings:
        if PRUNE_BUDGET <= 0:
            return None
        rings, Weff, _perr = _prune_rings(Weff, xf, trans_w, bigs, rings)
        if rings:
            return None

    # stage-A weights in phase-major tap order for split DMA (same as v2)
    wa2 = np.zeros((COUT, 9, 2, COUT), np.float32)
    for j, pos in TAP_POS.items():
        jy, jx = j // 3, j % 3
        for h2 in range(2):
            wa2[:, pos, h2, :] = trans_w[h2 * 128:(h2 + 1) * 128, :, jy, jx]
    wa_b = np.ascontiguousarray(wa2).astype(BF16).reshape(COUT, 2 * 9 * COUT)

    # Winograd-transformed stage-B weights: wg[c, half, k(4), kx(3), o]
    # k0 = w_dy0, k1 = (w0+w1+w2)/2, k2 = (w0-w1+w2)/2, k3 = w_dy2
    # (half-major so each half's weights ship as one contiguous DMA piece)
    wg_all = []
    for n in range(N_BATCH):
        wgl = np.zeros((COUT, 2, 4, 3, COUT), np.float32)
        for kx in range(3):
            w0 = Weff[n, 1, 1 + kx]     # [o(256), c(128)], shift dy=-1
            w1 = Weff[n, 2, 1 + kx]
            w2 = Weff[n, 3, 1 + kx]
            for k, wt in enumerate((w0, (w0 + w1 + w2) * 0.5,
                                    (w0 - w1 + w2) * 0.5, w2)):
                for half in range(2):
                    wgl[:, half, k, kx, :] = wt[128 * half:128 * (half + 1), :].T
        wg_all.append(np.ascontiguousarray(wgl).astype(BF16).reshape(COUT, -1))

    in_maps = []
    for core in range(N_CORES):
        n, r = core // STRIPS, core % STRIPS
        xs = np.zeros((COUT, 2, XR, XC), np.float32)
        r0 = 16 * r - 2
        lo, hi = max(0, r0), min(H0, r0 + XR)
        for h2 in range(2):
            xs[:, h2, lo - r0:hi - r0, :W0] = xf[n, h2 * 128:(h2 + 1) * 128, lo:hi, :]
        xsb = np.zeros((COUT, 3, 2, 8, XC), np.float32)
        for b, rb in enumerate(XS_R0):
            nr = min(8, XR - rb)
            xsb[:, b, :, :nr, :] = xs[:, :, rb:rb + nr, :]
        mk = np.full((COUT, 1), 0.0 if r == 0 else 1.0, np.float32)
        in_maps.append({
            "xs": np.ascontiguousarray(xsb.astype(BF16)
                                       .reshape(COUT, 3 * 2 * 8 * XC)),
            "wa": wa_b,
            "wg": wg_all[n],
            "mk": mk,
        })
    return in_maps, ("wg",)


def _prep_in_maps_hybrid(xf, trans_w, oy, ox, wa_b):
    S = HYBRID_S
    nslot = S + 2 * (9 - S)
    wb_all, bs_all, dsc_all, ofs_all = [], [], [], []
    for n in range(N_BATCH):
        wb = np.zeros((nslot, 2, COUT, COUT), np.float32)   # [slot, half, c, o]
        bs = np.zeros((9, 2), np.float32)
        dsc = np.zeros((max(S, 1), 2), np.float32)
        ofs = np.zeros((1, 9, 2), np.int32)
        for k in range(9):
            ky, kx = k // 3, k % 3
            ay = np.float32(ky - 1) + oy[n, k]
            ax = np.float32(kx - 1) + ox[n, k]
            Ay, Ax = int(np.floor(ay)), int(np.floor(ax))
            dy = float(ay) - Ay
            dx = float(ax) - Ax
            ofs[0, k] = (2 + Ay, 2 + Ax)
            bs[k] = (1.0 - dy, dy)
            wkT = np.stack([trans_w[h * 128:(h + 1) * 128, :, ky, kx].T
                            for h in range(2)])             # [half, c, o]
            if k < S:
                dsc[k] = (1.0 - dx, dx)
                wb[k] = wkT
            else:
                wb[S + 2 * (k - S) + 0] = (1.0 - dx) * wkT
                wb[S + 2 * (k - S) + 1] = dx * wkT
        wb = wb.transpose(2, 0, 1, 3)                       # [c, slot, half, o]
        wb_all.append(np.ascontiguousarray(wb).astype(BF16)
                      .reshape(COUT, nslot * 2 * COUT))
        bs_all.append(np.broadcast_to(bs.reshape(1, 9, 2),
                                      (COUT, 9, 2)).copy())
        dsc_all.append(np.broadcast_to(dsc.reshape(1, -1, 2),
                                       (COUT, max(S, 1), 2)).copy())
        ofs_all.append(ofs)

    in_maps = []
    for core in range(N_CORES):
        n, r = core // STRIPS, core % STRIPS
        xs = np.zeros((COUT, 2, XR, XC), np.float32)
        r0 = 16 * r - 2
        lo, hi = max(0, r0), min(H0, r0 + XR)
        for h2 in range(2):
            xs[:, h2, lo - r0:hi - r0, :W0] = xf[n, h2 * 128:(h2 + 1) * 128, lo:hi, :]
        mk = np.full((COUT, 1), 0.0 if r == 0 else 1.0, np.float32)
        in_maps.append({
            "xs": np.ascontiguousarray(xs.astype(BF16).reshape(COUT, 2 * XR * XC)),
            "wa": wa_b,
            "wb": wb_all[n],
            "mk": mk,
            "bs": bs_all[n].reshape(COUT, 18),
            "dsc": dsc_all[n].reshape(COUT, -1),
            "co": ofs_all[n],
        })
    return in_maps, nslot


# --------------------------------------------------------------------------
# device program (input-independent; same for all cores except r-dependent
# row validity -> handled by *uniform* structure: we compute all 36 up rows,
# rows outside [0,128) stay zero because their x inputs are zeroed host-side
# ... except parity bookkeeping differs per strip; we keep the program truly
# SPMD by computing the full 18 a'-rows per phase and masking via zero x.)
# --------------------------------------------------------------------------

def _build_nc_v2(key):
    """v3 device program: interleaved stage A bands / stage B block groups,
    static big + ring cells unified into one psum bank, fp16 output."""
    _, bigs, pairs, need_dx = key
    bigs, pairs, need_dx = list(bigs), list(pairs), list(need_dx)
    nbig, npair, ndx = len(bigs), len(pairs), len(need_dx)
    dx_slot = {d: i for i, d in enumerate(need_dx)}
    nc = bacc.Bacc("TRN2", target_bir_lowering=False, debug=False,
                   enable_asserts=False)

    xs_d = nc.dram_tensor("xs", [COUT, 3 * 2 * 8 * XC], mybir.dt.bfloat16,
                          kind="ExternalInput").ap()
    wa_d = nc.dram_tensor("wa", [COUT, 2 * 9 * COUT], mybir.dt.bfloat16,
                          kind="ExternalInput").ap()
    wb_d = nc.dram_tensor("wb", [COUT, 2 * nbig * COUT], mybir.dt.bfloat16,
                          kind="ExternalInput").ap()
    wr_d = nc.dram_tensor("wr", [COUT, max(npair, 1) * 2 * 2 * COUT],
                          mybir.dt.float8e4, kind="ExternalInput").ap()
    mk_d = nc.dram_tensor("mk", [COUT, 1], mybir.dt.float32,
                          kind="ExternalInput").ap()
    out_dt = {"f16": mybir.dt.float16, "bf16": mybir.dt.bfloat16,
              "f32": mybir.dt.float32}[V2_OUT]
    out_d = nc.dram_tensor("out", [CIN, OUT_R, W], out_dt,
                           kind="ExternalOutput").ap()

    with tile.TileContext(nc) as tc:
        with (
            tc.tile_pool(name="singles", bufs=1) as singles,
            tc.tile_pool(name="outp", bufs=4) as outp,
            tc.tile_pool(name="psA", bufs=4, space="PSUM") as psA,
            tc.tile_pool(name="psB", bufs=4, space="PSUM") as psB,
        ):
            xs_t = singles.tile([COUT, 3, 2, 8, XC], mybir.dt.bfloat16)
            wa_t = singles.tile([COUT, 9, 2, COUT], mybir.dt.bfloat16)
            wb_t = singles.tile([COUT, 2, nbig, COUT], mybir.dt.bfloat16)
            wr_t = singles.tile([COUT, max(npair, 1), 2, 2, COUT],
                                mybir.dt.float8e4)
            mk_t = singles.tile([COUT, 1], mybir.dt.float32)
            up_full = singles.tile([COUT, UR * UC + 12], mybir.dt.bfloat16)
            up_t = up_full[:, :UR * UC]
            upf_t = singles.tile([COUT, max(ndx, 1), UR, W], mybir.dt.float8e4)

            # ---- input DMA, ordered so the first stage-A matmuls (half 0,
            # band 0) can start as early as possible ----
            xs4 = xs_t[:]
            xs4_d = xs_d.rearrange("p (a b c d) -> p a b c d", a=3, b=2, c=8)
            wa_flat = wa_t[:].rearrange("p a b c -> p (a b c)")
            wb_flat = wb_t[:].rearrange("p a b c -> p (a b c)")
            # the three pieces gating the first matmuls go first, split
            # across both HWDGE queues (one trigger each ~0.7us):
            #   sync:   wa piece1 (tap j4, both halves), xs h1 band0, wa rest
            #   scalar: xs h0 band0, xs h0 rest, wb h0, xs h1 rest, wb h1
            nc.sync.dma_start(out=wa_flat[:, :WA_CUTS[1] * 2 * COUT],
                              in_=wa_d[:, :WA_CUTS[1] * 2 * COUT])
            nc.scalar.dma_start(out=xs4[:, 0], in_=xs4_d[:, 0])
            for c0, c1 in zip(WA_CUTS[1:-1], WA_CUTS[2:]):
                nc.sync.dma_start(out=wa_flat[:, c0 * 2 * COUT:c1 * 2 * COUT],
                                  in_=wa_d[:, c0 * 2 * COUT:c1 * 2 * COUT])
            nc.scalar.dma_start(out=xs4[:, 1], in_=xs4_d[:, 1])
            nc.scalar.dma_start(out=xs4[:, 2], in_=xs4_d[:, 2])
            nc.scalar.dma_start(out=wb_flat[:, :nbig * COUT],
                                in_=wb_d[:, :nbig * COUT])
            nc.scalar.dma_start(out=wb_flat[:, nbig * COUT:],
                                in_=wb_d[:, nbig * COUT:])
            if npair:
                nc.sync.dma_start(
                    out=wr_t[:].rearrange("p a b c d -> p (a b c d)"), in_=wr_d)
            nc.sync.dma_start(out=mk_t[:], in_=mk_d)

            up_w = up_t.rearrange("p (a q c r) -> p a q c r", q=2, c=66, r=2)
            up_r = up_t.rearrange("p (l c) -> p l c", c=132)
            # only the column margins and the tail pad are never written by
            # the stage-A scatter -- memset just those
            nc.vector.memset(up_r[:, :, 0:2], 0.0)
            nc.vector.memset(up_r[:, :, 130:132], 0.0)
            nc.vector.memset(up_full[:, UR * UC:], 0.0)
            upf_fl = upf_t[:].rearrange("p a b c -> p (a b c)")

            ytaps = {0: ((1, 0),), 1: ((2, 0), (0, 1))}
            band_blocks = ((0, 1), (2, 3, 4), (5, 6, 7))

            def cell_off(c, bi):
                return dx_slot[c % 5] * (UR * W) + (4 * bi + c // 5) * W

            def emit_phase(b, py, px):
                a0 = 6 * b
                rc = 6
                taps = [(jy, dy, jx, dx)
                        for jy, dy in ytaps[py] for jx, dx in ytaps[px]]
                ps = psA.tile([COUT, rc, 64], mybir.dt.float32, tag="psA",
                              name=f"psA_{b}_{py}_{px}")
                nmm = len(taps) * 2
                i = 0
                for h2 in range(2):
                    for (jy, dy, jx, dx) in taps:
                        r0x = a0 + 1 + dy - XS_R0[b]
                        nc.tensor.matmul(
                            ps[:, :rc, :],
                            lhsT=wa_t[:, TAP_POS[jy * 3 + jx], h2, :],
                            rhs=xs_t[:, b, h2, r0x:r0x + rc, dx:dx + 64],
                            start=(i == 0), stop=(i == nmm - 1),
                        )
                        i += 1
                nc.scalar.copy(
                    out=up_w[:, a0:a0 + rc, py, 1:65, px],
                    in_=ps[:, :rc, :],
                )

            def emit_block(bi):
                for half in range(2):
                    ps = psB.tile([COUT, RBLK, W], mybir.dt.float32,
                                  tag="psB", name=f"psB_{bi}_{half}")
                    mixed = V2_MIXED_GROUP and npair > 0
                    nmm = nbig + (npair if mixed else 0)
                    for s, ci in enumerate(bigs):
                        dyi, dxi = ci // 5, ci % 5
                        ys = 4 * bi + dyi
                        nc.tensor.matmul(
                            ps[:], lhsT=wb_t[:, half, s, :],
                            rhs=up_r[:, ys:ys + RBLK, dxi:dxi + W],
                            start=(s == 0), stop=(s == nmm - 1))
                    psr = ps if mixed else (
                        psA.tile([COUT, RBLK, W], mybir.dt.float32,
                                 tag="psA", name=f"psr_{bi}_{half}")
                        if npair else None)
                    for p, (c1, c2) in enumerate(pairs):
                        step = cell_off(c2, 0) - cell_off(c1, 0)
                        off = cell_off(c1, bi)
                        win = upf_fl[:, off:off + RBLK * W]
                        rhs = bass.AP(tensor=win.tensor, offset=win.offset,
                                      ap=[win.ap[0], [step, 2], win.ap[1]])
                        nc.tensor.matmul(
                            psr[:], lhsT=wr_t[:, p, half, :, :], rhs=rhs,
                            perf_mode=mybir.MatmulPerfMode.DoubleRow,
                            start=(False if mixed else p == 0),
                            stop=(nbig + p == nmm - 1) if mixed
                            else (p == npair - 1))
                    ob = outp.tile([COUT, RBLK, W], out_dt, tag="ob",
                                   name=f"ob_{bi}_{half}")
                    if V2_EVAC_ENG == "vector":
                        nc.vector.tensor_copy(ob[:], ps[:])
                    else:
                        nc.scalar.copy(out=ob[:], in_=ps[:])
                    if not mixed and npair:
                        nc.vector.scalar_tensor_tensor(
                            out=ob[:], in0=psr[:], scalar=1.0,
                            in1=ob[:], op0=mybir.AluOpType.mult,
                            op1=mybir.AluOpType.add)
                    nc.sync.dma_start(
                        out=out_d[128 * half:128 * (half + 1),
                                  RBLK * bi:RBLK * (bi + 1), :],
                        in_=ob[:])

            def emit_band_rest(b):
                if b == 0:
                    # zero the two bottom halo rows on the r=0 strip
                    nc.vector.tensor_scalar_mul(up_r[:, 0:2, :], up_r[:, 0:2, :],
                                                mk_t[:, 0:1])
                # fp8 ring planes for this band (x 1/RING_W_SCALE)
                for i, dxp in enumerate(need_dx):
                    src = up_r[:, 12 * b:12 * b + 12, dxp:dxp + W]
                    dst = upf_t[:, i, 12 * b:12 * b + 12, :]
                    if V2_VEC_PLANES and i % 2 == 1:
                        nc.vector.tensor_scalar_mul(dst, src,
                                                    1.0 / RING_W_SCALE)
                    else:
                        nc.scalar.mul(out=dst, in_=src, mul=1.0 / RING_W_SCALE)
                for bi in band_blocks[b]:
                    emit_block(bi)

            for b in range(3):
                for (py, px) in ((0, 0), (0, 1), (1, 0), (1, 1)):
                    emit_phase(b, py, px)
                emit_band_rest(b)

    nc.compile()
    return nc


def _build_nc_wg():
    """Winograd F(2,3)-rows device program.

    Stage A (transposed conv) unchanged.  Then per band, V planes
      V0[t] = u[2t+1]-u[2t+3], V1[t] = u[2t+2]+u[2t+3],
      V2[t] = u[2t+3]-u[2t+2], V3[t] = u[2t+2]-u[2t+4]   (u rows of `up`)
    are built on vector/gpsimd.  Stage B per (half, 4-row block):
      m_k = sum_kx wg[k,kx] @ V_k[2bi:2bi+2, kx+1 : kx+1+W]   (4 psum comps)
      out even rows = m0+m1+m2, odd rows = m1-m2-m3
    Combines: scalar evacuates m1,m3; DVE does the three psum-reading adds;
    gpsimd does the sbuf-only one.  PE sequence A0,A1,B0,A2,B1,B2 so V(b)
    always builds in the shadow of PE work on other data."""
    nc = bacc.Bacc("TRN2", target_bir_lowering=False, debug=False,
                   enable_asserts=False)

    xs_d = nc.dram_tensor("xs", [COUT, 3 * 2 * 8 * XC], mybir.dt.bfloat16,
                          kind="ExternalInput").ap()
    wa_d = nc.dram_tensor("wa", [COUT, 2 * 9 * COUT], mybir.dt.bfloat16,
                          kind="ExternalInput").ap()
    wg_d = nc.dram_tensor("wg", [COUT, 4 * 3 * 2 * COUT], mybir.dt.bfloat16,
                          kind="ExternalInput").ap()
    mk_d = nc.dram_tensor("mk", [COUT, 1], mybir.dt.float32,
                          kind="ExternalInput").ap()
    out_d = nc.dram_tensor("out", [CIN, OUT_R, W], mybir.dt.float16,
                           kind="ExternalOutput").ap()

    NT = OUT_R // 2            # 16 winograd tile rows
    mm = mybir.AluOpType.mult
    aa = mybir.AluOpType.add

    with tile.TileContext(nc) as tc:
        with (
            tc.tile_pool(name="singles", bufs=1) as singles,
            tc.tile_pool(name="outp", bufs=3) as outp,
            tc.tile_pool(name="evp", bufs=3) as evp,
            tc.tile_pool(name="psA", bufs=3, space="PSUM") as psA,
            tc.tile_pool(name="psB", bufs=1, space="PSUM") as psB,
        ):
            xs_t = singles.tile([COUT, 3, 2, 8, XC], mybir.dt.bfloat16)
            wa_t = singles.tile([COUT, 9, 2, COUT], mybir.dt.bfloat16)
            wg_t = singles.tile([COUT, 2, 4, 3, COUT], mybir.dt.bfloat16)
            mk_t = singles.tile([COUT, 1], mybir.dt.float32)
            up_t = singles.tile([COUT, UR * UC], mybir.dt.bfloat16)
            v_t = singles.tile([COUT, 4, NT, UC], mybir.dt.bfloat16)

            # ---- optional PE p-state warmup on zeroed dummy data ----
            if WG_WARMUP:
                wu_t = singles.tile([COUT, 384], mybir.dt.bfloat16)
                nc.vector.memset(wu_t[:], 0.0)
                for i in range(WG_WARMUP):
                    psw = psA.tile([COUT, 384], mybir.dt.float32, tag="psA",
                                   name=f"psw_{i}")
                    nc.tensor.matmul(psw[:], lhsT=wu_t[:, :COUT],
                                     rhs=wu_t[:], start=True, stop=True)

            # ---- input DMA, critical pieces first on both queues ----
            xs4 = xs_t[:]
            xs4_d = xs_d.rearrange("p (a b c d) -> p a b c d", a=3, b=2, c=8)
            wa_flat = wa_t[:].rearrange("p a b c -> p (a b c)")
            wg_flat = wg_t[:].rearrange("p a b c d -> p (a b c d)")
            def wa_piece(q, i):
                c0, c1 = WA_CUTS[i] * 2 * COUT, WA_CUTS[i + 1] * 2 * COUT
                q.dma_start(out=wa_flat[:, c0:c1], in_=wa_d[:, c0:c1])
            wa_piece(nc.sync, 0)                      # j4 (p00)
            nc.scalar.dma_start(out=xs4[:, 0], in_=xs4_d[:, 0])
            wa_piece(nc.sync, 1)                      # j3,j5 (p01)
            wa_piece(nc.scalar, 2)                    # j1,j7 (p10)
            wa_piece(nc.sync, 3)                      # p11 taps
            half_wg = 4 * 3 * COUT
            nc.sync.dma_start(out=wg_flat[:, :half_wg],
                              in_=wg_d[:, :half_wg])
            nc.scalar.dma_start(out=xs4[:, 1], in_=xs4_d[:, 1])
            nc.sync.dma_start(out=mk_t[:], in_=mk_d)
            nc.sync.dma_start(out=wg_flat[:, half_wg:],
                              in_=wg_d[:, half_wg:])
            nc.scalar.dma_start(out=xs4[:, 2], in_=xs4_d[:, 2])

            up_w = up_t.rearrange("p (a q c r) -> p a q c r", q=2, c=66, r=2)
            up_r = up_t.rearrange("p (l c) -> p l c", c=UC)
            up_pair = up_t.rearrange("p (l2 two c) -> p l2 two c",
                                     two=2, c=UC)
            nc.vector.memset(up_r[:, :, 0:2], 0.0)
            nc.vector.memset(up_r[:, :, 130:132], 0.0)

            ytaps = {0: ((1, 0),), 1: ((2, 0), (0, 1))}

            def emit_phase(b, py, px):
                a0 = 6 * b
                rc = 6
                taps = [(jy, dy, jx, dx)
                        for jy, dy in ytaps[py] for jx, dx in ytaps[px]]
                ps = psA.tile([COUT, rc, 64], mybir.dt.float32, tag="psA",
                              name=f"psA_{b}_{py}_{px}")
                nmm = len(taps) * 2
                i = 0
                for h2 in range(2):
                    for (jy, dy, jx, dx) in taps:
                        r0x = a0 + 1 + dy - XS_R0[b]
                        nc.tensor.matmul(
                            ps[:, :rc, :],
                            lhsT=wa_t[:, TAP_POS[jy * 3 + jx], h2, :],
                            rhs=xs_t[:, b, h2, r0x:r0x + rc, dx:dx + 64],
                            start=(i == 0), stop=(i == nmm - 1),
                        )
                        i += 1
                nc.scalar.copy(
                    out=up_w[:, a0:a0 + rc, py, 1:65, px],
                    in_=ps[:, :rc, :],
                )

            def emit_band_A(b):
                for (py, px) in ((0, 0), (0, 1), (1, 0), (1, 1)):
                    emit_phase(b, py, px)
                if b == 0:
                    nc.vector.tensor_scalar_mul(up_r[:, 0:2, :],
                                                up_r[:, 0:2, :], mk_t[:, 0:1])

            V_T0 = (0, 4, 10, 16)      # t-ranges per band

            def emit_V(b):
                t0, t1 = V_T0[b], V_T0[b + 1]
                n_ = t1 - t0
                # V0[t] = u[2t+1] - u[2t+3]
                nc.vector.scalar_tensor_tensor(
                    out=v_t[:, 0, t0:t1, :],
                    in0=up_pair[:, t0 + 1:t1 + 1, 1, :], scalar=-1.0,
                    in1=up_pair[:, t0:t1, 1, :], op0=mm, op1=aa)
                # V1[t] = u[2t+2] + u[2t+3]
                nc.vector.scalar_tensor_tensor(
                    out=v_t[:, 1, t0:t1, :],
                    in0=up_pair[:, t0 + 1:t1 + 1, 0, :], scalar=1.0,
                    in1=up_pair[:, t0 + 1:t1 + 1, 1, :], op0=mm, op1=aa)
                # V2[t] = u[2t+3] - u[2t+2]
                nc.gpsimd.tensor_tensor(
                    v_t[:, 2, t0:t1, :],
                    up_pair[:, t0 + 1:t1 + 1, 1, :],
                    up_pair[:, t0 + 1:t1 + 1, 0, :], mybir.AluOpType.subtract)
                # V3[t] = u[2t+2] - u[2t+4]
                nc.gpsimd.tensor_tensor(
                    v_t[:, 3, t0:t1, :],
                    up_pair[:, t0 + 1:t1 + 1, 0, :],
                    up_pair[:, t0 + 2:t1 + 2, 0, :], mybir.AluOpType.subtract)

            # output DMA groups (blocks per DMA, grouped within bands)
            OUT_GROUPS = ((0, 1), (2, 3), (4,), (5, 6), (7,))
            grp_of = {bi: g for g in OUT_GROUPS for bi in g}
            ob_tiles = {}

            def emit_block(bi, half):
                # matmul group order k1,k3,k0,k2 so the m1/m3 evacs and the
                # gpsimd o1 combine overlap the k0/k2 matmuls; after the last
                # group only the two DVE writes into ob remain.  One psum
                # tile per component so consumer reads never serialize
                # against later component matmuls.
                ps = [psB.tile([COUT, 2, W], mybir.dt.float32, tag=f"psB{k}",
                               name=f"psB{k}_{bi}_{half}",
                               bufs=(2 if k == 2 else 1)) for k in range(4)]
                g = grp_of[bi]
                if (g, half) not in ob_tiles:
                    ob_tiles[(g, half)] = outp.tile(
                        [COUT, len(g) * 4, W], mybir.dt.float16, tag="ob",
                        name=f"ob_{g[0]}_{half}", padded_shape=[COUT, 8, W])
                ob = ob_tiles[(g, half)]
                toff = 2 * (bi - g[0])
                obr = ob.rearrange("p (t s) c -> p t s c", s=2)
                m1s = evp.tile([COUT, 2, W], mybir.dt.float32, tag="m1s",
                               name=f"m1s_{bi}_{half}")
                m3s = evp.tile([COUT, 2, W], mybir.dt.float32, tag="m3s",
                               name=f"m3s_{bi}_{half}")
                e1 = evp.tile([COUT, 2, W], mybir.dt.float32, tag="e1",
                              name=f"e1_{bi}_{half}")
                o1 = evp.tile([COUT, 2, W], mybir.dt.float32, tag="o1",
                              name=f"o1_{bi}_{half}")

                def mmk(k):
                    for kx in range(3):
                        nc.tensor.matmul(
                            ps[k][:], lhsT=wg_t[:, half, k, kx, :],
                            rhs=v_t[:, k, 2 * bi:2 * bi + 2, kx + 1:kx + 1 + W],
                            start=(kx == 0), stop=(kx == 2))

                mmk(1)
                nc.scalar.copy(out=m1s[:], in_=ps[1][:])
                mmk(3)
                nc.scalar.copy(out=m3s[:], in_=ps[3][:])
                # o1 = m1 - m3 (sbuf-only, runs during k0/k2 matmuls)
                nc.gpsimd.tensor_tensor(o1[:], m1s[:], m3s[:],
                                        mybir.AluOpType.subtract)
                mmk(0)
                # e1 = m0 + m1 (runs during k2 matmuls)
                nc.vector.scalar_tensor_tensor(
                    out=e1[:], in0=ps[0][:], scalar=1.0, in1=m1s[:],
                    op0=mm, op1=aa)
                mmk(2)
                nc.vector.scalar_tensor_tensor(
                    out=obr[:, toff:toff + 2, 0, :], in0=ps[2][:], scalar=1.0,
                    in1=e1[:], op0=mm, op1=aa)
                nc.vector.scalar_tensor_tensor(
                    out=obr[:, toff:toff + 2, 1, :], in0=ps[2][:], scalar=-1.0,
                    in1=o1[:], op0=mm, op1=aa)
                if bi == g[-1]:
                    nc.sync.dma_start(
                        out=out_d[128 * half:128 * (half + 1),
                                  4 * g[0]:4 * g[0] + 4 * len(g), :],
                        in_=ob[:, :len(g) * 4, :])

            # ---- schedule: A0, A1, [V0] B0 B1, A2, [V1] B2 B3 B4, [V2] ... ----
            emit_band_A(0)
            emit_band_A(1)
            emit_V(0)
            for bi in (0, 1):
                for half in range(2):
                    emit_block(bi, half)
            emit_V(1)
            emit_band_A(2)
            for bi in (2, 3, 4):
                for half in range(2):
                    emit_block(bi, half)
            emit_V(2)
            for bi in (5, 6, 7):
                for half in range(2):
                    emit_block(bi, half)

    nc.compile()
    return nc


def _build_nc(ncell):
    if isinstance(ncell, tuple) and ncell[0] == "wg":
        return _build_nc_wg()
    if isinstance(ncell, tuple) and ncell[0] == "v2":
        return _build_nc_v2(ncell)
    fp8r = isinstance(ncell, tuple) and ncell[0] == "fp8r"
    if fp8r:
        bigs, pairs = list(ncell[1]), list(ncell[2])
        ncell = max(len(bigs), 1)
        cells, dyn = None, False
    elif isinstance(ncell, tuple):      # ("union", cell, cell, ...)
        cells = list(ncell[1:])
        ncell = len(cells)
        dyn = False
    else:
        cells = list(range(NCELL)) if VARIANT == "full25" else None
        dyn = VARIANT not in ("full25",)
    nc = bacc.Bacc("TRN2", target_bir_lowering=False, debug=False,
                   enable_asserts=False)

    xs_d = nc.dram_tensor("xs", [COUT, 3 * 2 * 8 * XC], mybir.dt.bfloat16,
                          kind="ExternalInput").ap()
    wa_d = nc.dram_tensor("wa", [COUT, 2 * 9 * COUT], mybir.dt.bfloat16,
                          kind="ExternalInput").ap()
    wb_d = nc.dram_tensor("wb", [COUT, ncell * 2 * COUT], mybir.dt.bfloat16,
                          kind="ExternalInput").ap()
    mk_d = nc.dram_tensor("mk", [COUT, 1], mybir.dt.float32,
                          kind="ExternalInput").ap()
    if fp8r:
        wr_d = nc.dram_tensor(
            "wr", [COUT, max(len(pairs), 1) * 2 * 2 * COUT],
            mybir.dt.float8e4, kind="ExternalInput").ap()
    hyb = VARIANT == "hybrid"
    S = HYBRID_S
    if hyb:
        co_d = nc.dram_tensor("co", [1, 9, 2], mybir.dt.int32,
                              kind="ExternalInput").ap()
        bs_d = nc.dram_tensor("bs", [COUT, 18], mybir.dt.float32,
                              kind="ExternalInput").ap()
        dsc_d = nc.dram_tensor("dsc", [COUT, 2 * max(S, 1)], mybir.dt.float32,
                               kind="ExternalInput").ap()
    elif dyn:
        co_d = nc.dram_tensor("co", [1, ncell, 2], mybir.dt.int32,
                              kind="ExternalInput").ap()
    out_d = nc.dram_tensor("out", [CIN, OUT_R, W], mybir.dt.float32,
                           kind="ExternalOutput").ap()

    with tile.TileContext(nc) as tc:
        with (
            tc.tile_pool(name="singles", bufs=1) as singles,
            tc.tile_pool(name="outp", bufs=4) as outp,
            tc.tile_pool(name="psB", bufs=4, space="PSUM") as psB,
            tc.tile_pool(name="psR", bufs=4, space="PSUM") as psR,
        ):
            xs_t = singles.tile([COUT, 3, 2, 8, XC], mybir.dt.bfloat16)
            wa_t = singles.tile([COUT, 9, 2, COUT], mybir.dt.bfloat16)
            wb_t = singles.tile([COUT, ncell, 2, COUT], mybir.dt.bfloat16)
            mk_t = singles.tile([COUT, 1], mybir.dt.float32)
            # +12 pad: hybrid vy reads may run a few elements past the last
            # row (col-window spill); padded region is zeroed, never consumed
            up_full = singles.tile([COUT, UR * UC + 12], mybir.dt.bfloat16)
            up_t = up_full[:, :UR * UC]

            # stage-A critical inputs split across both HWDGE queues; xs is
            # further split by row band so the first stage-A band can start
            # after ~0.3MB instead of the whole tensor.  Band a0 reads xs rows
            # a0+1+dy (dy<=1), so rows [0,9) cover band 0, [9,20) the rest.
            xs4 = xs_t[:]
            xs4_d = xs_d.rearrange("p (a b c d) -> p a b c d", a=3, b=2, c=8)
            for h2 in range(2):
                eng = nc.sync if h2 == 0 else nc.scalar
                eng.dma_start(out=xs4[:, h2, 0:9, :], in_=xs4_d[:, h2, 0:9, :])
            nc.sync.dma_start(out=wa_t[:].rearrange("p a b c -> p (a b c)"), in_=wa_d)
            for h2 in range(2):
                eng = nc.scalar if h2 == 0 else nc.sync
                eng.dma_start(out=xs4[:, h2, 9:, :], in_=xs4_d[:, h2, 9:, :])
            nc.sync.dma_start(out=mk_t[:], in_=mk_d)
            wb_flat = wb_t[:].rearrange("p a b c -> p (a b c)")
            nc.scalar.dma_start(out=wb_flat, in_=wb_d)
            if fp8r:
                wr_t = singles.tile([COUT, max(len(pairs), 1), 2, 2, COUT],
                                    mybir.dt.float8e4)
                nc.sync.dma_start(
                    out=wr_t[:].rearrange("p a b c d -> p (a b c d)"), in_=wr_d)
                upf_t = singles.tile([COUT, 5, UR, W], mybir.dt.float8e4)
            if hyb:
                co_t = singles.tile([1, 9, 2], mybir.dt.int32)
                bs_t = singles.tile([COUT, 9, 2], mybir.dt.float32)
                dsc_t = singles.tile([COUT, max(S, 1), 2], mybir.dt.float32)
                nc.sync.dma_start(out=co_t[:].rearrange("p a b -> p (a b)"),
                                  in_=co_d.rearrange("p a b -> p (a b)"))
                nc.sync.dma_start(out=bs_t[:].rearrange("p a b -> p (a b)"),
                                  in_=bs_d)
                nc.sync.dma_start(out=dsc_t[:].rearrange("p a b -> p (a b)"),
                                  in_=dsc_d)
            elif dyn:
                co_t = singles.tile([1, ncell, 2], mybir.dt.int32)
                nc.sync.dma_start(out=co_t[:].rearrange("p a b -> p (a b)"),
                                  in_=co_d.rearrange("p a b -> p (a b)"))

            # zero the up tile (margins + potentially-invalid rows)
            nc.vector.memset(up_full[:], 0.0)

            # views of up: [p, a'(18), q(2), cc(66), r(2)] for phase writes,
            # [p, l(36), c(132)] for stage-B reads
            up_w = up_t.rearrange("p (a q c r) -> p a q c r", q=2, c=66, r=2)
            up_r = up_t.rearrange("p (l c) -> p l c", c=132)

            # ---- stage A: transposed conv -> up ----
            # row-major (a0 outer) so each 12-row band of up completes early;
            # for fp8r the band's fp8 casts are emitted right behind it, so
            # the ring matmuls never wait on a late cast burst
            ytaps = {0: ((1, 0),), 1: ((2, 0), (0, 1))}
            if fp8r:
                need_dx = sorted({c % 5 for pr in pairs for c in pr})
            for a0 in range(0, 18, 6):
                rc = 6
                for py in (0, 1):
                    for px in (0, 1):
                        taps = [(jy, dy, jx, dx)
                                for jy, dy in ytaps[py] for jx, dx in ytaps[px]]
                        # stage A borrows the ring pool (idle here) so its
                        # evacuations never block stage-B big-cell psum slots
                        pool = psR if fp8r else psB
                        ps = pool.tile([COUT, 6, 64], mybir.dt.float32,
                                       tag="psR" if fp8r else "psB")
                        nmm = len(taps) * 2
                        i = 0
                        for (jy, dy, jx, dx) in taps:
                            for h2 in range(2):
                                r0x = a0 + 1 + dy - XS_R0[b]
                                nc.tensor.matmul(
                                    ps[:, :rc, :],
                                    lhsT=wa_t[:, TAP_POS[jy * 3 + jx], h2, :],
                                    rhs=xs_t[:, b, h2, r0x:r0x + rc,
                                             dx:dx + 64],
                                    start=(i == 0), stop=(i == nmm - 1),
                                )
                                i += 1
                        # scatter phase result into up (cast to bf16)
                        nc.scalar.copy(
                            out=up_w[:, a0:a0 + rc, py, 1:65, px],
                            in_=ps[:, :rc, :],
                        )
                if a0 == 0:
                    # zero the bottom two halo rows on the r=0 strip (g=-2,-1):
                    # the phase formula extended below the image is invalid there
                    nc.vector.tensor_scalar_mul(up_r[:, 0:2, :], up_r[:, 0:2, :],
                                                mk_t[:, 0:1])
                if fp8r:
                    for dx in need_dx:
                        nc.scalar.copy(
                            out=upf_t[:, dx, 2 * a0:2 * a0 + 12, :],
                            in_=up_r[:, 2 * a0:2 * a0 + 12, dx:dx + W])

            # ---- stage B: effective-cell conv -> out ----
            if fp8r:
                _stage_b_fp8r(nc, tc, up_r, upf_t, wb_t, wr_t, bigs, pairs,
                              psB, psR, outp, out_d)
            elif hyb:
                with (
                    tc.tile_pool(name="vyp", bufs=2) as vyp,
                    tc.tile_pool(name="smp", bufs=2) as smp,
                ):
                    # per-tap (row, col) bases into vector-engine registers
                    rvs = [nc.vector.value_load(co_t[0:1, k, 0:1],
                                                min_val=0, max_val=3)
                           for k in range(9)]
                    cvs = [nc.vector.value_load(co_t[0:1, k, 1:2],
                                                min_val=0, max_val=3)
                           for k in range(9)]
                    mm = mybir.AluOpType.mult
                    aa = mybir.AluOpType.add
                    up_fl = up_full[:]
                    for sb in range(OUT_R // SBR):
                        vys, samps = [], []
                        for k in range(9):
                            vy = vyp.tile([COUT, SBR, UC], mybir.dt.bfloat16,
                                          tag=f"vy{k}")
                            # [SBR rows x UC cols] shifted window == contiguous
                            # flat block of SBR*UC elements
                            base = rvs[k] * UC + cvs[k] + (SBR * sb) * UC
                            i0 = up_fl[:, bass.ds(base, SBR * UC)].rearrange(
                                "p (a b) -> p a b", b=UC)
                            i1 = up_fl[:, bass.ds(base + UC, SBR * UC)].rearrange(
                                "p (a b) -> p a b", b=UC)
                            nc.vector.tensor_scalar_mul(vy[:], i0, bs_t[:, k, 0:1])
                            nc.vector.scalar_tensor_tensor(
                                out=vy[:], in0=i1, scalar=bs_t[:, k, 1:2],
                                in1=vy[:], op0=mm, op1=aa)
                            vys.append(vy)
                        for k in range(S):
                            sa = smp.tile([COUT, SBR, W], mybir.dt.bfloat16,
                                          tag=f"sa{k}")
                            nc.vector.tensor_scalar_mul(
                                sa[:], vys[k][:, :, 0:W], dsc_t[:, k, 0:1])
                            nc.vector.scalar_tensor_tensor(
                                out=sa[:], in0=vys[k][:, :, 1:W + 1],
                                scalar=dsc_t[:, k, 1:2], in1=sa[:],
                                op0=mm, op1=aa)
                            samps.append(sa)
                        for sub in range(SBR // RBLK):
                            rs = slice(RBLK * sub, RBLK * (sub + 1))
                            bi = (SBR * sb) // RBLK + sub
                            for half in range(2):
                                ps = psB.tile([COUT, RBLK, W], mybir.dt.float32,
                                              tag="psB")
                                nmm = S + 2 * (9 - S)
                                si = 0
                                for k in range(9):
                                    if k < S:
                                        rhss = [samps[k][:, rs, :]]
                                    else:
                                        rhss = [vys[k][:, rs, 0:W],
                                                vys[k][:, rs, 1:W + 1]]
                                    for rhs in rhss:
                                        nc.tensor.matmul(
                                            ps[:], lhsT=wb_t[:, si, half, :],
                                            rhs=rhs, start=(si == 0),
                                            stop=(si == nmm - 1))
                                        si += 1
                                ob = outp.tile([COUT, RBLK, W], mybir.dt.float32,
                                               tag="ob")
                                nc.scalar.copy(out=ob[:], in_=ps[:])
                                nc.sync.dma_start(
                                    out=out_d[128 * half:128 * (half + 1),
                                              RBLK * bi:RBLK * (bi + 1), :],
                                    in_=ob[:])
            else:
                if dyn:
                    # per-slot (row, col) bases into tensor-engine registers
                    rvs = [nc.tensor.value_load(co_t[0:1, ci, 0:1],
                                                min_val=0, max_val=4)
                           for ci in range(ncell)]
                    cvs = [nc.tensor.value_load(co_t[0:1, ci, 1:2],
                                                min_val=0, max_val=4)
                           for ci in range(ncell)]
                for bi in range(OUT_R // RBLK):
                    for half in range(2):
                        ps = psB.tile([COUT, RBLK, W], mybir.dt.float32, tag="psB")
                        for ci in range(ncell):
                            if dyn:
                                rhs = up_r[:, bass.ds(rvs[ci] + 4 * bi, RBLK),
                                           bass.ds(cvs[ci], W)]
                            else:
                                dyi, dxi = cells[ci] // 5, cells[ci] % 5
                                ys = 4 * bi + dyi  # up row = o_l + 2 + (dyi-2)
                                rhs = up_r[:, ys:ys + RBLK, dxi:dxi + W]
                            nc.tensor.matmul(
                                ps[:],
                                lhsT=wb_t[:, ci, half, :],
                                rhs=rhs,
                                start=(ci == 0), stop=(ci == ncell - 1),
                            )
                        ob = outp.tile([COUT, RBLK, W], mybir.dt.float32, tag="ob")
                        nc.scalar.copy(out=ob[:], in_=ps[:])
                        nc.sync.dma_start(
                            out=out_d[128 * half:128 * (half + 1),
                                      4 * bi:4 * bi + RBLK, :],
                            in_=ob[:],
                        )

    nc.compile()
    return nc


def _stage_b_fp8r(nc, tc, up_r, upf_t, wb_t, wr_t, bigs, pairs,
                  psB, psR, outp, out_d):
    """Stage B with big cells in bf16 and ring-cell pairs in fp8 DoubleRow.

    upf_t[dx] holds a margin-free fp8 copy of up cols [dx, dx+128), so every
    cell window is a contiguous 512-element block and pair steps are
    automatically 16-aligned (multiples of 128)."""
    mm = mybir.AluOpType.mult
    aa = mybir.AluOpType.add

    # (fp8 casts of up are emitted inline with stage A, band by band)

    upf_fl = upf_t[:].rearrange("p a b c -> p (a b c)")

    def cell_off(c, bi):
        return (c % 5) * (UR * W) + ((4 * bi) + (c // 5)) * W

    G = 2  # blocks per weight-reuse group
    for half in range(2):
        for bg in range(OUT_R // RBLK // G):
            pscs = [psB.tile([COUT, RBLK, W], mybir.dt.float32, tag="psB",
                             name=f"psc_{half}_{bg}_{g}") for g in range(G)]
            for si, ci in enumerate(bigs):
                dyi, dxi = ci // 5, ci % 5
                for g in range(G):
                    bi = G * bg + g
                    ys = 4 * bi + dyi
                    nc.tensor.matmul(
                        pscs[g][:], lhsT=wb_t[:, si, half, :],
                        rhs=up_r[:, ys:ys + RBLK, dxi:dxi + W],
                        start=(si == 0), stop=(si == len(bigs) - 1))
            psrs = None
            if pairs:
                psrs = [psR.tile([COUT, RBLK, W], mybir.dt.float32, tag="psR",
                                 name=f"psr_{half}_{bg}_{g}") for g in range(G)]
                for p, (c1, c2) in enumerate(pairs):
                    step = cell_off(c2, 0) - cell_off(c1, 0)
                    assert step > 0 and step % 16 == 0
                    for g in range(G):
                        bi = G * bg + g
                        win = upf_fl[:, cell_off(c1, bi):cell_off(c1, bi) + RBLK * W]
                        rhs = bass.AP(tensor=win.tensor, offset=win.offset,
                                      ap=[win.ap[0], [step, 2], win.ap[1]])
                        nc.tensor.matmul(
                            psrs[g][:], lhsT=wr_t[:, p, half, :, :], rhs=rhs,
                            perf_mode=mybir.MatmulPerfMode.DoubleRow,
                            start=(p == 0), stop=(p == len(pairs) - 1))
            for g in range(G):
                bi = G * bg + g
                ob = outp.tile([COUT, RBLK, W], mybir.dt.float32, tag="ob")
                nc.scalar.copy(out=ob[:], in_=pscs[g][:])
                if pairs:
                    # TensorScalarPtr may read only one PSUM input
                    nc.vector.scalar_tensor_tensor(
                        out=ob[:], in0=psrs[g][:], scalar=1.0 / RING_SCALE,
                        in1=ob[:], op0=mm, op1=aa)
                nc.sync.dma_start(
                    out=out_d[128 * half:128 * (half + 1),
                              RBLK * bi:RBLK * (bi + 1), :],
                    in_=ob[:])


# --------------------------------------------------------------------------
# entry point
# --------------------------------------------------------------------------

def kernel(x, lateral_feat, trans_w, off_w1, off_b1, off_w2, off_b2):
    x = np.asarray(x)
    oy, ox = _offsets_from_inputs(np.asarray(lateral_feat), np.asarray(off_w1),
                                  np.asarray(off_b1), np.asarray(off_w2),
                                  np.asarray(off_b2))
    in_maps, ncell = _prep_in_maps(x, np.asarray(trans_w), oy, ox)

    key = (VARIANT, ncell)
    if key not in _CACHED_NC:
        _CACHED_NC[key] = _build_nc(ncell)
    nc = _CACHED_NC[key]

    res = run_bass_kernel_spmd(nc, in_maps, core_ids=list(range(N_CORES)))

    out = np.empty((N_BATCH, CIN, H, W), np.float32)
    for core in range(N_CORES):
        n, r = core // STRIPS, core % STRIPS
        out[n, :, OUT_R * r:OUT_R * (r + 1), :] = res.results[core]["out"]
    return out

